# revision 1
# baseline (speedup 1.0000x reference)
"""Multi-head attention (B=4, S=2048, D=1024, H=16, E=64) on 8 TRN2 NeuronCores.

Level-2 sharding: core c handles batch b=c//2 and HEAD-GROUP hg=c%2 (8 heads),
over the full 2048-token sequence — no duplicated K/V projection work. After
each pass the 2-head attT tile is exchanged between the batch's core pair with
a pair-group AllGather (2-core AllToAll is unsupported), so each core ends up
with all 16 heads' attention output for ITS token half (half hg) and runs the
output projection for those 1024 tokens. Each core's xT is supplied with its
own tokens first, which makes the peer's gathered chunk a program-uniform
column slice; the loopback duplicate is neutralized by zeroing its wo rows
host-side (the wo input is a per-core 12-block augmented layout).

Per-core program (SPMD):
  V projection (2 head-quads of 256 cols): V = x @ wv + bv, stored
    [tok, head, 65] with a ones-column per head (softmax sums fall out of the
    att@V matmul), spilled to DRAM scratch. Quad 1 is emitted after pass 0 so
    the scheduler uses it as PE filler inside the exp-bound inner loop.
  passes p=0..3 (local heads 2p, 2p+1):
    KT[128he, 2048tok] = (wk_p.T @ xT) + bk
    QT[128he, 2048tq]  = (wq_p.T @ xT) + bq     (full sequence of queries)
    per (head, tq-tile of 512): scoresT -> exp (ScalarE, scale=1/8) -> att@V
      with the softmax sum in PSUM row 64; normalize via reciprocal_approx
      (VectorE) + partition_broadcast (GpSimd) + multiply (VectorE)
    exchange: attT written in bf16, DMA'd to att_gin[p], AllGathered over the
    pair into att_gout[p] = both members' pass-p attT tiles
  output projection: out[1024tok, 1024] = att_all16 @ wo.T + bo in bf16
    (lhsT = 4 own bf16 attT tiles + 8 gathered peer-half tiles, rhs = bf16
    augmented wo with zeroed loopback blocks).

All attention/projection matmuls run in float32r except the bf16 epilogue.
"""

import numpy as np
import ml_dtypes

import concourse.bass as bass
import concourse.mybir as mybir
import concourse.tile as tile
from concourse import bacc
from concourse.bass_utils import run_bass_kernel_spmd

FP32 = mybir.dt.float32
FP32R = mybir.dt.float32r
BF16 = mybir.dt.bfloat16
AF = mybir.ActivationFunctionType

B, S, D, H, E = 4, 2048, 1024, 16, 64
NCORES = 8
TQ = S // 2  # tokens per core for the output projection
HL = H // 2  # local heads per core
SCALE = 1.0 / float(np.sqrt(E))
PAIRS = [[0, 1], [2, 3], [4, 5], [6, 7]]

_CACHE = {}


def build_nc():
    nc = bacc.Bacc("TRN2", target_bir_lowering=False)

    xT = nc.dram_tensor("xT", [D, S], FP32R, kind="ExternalInput")
    wq_t = nc.dram_tensor("wq_t", [D, HL * E], FP32R, kind="ExternalInput")
    wk_t = nc.dram_tensor("wk_t", [D, HL * E], FP32R, kind="ExternalInput")
    wv_t = nc.dram_tensor("wv_t", [D, HL * E], FP32R, kind="ExternalInput")
    wo_b = nc.dram_tensor("wo_b", [12 * 128, D], BF16, kind="ExternalInput")
    bqp = nc.dram_tensor("bqp", [128, 4], FP32, kind="ExternalInput")
    bkp = nc.dram_tensor("bkp", [128, 4], FP32, kind="ExternalInput")
    bv_row = nc.dram_tensor("bv_row", [1, HL * E], FP32R, kind="ExternalInput")
    bo_row = nc.dram_tensor("bo_row", [1, D], FP32R, kind="ExternalInput")
    out = nc.dram_tensor("out", [TQ, D], FP32, kind="ExternalOutput")
    # V spill: [tok-tile, tok-in-tile, local head, E+1]
    v_spill = nc.dram_tensor("v_spill", [16, 128, HL, E + 1], BF16)
    att_gin = nc.dram_tensor("att_gin", [4, 128, TQ], BF16)
    att_gout = nc.dram_tensor("att_gout", [4, 2, 128, TQ], BF16)

    xT_r = xT.rearrange("(t p) s -> p t s", p=128)  # [128, 8, 2048]
    wq_r = wq_t.rearrange("(t p) m -> p t m", p=128)  # [128, 8, 512]
    wk_r = wk_t.rearrange("(t p) m -> p t m", p=128)
    wv_r = wv_t.rearrange("(t p) m -> p t m", p=128)
    wo_r = wo_b.rearrange("(t p) m -> p t m", p=128)  # [128, 12, 1024]

    from contextlib import ExitStack

    with tile.TileContext(nc) as tc:
        with ExitStack() as _es:
            xt_pool = _es.enter_context(tc.tile_pool(name="xt", bufs=1))
            wkq_pool = _es.enter_context(tc.tile_pool(name="wkq", bufs=2))
            kt_pool = _es.enter_context(tc.tile_pool(name="ktp", bufs=2))
            w256_pool = _es.enter_context(tc.tile_pool(name="w256", bufs=4))
            qt_pool = _es.enter_context(tc.tile_pool(name="qt", bufs=2))
            vbuf_pool = _es.enter_context(tc.tile_pool(name="vbuf", bufs=2))
            vst_pool = _es.enter_context(tc.tile_pool(name="vst", bufs=1))
            exp_pool = _es.enter_context(tc.tile_pool(name="expp", bufs=4))
            attw_pool = _es.enter_context(tc.tile_pool(name="attw", bufs=2))
            own_pool = _es.enter_context(tc.tile_pool(name="own", bufs=4))
            recv_pool = _es.enter_context(tc.tile_pool(name="recv", bufs=8))
            small_pool = _es.enter_context(tc.tile_pool(name="small", bufs=2))
            ones_pool = _es.enter_context(tc.tile_pool(name="ones", bufs=1))
            ps_scores = _es.enter_context(tc.tile_pool(name="ps_s", bufs=2, space="PSUM"))
            ps_att = _es.enter_context(tc.tile_pool(name="ps_a", bufs=2, space="PSUM"))
            ps_gen = _es.enter_context(tc.tile_pool(name="ps_g", bufs=2, space="PSUM"))

            # ---- persistent tiles ----
            xt_sb = xt_pool.tile([128, 8, S], FP32R, tag="xt")  # 64KB/part
            for k in range(8):
                nc.sync.dma_start(out=xt_sb[:, k, :], in_=xT_r[:, k, :])

            ones_col_f = ones_pool.tile([128, 4], FP32, tag="onescf")
            nc.vector.memset(ones_col_f, 1.0)
            ones_col = ones_pool.tile([128, 4], BF16, tag="onescol")
            nc.vector.tensor_copy(out=ones_col, in_=ones_col_f)
            bq_sb = ones_pool.tile([128, 4], FP32, tag="bq")
            bk_sb = ones_pool.tile([128, 4], FP32, tag="bk")
            nc.sync.dma_start(out=bq_sb, in_=bqp[:, :])
            nc.sync.dma_start(out=bk_sb, in_=bkp[:, :])

            bv_sb = w256_pool.tile([1, HL * E], FP32R, tag="w256", name="bvrow")
            bo_sb = w256_pool.tile([1, D], FP32R, tag="w256", name="borow")
            nc.sync.dma_start(out=bv_sb, in_=bv_row[:, :])
            nc.sync.dma_start(out=bo_sb, in_=bo_row[:, :])
            bv_bc = ones_pool.tile([128, HL * E], FP32R, tag="bvbc")
            bo_bc = ones_pool.tile([128, D], FP32R, tag="bobc")
            nc.gpsimd.partition_broadcast(bv_bc, bv_sb)
            nc.gpsimd.partition_broadcast(bo_bc, bo_sb)

            # ---- V projection for one head-quad (4 heads, 256 cols) ----
            def v_quad(vh, tokts=range(16), wv_cache={}):
                if vh in wv_cache:
                    wv_sb = wv_cache[vh]
                else:
                    wv_sb = w256_pool.tile([128, 8, 256], FP32R, tag="w256")
                    nc.sync.dma_start(
                        out=wv_sb, in_=wv_r[:, :, vh * 256 : (vh + 1) * 256]
                    )
                    wv_cache[vh] = wv_sb
                for tokt in tokts:
                    ps = ps_gen.tile([128, 256], FP32, tag="gen")
                    for k in range(8):
                        nc.tensor.matmul(
                            out=ps,
                            lhsT=xt_sb[:, k, tokt * 128 : (tokt + 1) * 128],
                            rhs=wv_sb[:, k, :],
                            start=(k == 0),
                            stop=(k == 7),
                        )
                    vstage = vst_pool.tile([128, 4, E + 1], BF16, tag="vst")
                    nc.vector.tensor_add(
                        out=vstage[:, :, :E],
                        in0=ps.rearrange("p (h e) -> p h e", e=E),
                        in1=bv_bc[:, vh * 256 : (vh + 1) * 256].rearrange(
                            "p (h e) -> p h e", e=E
                        ),
                    )
                    nc.vector.tensor_copy(
                        out=vstage[:, :, E : E + 1], in_=ones_col[:, :4].unsqueeze(2)
                    )
                    nc.sync.dma_start(
                        out=v_spill[tokt, :, vh * 4 : (vh + 1) * 4, :], in_=vstage
                    )

            v_quad(0)

            own_tiles = []

            # ---- passes: 2 local heads each ----
            for p in range(4):
                wk_sb = wkq_pool.tile([128, 8, 128], FP32R, tag="wk")
                wq_sb = wkq_pool.tile([128, 8, 128], FP32R, tag="wq")
                nc.sync.dma_start(out=wk_sb, in_=wk_r[:, :, p * 128 : (p + 1) * 128])
                nc.sync.dma_start(out=wq_sb, in_=wq_r[:, :, p * 128 : (p + 1) * 128])

                kt_sb = kt_pool.tile([128, S], FP32R, tag="kt")
                qt_sb = qt_pool.tile([128, S], FP32R, tag="qt")

                for ts in range(4):
                    ps = ps_gen.tile([128, 512], FP32, tag="gen")
                    for k in range(8):
                        nc.tensor.matmul(
                            out=ps,
                            lhsT=wk_sb[:, k, :],
                            rhs=xt_sb[:, k, ts * 512 : (ts + 1) * 512],
                            start=(k == 0),
                            stop=(k == 7),
                        )
                    nc.vector.tensor_scalar_add(
                        out=kt_sb[:, ts * 512 : (ts + 1) * 512],
                        in0=ps,
                        scalar1=bk_sb[:, p : p + 1],
                    )
                for qs in range(4):
                    ps = ps_gen.tile([128, 512], FP32, tag="gen")
                    for k in range(8):
                        nc.tensor.matmul(
                            out=ps,
                            lhsT=wq_sb[:, k, :],
                            rhs=xt_sb[:, k, qs * 512 : (qs + 1) * 512],
                            start=(k == 0),
                            stop=(k == 7),
                        )
                    nc.vector.tensor_scalar_add(
                        out=qt_sb[:, qs * 512 : (qs + 1) * 512],
                        in0=ps,
                        scalar1=bq_sb[:, p : p + 1],
                    )

                attw = attw_pool.tile([128, S], BF16, tag="attw")
                for hh in range(2):
                    base = hh * 64
                    h = 2 * p + hh
                    vh_sb = vbuf_pool.tile([128, 16, E + 1], BF16, tag="vbuf")
                    nc.sync.dma_start(
                        out=vh_sb, in_=v_spill[:, :, h, :].transpose([1, 0, 2])
                    )
                    for tqt in range(4):
                        att_ps = ps_att.tile([E + 1, 512], FP32, tag="att")

                        def att_group(gg, exp_t):
                            for j in range(2):
                                t = gg * 2 + j
                                nc.tensor.matmul(
                                    out=att_ps,
                                    lhsT=vh_sb[:, t, :],
                                    rhs=exp_t[:, j, :],
                                    start=(t == 0),
                                    stop=(t == 15),
                                )

                        exp_tiles = []
                        for g in range(8):
                            ps_s = ps_scores.tile([128, 2, 512], FP32, tag="sc")
                            for j in range(2):
                                t = g * 2 + j
                                nc.tensor.matmul(
                                    out=ps_s[:, j, :],
                                    lhsT=kt_sb[
                                        base : base + 64, t * 128 : (t + 1) * 128
                                    ],
                                    rhs=qt_sb[
                                        base : base + 64, tqt * 512 : (tqt + 1) * 512
                                    ],
                                    start=True,
                                    stop=True,
                                )
                            exp_t = exp_pool.tile([128, 2, 512], BF16, tag="exp")
                            nc.scalar.activation(
                                out=exp_t, in_=ps_s, func=AF.Exp, scale=SCALE
                            )
                            exp_tiles.append(exp_t)
                            if g >= 2:
                                att_group(g - 2, exp_tiles[g - 2])
                        att_group(6, exp_tiles[6])
                        att_group(7, exp_tiles[7])
                        sums_sb = small_pool.tile([1, 512], FP32, tag="sums", bufs=1)
                        nc.vector.tensor_copy(out=sums_sb, in_=att_ps[E : E + 1, :])
                        recip_r = small_pool.tile([1, 512], FP32, tag="recr", bufs=1)
                        recip_s = small_pool.tile([1, 512], FP32, tag="recs", bufs=1)
                        nc.vector.reciprocal_approx_accurate(
                            out=recip_r, in_=sums_sb, scratch=recip_s
                        )
                        rb_sb = small_pool.tile([64, 512], FP32, tag="rbb", bufs=2)
                        nc.gpsimd.partition_broadcast(rb_sb, recip_r)
                        nc.vector.tensor_mul(
                            out=attw[base : base + 64, tqt * 512 : (tqt + 1) * 512],
                            in0=att_ps[:E, :],
                            in1=rb_sb,
                        )

                # pair exchange of this pass's 2-head attT (bf16): AllGather
                # the full tile (2-core AllToAll is unsupported); the epilogue
                # reads BOTH gathered chunks and the host zeroes the wo rows
                # of the loopback duplicate
                own_t = own_pool.tile([128, TQ], BF16, tag="own", name=f"own{p}")
                nc.vector.tensor_copy(out=own_t, in_=attw[:, 0:TQ])
                own_tiles.append(own_t)
                # only the peer-token half needs to cross cores: the own
                # half is consumed from own_t, and the loopback chunk's wo
                # rows are zero anyway
                nc.sync.dma_start(out=att_gin[p], in_=attw[:, TQ:S])
                nc.gpsimd.collective_compute(
                    kind="AllGather",
                    op=mybir.AluOpType.bypass,
                    replica_groups=PAIRS,
                    ins=[att_gin[p]],
                    outs=[att_gout[p]],
                )

                if p == 0:
                    v_quad(1, range(8))
                elif p == 1:
                    v_quad(1, range(8, 16))

            # ---- output projection over all 16 heads, my token half ----
            # lhsT tiles: own passes 0-3, then gathered chunks (p, g); the
            # peer's chunk holds its attT for MY tokens (cols TQ:2S with the
            # own-first local token order); the loopback chunk's wo rows are
            # zeroed host-side
            lhs_tiles = list(own_tiles)
            for p in range(4):
                for g in range(2):
                    rt = recv_pool.tile([128, TQ], BF16, tag="recv", name=f"rc{p}{g}")
                    nc.sync.dma_start(out=rt, in_=att_gout[p, g])
                    lhs_tiles.append(rt)

            wo_sbs = []
            for ohalf in range(4):
                wo_sb = w256_pool.tile(
                    [128, 12, 256], BF16, tag="w256", name=f"wo{ohalf}"
                )
                nc.sync.dma_start(
                    out=wo_sb, in_=wo_r[:, :, ohalf * 256 : (ohalf + 1) * 256]
                )
                wo_sbs.append(wo_sb)
            for ohalf in range(4):
                wo_sb = wo_sbs[ohalf]
                for tokt in range(8):
                    ps = ps_gen.tile([128, 256], FP32, tag="gen")
                    for t in range(12):
                        nc.tensor.matmul(
                            out=ps,
                            lhsT=lhs_tiles[t][:, tokt * 128 : (tokt + 1) * 128],
                            rhs=wo_sb[:, t, :],
                            start=(t == 0),
                            stop=(t == 11),
                        )
                    ostg = small_pool.tile([128, 256], FP32, tag="stg", bufs=2)
                    nc.vector.tensor_add(
                        out=ostg, in0=ps, in1=bo_bc[:, ohalf * 256 : (ohalf + 1) * 256]
                    )
                    nc.sync.dma_start(
                        out=out[
                            tokt * 128 : (tokt + 1) * 128,
                            ohalf * 256 : (ohalf + 1) * 256,
                        ],
                        in_=ostg,
                    )

    nc.compile()
    return nc


def kernel(x, wq, bq, wk, bk, wv, bv, wo, bo, trace=False):
    x = np.asarray(x, dtype=np.float32)
    wq = np.asarray(wq, dtype=np.float32)
    bq = np.asarray(bq, dtype=np.float32)
    wk = np.asarray(wk, dtype=np.float32)
    bk = np.asarray(bk, dtype=np.float32)
    wv = np.asarray(wv, dtype=np.float32)
    bv = np.asarray(bv, dtype=np.float32)
    wo = np.asarray(wo, dtype=np.float32)
    bo = np.asarray(bo, dtype=np.float32)

    if "nc" not in _CACHE:
        _CACHE["nc"] = build_nc()
    nc = _CACHE["nc"]

    wq_f = wq.transpose(1, 0, 2).reshape(D, H * E)  # [D, heads*E] head-major cols
    wk_f = wk.transpose(1, 0, 2).reshape(D, H * E)
    wv_f = wv.transpose(1, 0, 2).reshape(D, H * E)
    wo_t = wo.T  # [in 1024, out 1024], in-dim = global head-major
    bo_row = np.ascontiguousarray(bo.reshape(1, D))

    in_maps = []
    for c in range(NCORES):
        b, hg = c // 2, c % 2
        cs = slice(hg * HL * E, (hg + 1) * HL * E)
        xT_b = x[b].T
        if hg == 0:
            xT_c = np.ascontiguousarray(xT_b)
        else:
            # own out-tokens first; K/V/Q all share this local token order
            xT_c = np.ascontiguousarray(
                np.concatenate([xT_b[:, TQ:], xT_b[:, :TQ]], axis=1)
            )
        # augmented wo: 4 own head-pair blocks, then (p, g) gathered blocks
        # with the loopback (g == hg) blocks zeroed
        wo_aug = np.zeros((12 * 128, D), dtype=np.float32)
        for p_ in range(4):
            wo_aug[p_ * 128 : (p_ + 1) * 128] = wo_t[
                hg * 512 + p_ * 128 : hg * 512 + (p_ + 1) * 128
            ]
        for p_ in range(4):
            for g_ in range(2):
                if g_ == hg:
                    continue
                t_ = 4 + p_ * 2 + g_
                wo_aug[t_ * 128 : (t_ + 1) * 128] = wo_t[
                    g_ * 512 + p_ * 128 : g_ * 512 + (p_ + 1) * 128
                ]
        m = {
            "xT": xT_c,
            "wq_t": np.ascontiguousarray(wq_f[:, cs]),
            "wk_t": np.ascontiguousarray(wk_f[:, cs]),
            "wv_t": np.ascontiguousarray(wv_f[:, cs]),
            "wo_b": np.ascontiguousarray(wo_aug).astype(ml_dtypes.bfloat16),
            "bqp": np.ascontiguousarray(
                bq.reshape(H * E)[cs].reshape(4, 128).T
            ),
            "bkp": np.ascontiguousarray(
                bk.reshape(H * E)[cs].reshape(4, 128).T
            ),
            "bv_row": np.ascontiguousarray(bv.reshape(1, H * E)[:, cs]),
            "bo_row": bo_row,
        }
        in_maps.append(m)

    res = run_bass_kernel_spmd(nc, in_maps, list(range(NCORES)), trace=trace)

    out = np.empty((B, S, D), dtype=np.float32)
    for c in range(NCORES):
        b, hg = c // 2, c % 2
        out[b, hg * TQ : (hg + 1) * TQ, :] = res.results[c]["out"]
    if trace:
        return out, res
    return out



# revision 9
# speedup vs baseline: 1.3214x; 1.3214x over previous
"""Multi-head attention (B=4, S=2048, D=1024, H=16, E=64) on 8 TRN2 NeuronCores.

Sharding: core c handles batch b=c//2 and head-group hg=c%2 (8 heads) over the
full 2048-token sequence; after each 2-head pass the peer-token half of the
attention output is exchanged pairwise (AllGather over [2c,2c+1]) and each
core runs the output projection for its own 1024 tokens over all 16 heads
(augmented wo layout with zeroed loopback blocks, as the program is SPMD).

Kernel structure (v2 — uniform 128x128 PE mode, exp/PE balanced pipeline):
  * All matmuls use the full 128-row PE config; the per-head scores matmuls
    (contraction = E = 64) are padded to 128 contraction rows via a
    zero-padded Q layout qt[128, 2, S]: strip 0 holds head A's Q in rows
    0-63 (rows 64-127 zero), strip 1 head B's in rows 64-127.  lhsT is the
    shared kt[:, kt-tile] so K rows of the other head multiply zeros.  This
    avoids the (64,128)<->(128,128) tiling-mode drains the PE would pay when
    interleaving scores and att@V matmuls.
  * Inner loop per (pass, tqt of 512 queries, kt of 128 keys): 2 scores
    matmuls (one per head, adjacent PSUM banks), one exp ACTIVATE over
    [128, 2, 512] (N=1024), 2 att@V accumulation matmuls (M=65: V plus a
    ones column that yields the softmax sums in PSUM row 64).
  * V is computed once (fp32r) and kept resident in SBUF as bf16
    [128 tok-part, 16 tok-tile, 4 head, 65] (two quads) — no DRAM spill.
  * K/Q are stored bf16 (scores matmul in bf16; PSUM accumulates fp32).
  * Softmax normalize: fast PSUM evacuation (sums row + 64 att rows copied
    to SBUF by DVE to free the bank), reciprocal_approx_fast, GpSimd
    partition broadcast, DVE multiply straight into the own-half tile or the
    exchange staging tile.
  * A filler queue interleaves the V/K/Q projection matmuls (and late DMAs)
    into the exp-bound attention loop at ~2 instructions per kt so the PE
    never idles (keeps the tensor-engine DVFS p-state at max clock).
"""

import numpy as np
import ml_dtypes

import concourse.bass as bass
import concourse.mybir as mybir
import concourse.tile as tile
from concourse import bacc
from concourse.bass_utils import run_bass_kernel_spmd

FP32 = mybir.dt.float32
FP32R = mybir.dt.float32r
BF16 = mybir.dt.bfloat16
AF = mybir.ActivationFunctionType

B, S, D, H, E = 4, 2048, 1024, 16, 64
NCORES = 8
TQ = S // 2  # tokens per core for the output projection
HL = H // 2  # local heads per core
SCALE = 1.0 / float(np.sqrt(E))
PAIRS = [[0, 1], [2, 3], [4, 5], [6, 7]]

_CACHE = {}


def build_nc():
    nc = bacc.Bacc("TRN2", target_bir_lowering=False)

    xT = nc.dram_tensor("xT", [D, S], FP32R, kind="ExternalInput")
    wq_t = nc.dram_tensor("wq_t", [D, HL * E], FP32R, kind="ExternalInput")
    wk_t = nc.dram_tensor("wk_t", [D, HL * E], FP32R, kind="ExternalInput")
    wv_t = nc.dram_tensor("wv_t", [D, HL * E], FP32R, kind="ExternalInput")
    wo_b = nc.dram_tensor("wo_b", [12 * 128, D], BF16, kind="ExternalInput")
    bqp = nc.dram_tensor("bqp", [128, 4], FP32, kind="ExternalInput")
    bkp = nc.dram_tensor("bkp", [128, 4], FP32, kind="ExternalInput")
    bv_row = nc.dram_tensor("bv_row", [1, HL * E], FP32R, kind="ExternalInput")
    bo_row = nc.dram_tensor("bo_row", [1, D], FP32R, kind="ExternalInput")
    out = nc.dram_tensor("out", [TQ, D], FP32, kind="ExternalOutput")
    att_gin = nc.dram_tensor("att_gin", [4, 128, TQ], BF16)
    att_gout = nc.dram_tensor("att_gout", [4, 2, 128, TQ], BF16)

    xT_r = xT.rearrange("(t p) s -> p t s", p=128)  # [128, 8, 2048]
    wq_r = wq_t.rearrange("(t p) m -> p t m", p=128)  # [128, 8, 512]
    wk_r = wk_t.rearrange("(t p) m -> p t m", p=128)
    wv_r = wv_t.rearrange("(t p) m -> p t m", p=128)
    wo_r = wo_b.rearrange("(t p) m -> p t m", p=128)  # [128, 12, 1024]

    from contextlib import ExitStack

    with tile.TileContext(nc) as tc:
        with ExitStack() as _es:
            xt_pool = _es.enter_context(tc.tile_pool(name="xt", bufs=1))
            qt_pool = _es.enter_context(tc.tile_pool(name="qtp", bufs=2))
            kt_pool = _es.enter_context(tc.tile_pool(name="ktp", bufs=2))
            v_pool = _es.enter_context(tc.tile_pool(name="vp", bufs=2))
            wkq_pool = _es.enter_context(tc.tile_pool(name="wkq", bufs=2))
            w_pool = _es.enter_context(tc.tile_pool(name="wp", bufs=4))
            exp_pool = _es.enter_context(tc.tile_pool(name="expp", bufs=3))
            own_pool = _es.enter_context(tc.tile_pool(name="own", bufs=4))
            gin_pool = _es.enter_context(tc.tile_pool(name="gin", bufs=2))
            recv_pool = _es.enter_context(tc.tile_pool(name="recv", bufs=8))
            stage_pool = _es.enter_context(tc.tile_pool(name="astg", bufs=2))
            ostg_pool = _es.enter_context(tc.tile_pool(name="ostg", bufs=2))
            small_pool = _es.enter_context(tc.tile_pool(name="small", bufs=2))
            rb_pool = _es.enter_context(tc.tile_pool(name="rbp", bufs=2))
            ones_pool = _es.enter_context(tc.tile_pool(name="ones", bufs=1))
            ps_sc = _es.enter_context(tc.tile_pool(name="ps_s", bufs=2, space="PSUM"))
            ps_att = _es.enter_context(tc.tile_pool(name="ps_a", bufs=2, space="PSUM"))
            ps_gen = _es.enter_context(tc.tile_pool(name="ps_g", bufs=2, space="PSUM"))

            # ---- persistent tiles ----
            xt_sb = xt_pool.tile([128, 8, S], FP32R, tag="xt")  # 64KB/part
            for k in range(8):
                nc.sync.dma_start(out=xt_sb[:, k, :], in_=xT_r[:, k, :])

            ones_col_f = ones_pool.tile([128, 4], FP32, tag="onescf")
            nc.vector.memset(ones_col_f, 1.0)
            ones_col = ones_pool.tile([128, 4], BF16, tag="onescol")
            nc.vector.tensor_copy(out=ones_col, in_=ones_col_f)
            bq_sb = ones_pool.tile([128, 4], FP32, tag="bq")
            bk_sb = ones_pool.tile([128, 4], FP32, tag="bk")
            nc.sync.dma_start(out=bq_sb, in_=bqp[:, :])
            nc.sync.dma_start(out=bk_sb, in_=bkp[:, :])

            bv_bc = ones_pool.tile([128, HL * E], FP32R, tag="bvbc")
            bo_bc = ones_pool.tile([128, D], FP32R, tag="bobc")
            nc.sync.dma_start(
                out=bv_bc, in_=bv_row[:, :].to_broadcast([128, HL * E])
            )
            nc.sync.dma_start(out=bo_bc, in_=bo_row[:, :].to_broadcast([128, D]))

            # V resident in SBUF: quad q holds heads 4q..4q+3:
            # [tok-in-tile(part), tok-tile, head, E+1]
            v_tiles = [
                v_pool.tile([128, 16, 4, E + 1], BF16, tag="vsb", name=f"v{q}")
                for q in range(2)
            ]
            # zero-padded Q, ping-pong across passes: strip 0 = head A rows
            # 0-63 (rest zero), strip 1 = head B rows 64-127 (rest zero)
            qt_tiles = [
                qt_pool.tile([128, 2, S], BF16, tag="qt", name=f"qt{i}")
                for i in range(2)
            ]
            for i in range(2):
                nc.vector.memset(qt_tiles[i][64:128, 0, :], 0.0)
                nc.vector.memset(qt_tiles[i][0:64, 1, :], 0.0)

            # ---- filler queue ----
            fill_q = []

            def step(n=2):
                if len(fill_q) > 120:
                    n += 1
                for _ in range(n):
                    if fill_q:
                        fill_q.pop(0)()

            def flush():
                while fill_q:
                    fill_q.pop(0)()

            done_marks = set()

            def mark(tag):
                def m():
                    done_marks.add(tag)

                return [m]

            def ensure(tag):
                while tag not in done_marks and fill_q:
                    fill_q.pop(0)()

            # ---- projection group emitters (closure lists) ----
            wv_sbs = {}

            def wv_dma(q):
                def go():
                    wv_sbs[q] = w_pool.tile(
                        [128, 8, 256], FP32R, tag="wp", name=f"wv{q}"
                    )
                    nc.sync.dma_start(
                        out=wv_sbs[q], in_=wv_r[:, :, q * 256 : (q + 1) * 256]
                    )

                return [go]

            def v_group(q, tokt):
                cell = {}

                def mm(k):
                    if k == 0:
                        cell["ps"] = ps_gen.tile(
                            [128, 256], FP32, tag="gen", name=f"vps{q}_{tokt}"
                        )
                    nc.tensor.matmul(
                        out=cell["ps"],
                        lhsT=xt_sb[:, k, tokt * 128 : (tokt + 1) * 128],
                        rhs=wv_sbs[q][:, k, :],
                        start=(k == 0),
                        stop=(k == 7),
                    )

                def fin():
                    ps = cell["ps"]
                    nc.vector.tensor_add(
                        out=v_tiles[q][:, tokt, :, :E],
                        in0=ps.rearrange("p (h e) -> p h e", e=E),
                        in1=bv_bc[:, q * 256 : (q + 1) * 256].rearrange(
                            "p (h e) -> p h e", e=E
                        ),
                    )
                    nc.vector.tensor_copy(
                        out=v_tiles[q][:, tokt, :, E : E + 1],
                        in_=ones_col[:, :4].unsqueeze(2),
                    )

                return (
                    [lambda k=k: mm(k) for k in range(8)]
                    + [fin]
                    + mark((f"v{q}", tokt))
                )

            wkq_sbs = {}

            def wkq_dma(p):
                def go():
                    wk_sb = wkq_pool.tile(
                        [128, 8, 128], FP32R, tag="wk", name=f"wk{p}"
                    )
                    wq_sb = wkq_pool.tile(
                        [128, 8, 128], FP32R, tag="wq", name=f"wq{p}"
                    )
                    nc.sync.dma_start(
                        out=wk_sb, in_=wk_r[:, :, p * 128 : (p + 1) * 128]
                    )
                    nc.sync.dma_start(
                        out=wq_sb, in_=wq_r[:, :, p * 128 : (p + 1) * 128]
                    )
                    wkq_sbs[p] = (wk_sb, wq_sb)

                return [go]

            kt_sbs = {}

            def kt_alloc(p):
                def go():
                    kt_sbs[p] = kt_pool.tile([128, S], BF16, tag="kt", name=f"kt{p}")

                return [go]

            def k_group(p, ts):
                cell = {}

                def mm(k):
                    if k == 0:
                        cell["ps"] = ps_gen.tile(
                            [128, 512], FP32, tag="gen", name=f"kps{p}_{ts}"
                        )
                    nc.tensor.matmul(
                        out=cell["ps"],
                        lhsT=wkq_sbs[p][0][:, k, :],
                        rhs=xt_sb[:, k, ts * 512 : (ts + 1) * 512],
                        start=(k == 0),
                        stop=(k == 7),
                    )

                def fin():
                    nc.vector.tensor_scalar_add(
                        out=kt_sbs[p][:, ts * 512 : (ts + 1) * 512],
                        in0=cell["ps"],
                        scalar1=bk_sb[:, p : p + 1],
                    )

                return (
                    [lambda k=k: mm(k) for k in range(8)]
                    + [fin]
                    + mark(("k", p, ts))
                )

            def q_group(p, qs):
                cell = {}
                qt_t = qt_tiles[p % 2]

                def mm(k):
                    if k == 0:
                        cell["ps"] = ps_gen.tile(
                            [128, 512], FP32, tag="gen", name=f"qps{p}_{qs}"
                        )
                    nc.tensor.matmul(
                        out=cell["ps"],
                        lhsT=wkq_sbs[p][1][:, k, :],
                        rhs=xt_sb[:, k, qs * 512 : (qs + 1) * 512],
                        start=(k == 0),
                        stop=(k == 7),
                    )

                def fin():
                    ps = cell["ps"]
                    nc.vector.tensor_scalar_add(
                        out=qt_t[0:64, 0, qs * 512 : (qs + 1) * 512],
                        in0=ps[0:64, :],
                        scalar1=bq_sb[0:64, p : p + 1],
                    )
                    nc.vector.tensor_scalar_add(
                        out=qt_t[64:128, 1, qs * 512 : (qs + 1) * 512],
                        in0=ps[64:128, :],
                        scalar1=bq_sb[64:128, p : p + 1],
                    )

                return (
                    [lambda k=k: mm(k) for k in range(8)]
                    + [fin]
                    + mark(("q", p, qs))
                )

            # ---- upfront: weights + pass-0 K/Q + V quad 0 ----
            for cl in (
                wv_dma(0)
                + wkq_dma(0)
                + kt_alloc(0)
                + [c for ts in range(4) for c in k_group(0, ts)]
                + [c for c_ in [0] for c in q_group(0, 0)]
                + [c for t in range(8) for c in v_group(0, t)]
            ):
                cl()

            # filler for pass 0: rest of pass-0 Q and V quad 0, V quad 1,
            # pass-1 K/Q.  ensure() marks make consumers wait for these.
            fill_q.extend(
                [c for qs in range(1, 4) for c in q_group(0, qs)]
                + [c for t in range(8, 16) for c in v_group(0, t)]
                + wv_dma(1)
                + wkq_dma(1)
                + kt_alloc(1)
                + [c for ts in range(4) for c in k_group(1, ts)]
                + [c for qs in range(4) for c in q_group(1, qs)]
                + [c for t in range(16) for c in v_group(1, t)]
            )

            own_tiles = []
            wo_sbs = {}
            recv_tiles = {}

            def wo_dma(oh):
                def go():
                    wo_sbs[oh] = w_pool.tile(
                        [128, 12, 256], BF16, tag="wp", name=f"wo{oh}"
                    )
                    nc.sync.dma_start(
                        out=wo_sbs[oh], in_=wo_r[:, :, oh * 256 : (oh + 1) * 256]
                    )

                return [go]

            def recv_dma(p, g):
                def go():
                    rt = recv_pool.tile([128, TQ], BF16, tag="recv", name=f"rc{p}{g}")
                    nc.sync.dma_start(out=rt, in_=att_gout[p, g])
                    recv_tiles[(p, g)] = rt

                return [go]

            # ---- attention passes ----
            for p in range(4):
                qt_t = qt_tiles[p % 2]
                kt_t = kt_sbs[p]
                vq = 0 if p < 2 else 1
                vA, vB = 2 * (p % 2), 2 * (p % 2) + 1
                v_t = v_tiles[vq]

                own_t = own_pool.tile([128, TQ], BF16, tag="own", name=f"own{p}")
                own_tiles.append(own_t)
                gin_t = gin_pool.tile([128, TQ], BF16, tag="gin", name=f"gin{p}")

                for tqt in range(4):
                    ensure(("q", p, tqt))
                    att_A = ps_att.tile([E + 1, 512], FP32, tag="att")
                    att_B = ps_att.tile([E + 1, 512], FP32, tag="att")
                    for kt in range(16):
                        ensure(("k", p, kt // 4))
                        ensure((f"v{vq}", kt))
                        ps = ps_sc.tile([128, 2, 512], FP32, tag="sc")
                        nc.tensor.matmul(
                            out=ps[:, 0, :],
                            lhsT=kt_t[:, kt * 128 : (kt + 1) * 128],
                            rhs=qt_t[:, 0, tqt * 512 : (tqt + 1) * 512],
                            start=True,
                            stop=True,
                        )
                        nc.tensor.matmul(
                            out=ps[:, 1, :],
                            lhsT=kt_t[:, kt * 128 : (kt + 1) * 128],
                            rhs=qt_t[:, 1, tqt * 512 : (tqt + 1) * 512],
                            start=True,
                            stop=True,
                        )
                        ex = exp_pool.tile([128, 2, 512], BF16, tag="exp")
                        nc.scalar.activation(out=ex, in_=ps, func=AF.Exp, scale=SCALE)
                        nc.tensor.matmul(
                            out=att_A,
                            lhsT=v_t[:, kt, vA, :],
                            rhs=ex[:, 0, :],
                            start=(kt == 0),
                            stop=(kt == 15),
                        )
                        nc.tensor.matmul(
                            out=att_B,
                            lhsT=v_t[:, kt, vB, :],
                            rhs=ex[:, 1, :],
                            start=(kt == 0),
                            stop=(kt == 15),
                        )
                        step(2)

                    for hh, att_ps in ((0, att_A), (1, att_B)):
                        # fast PSUM evacuation, then normalize from SBUF
                        sums = small_pool.tile([1, 512], FP32, tag="sums", bufs=1)
                        nc.vector.tensor_copy(out=sums, in_=att_ps[E : E + 1, :])
                        a_sb = stage_pool.tile([64, 512], FP32, tag="astg")
                        nc.vector.tensor_copy(out=a_sb, in_=att_ps[:E, :])
                        recip = small_pool.tile([1, 512], FP32, tag="recip", bufs=1)
                        nc.vector.reciprocal_approx_fast(out=recip, in_=sums)
                        rb = rb_pool.tile([64, 512], FP32, tag="rbb")
                        nc.gpsimd.partition_broadcast(rb, recip)
                        if tqt < 2:
                            dest = own_t[
                                hh * 64 : (hh + 1) * 64,
                                tqt * 512 : (tqt + 1) * 512,
                            ]
                        else:
                            dest = gin_t[
                                hh * 64 : (hh + 1) * 64,
                                (tqt - 2) * 512 : (tqt - 1) * 512,
                            ]
                        nc.vector.tensor_mul(out=dest, in0=a_sb, in1=rb)

                # pair exchange of this pass's peer-token half (bf16)
                nc.sync.dma_start(out=att_gin[p], in_=gin_t)
                nc.gpsimd.collective_compute(
                    kind="AllGather",
                    op=mybir.AluOpType.bypass,
                    replica_groups=PAIRS,
                    ins=[att_gin[p]],
                    outs=[att_gout[p]],
                )

                if p == 0:
                    fill_q.extend(
                        wkq_dma(2)
                        + kt_alloc(2)
                        + [c for ts in range(4) for c in k_group(2, ts)]
                        + [c for qs in range(4) for c in q_group(2, qs)]
                    )
                elif p == 1:
                    fill_q.extend(
                        wkq_dma(3)
                        + kt_alloc(3)
                        + [c for ts in range(4) for c in k_group(3, ts)]
                        + [c for qs in range(4) for c in q_group(3, qs)]
                        + recv_dma(0, 0)
                        + recv_dma(0, 1)
                    )
                elif p == 2:
                    fill_q.extend(
                        recv_dma(1, 0)
                        + recv_dma(1, 1)
                        + [c for oh in range(4) for c in wo_dma(oh)]
                    )
                elif p == 3:
                    fill_q.extend(recv_dma(2, 0) + recv_dma(2, 1))

            flush()
            for cl in recv_dma(3, 0) + recv_dma(3, 1):
                cl()

            # ---- output projection over all 16 heads, my token half ----
            lhs_tiles = list(own_tiles) + [
                recv_tiles[(p, g)] for p in range(4) for g in range(2)
            ]
            for oh in range(4):
                wo_sb = wo_sbs[oh]
                for tokt in range(8):
                    ps = ps_gen.tile([128, 256], FP32, tag="gen")
                    for t in range(12):
                        nc.tensor.matmul(
                            out=ps,
                            lhsT=lhs_tiles[t][:, tokt * 128 : (tokt + 1) * 128],
                            rhs=wo_sb[:, t, :],
                            start=(t == 0),
                            stop=(t == 11),
                        )
                    ostg = ostg_pool.tile([128, 256], FP32, tag="ostg")
                    nc.vector.tensor_add(
                        out=ostg, in0=ps, in1=bo_bc[:, oh * 256 : (oh + 1) * 256]
                    )
                    nc.sync.dma_start(
                        out=out[
                            tokt * 128 : (tokt + 1) * 128,
                            oh * 256 : (oh + 1) * 256,
                        ],
                        in_=ostg,
                    )

    nc.compile()
    return nc


def kernel(x, wq, bq, wk, bk, wv, bv, wo, bo, trace=False):
    x = np.asarray(x, dtype=np.float32)
    wq = np.asarray(wq, dtype=np.float32)
    bq = np.asarray(bq, dtype=np.float32)
    wk = np.asarray(wk, dtype=np.float32)
    bk = np.asarray(bk, dtype=np.float32)
    wv = np.asarray(wv, dtype=np.float32)
    bv = np.asarray(bv, dtype=np.float32)
    wo = np.asarray(wo, dtype=np.float32)
    bo = np.asarray(bo, dtype=np.float32)

    if "nc" not in _CACHE:
        _CACHE["nc"] = build_nc()
    nc = _CACHE["nc"]

    wq_f = wq.transpose(1, 0, 2).reshape(D, H * E)  # [D, heads*E] head-major cols
    wk_f = wk.transpose(1, 0, 2).reshape(D, H * E)
    wv_f = wv.transpose(1, 0, 2).reshape(D, H * E)
    wo_t = wo.T  # [in 1024, out 1024], in-dim = global head-major
    bo_row = np.ascontiguousarray(bo.reshape(1, D))

    in_maps = []
    for c in range(NCORES):
        b, hg = c // 2, c % 2
        cs = slice(hg * HL * E, (hg + 1) * HL * E)
        xT_b = x[b].T
        if hg == 0:
            xT_c = np.ascontiguousarray(xT_b)
        else:
            # own out-tokens first; K/V/Q all share this local token order
            xT_c = np.ascontiguousarray(
                np.concatenate([xT_b[:, TQ:], xT_b[:, :TQ]], axis=1)
            )
        # augmented wo: 4 own head-pair blocks, then (p, g) gathered blocks
        # with the loopback (g == hg) blocks zeroed
        wo_aug = np.zeros((12 * 128, D), dtype=np.float32)
        for p_ in range(4):
            wo_aug[p_ * 128 : (p_ + 1) * 128] = wo_t[
                hg * 512 + p_ * 128 : hg * 512 + (p_ + 1) * 128
            ]
        for p_ in range(4):
            for g_ in range(2):
                if g_ == hg:
                    continue
                t_ = 4 + p_ * 2 + g_
                wo_aug[t_ * 128 : (t_ + 1) * 128] = wo_t[
                    g_ * 512 + p_ * 128 : g_ * 512 + (p_ + 1) * 128
                ]
        m = {
            "xT": xT_c,
            "wq_t": np.ascontiguousarray(wq_f[:, cs]),
            "wk_t": np.ascontiguousarray(wk_f[:, cs]),
            "wv_t": np.ascontiguousarray(wv_f[:, cs]),
            "wo_b": np.ascontiguousarray(wo_aug).astype(ml_dtypes.bfloat16),
            "bqp": np.ascontiguousarray(
                bq.reshape(H * E)[cs].reshape(4, 128).T
            ),
            "bkp": np.ascontiguousarray(
                bk.reshape(H * E)[cs].reshape(4, 128).T
            ),
            "bv_row": np.ascontiguousarray(bv.reshape(1, H * E)[:, cs]),
            "bo_row": bo_row,
        }
        in_maps.append(m)

    res = run_bass_kernel_spmd(nc, in_maps, list(range(NCORES)), trace=trace)

    out = np.empty((B, S, D), dtype=np.float32)
    for c in range(NCORES):
        b, hg = c // 2, c % 2
        out[b, hg * TQ : (hg + 1) * TQ, :] = res.results[c]["out"]
    if trace:
        return out, res
    return out


# revision 10
# speedup vs baseline: 1.3284x; 1.0053x over previous
"""Multi-head attention (B=4, S=2048, D=1024, H=16, E=64) on 8 TRN2 NeuronCores.

Sharding: core c handles batch b=c//2 and head-group hg=c%2 (8 heads) over the
full 2048-token sequence; after each 2-head pass the peer-token half of the
attention output is exchanged pairwise (AllGather over [2c,2c+1]) and each
core runs the output projection for its own 1024 tokens over all 16 heads
(augmented wo layout with zeroed loopback blocks, as the program is SPMD).

Kernel structure (v2 — uniform 128x128 PE mode, exp/PE balanced pipeline):
  * All matmuls use the full 128-row PE config; the per-head scores matmuls
    (contraction = E = 64) are padded to 128 contraction rows via a
    zero-padded Q layout qt[128, 2, S]: strip 0 holds head A's Q in rows
    0-63 (rows 64-127 zero), strip 1 head B's in rows 64-127.  lhsT is the
    shared kt[:, kt-tile] so K rows of the other head multiply zeros.  This
    avoids the (64,128)<->(128,128) tiling-mode drains the PE would pay when
    interleaving scores and att@V matmuls.
  * Inner loop per (pass, tqt of 512 queries, kt of 128 keys): 2 scores
    matmuls (one per head, adjacent PSUM banks), one exp ACTIVATE over
    [128, 2, 512] (N=1024), 2 att@V accumulation matmuls (M=65: V plus a
    ones column that yields the softmax sums in PSUM row 64).
  * V is computed once (fp32r) and kept resident in SBUF as bf16
    [128 tok-part, 16 tok-tile, 4 head, 65] (two quads) — no DRAM spill.
  * K/Q are stored bf16 (scores matmul in bf16; PSUM accumulates fp32).
  * Softmax normalize: fast PSUM evacuation (sums row + 64 att rows copied
    to SBUF by DVE to free the bank), reciprocal_approx_fast, GpSimd
    partition broadcast, DVE multiply straight into the own-half tile or the
    exchange staging tile.
  * A filler queue interleaves the V/K/Q projection matmuls (and late DMAs)
    into the exp-bound attention loop at ~2 instructions per kt so the PE
    never idles (keeps the tensor-engine DVFS p-state at max clock).
"""

import numpy as np
import ml_dtypes

import concourse.bass as bass
import concourse.mybir as mybir
import concourse.tile as tile
from concourse import bacc
from concourse.bass_utils import run_bass_kernel_spmd

FP32 = mybir.dt.float32
FP32R = mybir.dt.float32r
BF16 = mybir.dt.bfloat16
AF = mybir.ActivationFunctionType

B, S, D, H, E = 4, 2048, 1024, 16, 64
NCORES = 8
TQ = S // 2  # tokens per core for the output projection
HL = H // 2  # local heads per core
SCALE = 1.0 / float(np.sqrt(E))
PAIRS = [[0, 1], [2, 3], [4, 5], [6, 7]]

_CACHE = {}


def build_nc():
    nc = bacc.Bacc("TRN2", target_bir_lowering=False)

    xT = nc.dram_tensor("xT", [D, S], FP32R, kind="ExternalInput")
    wq_t = nc.dram_tensor("wq_t", [D, HL * E], FP32R, kind="ExternalInput")
    wk_t = nc.dram_tensor("wk_t", [D, HL * E], FP32R, kind="ExternalInput")
    wv_t = nc.dram_tensor("wv_t", [D, HL * E], FP32R, kind="ExternalInput")
    wo_b = nc.dram_tensor("wo_b", [12 * 128, D], BF16, kind="ExternalInput")
    bqp = nc.dram_tensor("bqp", [128, 4], FP32, kind="ExternalInput")
    bkp = nc.dram_tensor("bkp", [128, 4], FP32, kind="ExternalInput")
    bv_row = nc.dram_tensor("bv_row", [1, HL * E], FP32R, kind="ExternalInput")
    bo_row = nc.dram_tensor("bo_row", [1, D], FP32R, kind="ExternalInput")
    out = nc.dram_tensor("out", [TQ, D], FP32, kind="ExternalOutput")
    att_gin = nc.dram_tensor("att_gin", [4, 128, TQ], BF16)
    att_gout = nc.dram_tensor("att_gout", [4, 2, 128, TQ], BF16)

    xT_r = xT.rearrange("(t p) s -> p t s", p=128)  # [128, 8, 2048]
    wq_r = wq_t.rearrange("(t p) m -> p t m", p=128)  # [128, 8, 512]
    wk_r = wk_t.rearrange("(t p) m -> p t m", p=128)
    wv_r = wv_t.rearrange("(t p) m -> p t m", p=128)
    wo_r = wo_b.rearrange("(t p) m -> p t m", p=128)  # [128, 12, 1024]

    from contextlib import ExitStack

    with tile.TileContext(nc) as tc:
        with ExitStack() as _es:
            qt_pool = _es.enter_context(tc.tile_pool(name="qtp", bufs=2))
            kt_pool = _es.enter_context(tc.tile_pool(name="ktp", bufs=2))
            v_pool = _es.enter_context(tc.tile_pool(name="vp", bufs=2))
            w_pool = _es.enter_context(tc.tile_pool(name="wp", bufs=4))
            exp_pool = _es.enter_context(tc.tile_pool(name="expp", bufs=3))
            own_pool = _es.enter_context(tc.tile_pool(name="own", bufs=4))
            gin_pool = _es.enter_context(tc.tile_pool(name="gin", bufs=2))
            recv_pool = _es.enter_context(tc.tile_pool(name="recv", bufs=8))
            stage_pool = _es.enter_context(tc.tile_pool(name="astg", bufs=2))
            ostg_pool = _es.enter_context(tc.tile_pool(name="ostg", bufs=2))
            small_pool = _es.enter_context(tc.tile_pool(name="small", bufs=2))
            rb_pool = _es.enter_context(tc.tile_pool(name="rbp", bufs=2))
            ones_pool = _es.enter_context(tc.tile_pool(name="ones", bufs=1))
            ps_sc = _es.enter_context(tc.tile_pool(name="ps_s", bufs=2, space="PSUM"))
            ps_att = _es.enter_context(tc.tile_pool(name="ps_a", bufs=2, space="PSUM"))
            ps_gen = _es.enter_context(tc.tile_pool(name="ps_g", bufs=2, space="PSUM"))

            # inner scope: released after pass 2 so the staging pool can
            # reuse the 72KB (xt + K/Q weights are dead by then)
            inner_es = ExitStack()
            xt_pool = inner_es.enter_context(tc.tile_pool(name="xt", bufs=1))
            wkq_pool = inner_es.enter_context(tc.tile_pool(name="wkq", bufs=2))

            # ---- persistent tiles ----
            xt_sb = xt_pool.tile([128, 8, S], FP32R, tag="xt")  # 64KB/part
            for k in range(8):
                nc.sync.dma_start(out=xt_sb[:, k, :], in_=xT_r[:, k, :])

            ones_col_f = ones_pool.tile([128, 4], FP32, tag="onescf")
            nc.vector.memset(ones_col_f, 1.0)
            ones_col = ones_pool.tile([128, 4], BF16, tag="onescol")
            nc.vector.tensor_copy(out=ones_col, in_=ones_col_f)
            # tiny dummy exp: preload the ACT exp table during the xT DMA
            exp_warm = ones_pool.tile([1, 4], FP32, tag="expwarm")
            nc.scalar.activation(
                out=exp_warm, in_=ones_col_f[0:1, :], func=AF.Exp, scale=1.0
            )
            bq_sb = ones_pool.tile([128, 4], FP32, tag="bq")
            bk_sb = ones_pool.tile([128, 4], FP32, tag="bk")
            nc.sync.dma_start(out=bq_sb, in_=bqp[:, :])
            nc.sync.dma_start(out=bk_sb, in_=bkp[:, :])

            bv_bc = ones_pool.tile([128, HL * E], FP32R, tag="bvbc")
            bo_bc = ones_pool.tile([128, D], FP32R, tag="bobc")
            nc.sync.dma_start(
                out=bv_bc, in_=bv_row[:, :].to_broadcast([128, HL * E])
            )
            nc.sync.dma_start(out=bo_bc, in_=bo_row[:, :].to_broadcast([128, D]))

            # V resident in SBUF: quad q holds heads 4q..4q+3:
            # [tok-in-tile(part), tok-tile, head, E+1]
            v_tiles = [
                v_pool.tile([128, 16, 4, E + 1], BF16, tag="vsb", name=f"v{q}")
                for q in range(2)
            ]
            # zero-padded Q, ping-pong across passes: strip 0 = head A rows
            # 0-63 (rest zero), strip 1 = head B rows 64-127 (rest zero)
            qt_tiles = [
                qt_pool.tile([128, 2, S], BF16, tag="qt", name=f"qt{i}")
                for i in range(2)
            ]
            for i in range(2):
                nc.vector.memset(qt_tiles[i][64:128, 0, :], 0.0)
                nc.vector.memset(qt_tiles[i][0:64, 1, :], 0.0)

            # ---- filler queue ----
            fill_q = []

            def step(n=2):
                if len(fill_q) > 120:
                    n += 1
                for _ in range(n):
                    if fill_q:
                        fill_q.pop(0)()

            def flush():
                while fill_q:
                    fill_q.pop(0)()

            done_marks = set()

            def mark(tag):
                def m():
                    done_marks.add(tag)

                return [m]

            def ensure(tag):
                while tag not in done_marks and fill_q:
                    fill_q.pop(0)()

            # ---- projection group emitters (closure lists) ----
            wv_sbs = {}

            def wv_dma(q):
                def go():
                    wv_sbs[q] = w_pool.tile(
                        [128, 8, 256], FP32R, tag="wp", name=f"wv{q}"
                    )
                    nc.sync.dma_start(
                        out=wv_sbs[q], in_=wv_r[:, :, q * 256 : (q + 1) * 256]
                    )

                return [go]

            def v_group(q, tokt):
                cell = {}

                def mm(k):
                    if k == 0:
                        cell["ps"] = ps_gen.tile(
                            [128, 256], FP32, tag="gen", name=f"vps{q}_{tokt}"
                        )
                    nc.tensor.matmul(
                        out=cell["ps"],
                        lhsT=xt_sb[:, k, tokt * 128 : (tokt + 1) * 128],
                        rhs=wv_sbs[q][:, k, :],
                        start=(k == 0),
                        stop=(k == 7),
                    )

                def fin():
                    ps = cell["ps"]
                    nc.vector.tensor_add(
                        out=v_tiles[q][:, tokt, :, :E],
                        in0=ps.rearrange("p (h e) -> p h e", e=E),
                        in1=bv_bc[:, q * 256 : (q + 1) * 256].rearrange(
                            "p (h e) -> p h e", e=E
                        ),
                    )
                    nc.vector.tensor_copy(
                        out=v_tiles[q][:, tokt, :, E : E + 1],
                        in_=ones_col[:, :4].unsqueeze(2),
                    )

                return (
                    [lambda k=k: mm(k) for k in range(8)]
                    + [fin]
                    + mark((f"v{q}", tokt))
                )

            wkq_sbs = {}

            def wkq_dma(p):
                def go():
                    wk_sb = wkq_pool.tile(
                        [128, 8, 128], FP32R, tag="wk", name=f"wk{p}"
                    )
                    wq_sb = wkq_pool.tile(
                        [128, 8, 128], FP32R, tag="wq", name=f"wq{p}"
                    )
                    nc.sync.dma_start(
                        out=wk_sb, in_=wk_r[:, :, p * 128 : (p + 1) * 128]
                    )
                    nc.sync.dma_start(
                        out=wq_sb, in_=wq_r[:, :, p * 128 : (p + 1) * 128]
                    )
                    wkq_sbs[p] = (wk_sb, wq_sb)

                return [go]

            kt_sbs = {}

            def kt_alloc(p):
                def go():
                    kt_sbs[p] = kt_pool.tile([128, S], BF16, tag="kt", name=f"kt{p}")

                return [go]

            def k_group(p, ts):
                cell = {}

                def mm(k):
                    if k == 0:
                        cell["ps"] = ps_gen.tile(
                            [128, 512], FP32, tag="gen", name=f"kps{p}_{ts}"
                        )
                    nc.tensor.matmul(
                        out=cell["ps"],
                        lhsT=wkq_sbs[p][0][:, k, :],
                        rhs=xt_sb[:, k, ts * 512 : (ts + 1) * 512],
                        start=(k == 0),
                        stop=(k == 7),
                    )

                def fin():
                    nc.vector.tensor_scalar_add(
                        out=kt_sbs[p][:, ts * 512 : (ts + 1) * 512],
                        in0=cell["ps"],
                        scalar1=bk_sb[:, p : p + 1],
                    )

                return (
                    [lambda k=k: mm(k) for k in range(8)]
                    + [fin]
                    + mark(("k", p, ts))
                )

            def q_group(p, qs):
                cell = {}
                qt_t = qt_tiles[p % 2]

                def mm(k):
                    if k == 0:
                        cell["ps"] = ps_gen.tile(
                            [128, 512], FP32, tag="gen", name=f"qps{p}_{qs}"
                        )
                    nc.tensor.matmul(
                        out=cell["ps"],
                        lhsT=wkq_sbs[p][1][:, k, :],
                        rhs=xt_sb[:, k, qs * 512 : (qs + 1) * 512],
                        start=(k == 0),
                        stop=(k == 7),
                    )

                def fin():
                    ps = cell["ps"]
                    nc.vector.tensor_scalar_add(
                        out=qt_t[0:64, 0, qs * 512 : (qs + 1) * 512],
                        in0=ps[0:64, :],
                        scalar1=bq_sb[0:64, p : p + 1],
                    )
                    nc.vector.tensor_scalar_add(
                        out=qt_t[64:128, 1, qs * 512 : (qs + 1) * 512],
                        in0=ps[64:128, :],
                        scalar1=bq_sb[64:128, p : p + 1],
                    )

                return (
                    [lambda k=k: mm(k) for k in range(8)]
                    + [fin]
                    + mark(("q", p, qs))
                )

            # ---- upfront: just enough for pass-0 tqt0's first key tiles ----
            for cl in (
                wv_dma(0)
                + wkq_dma(0)
                + kt_alloc(0)
                + [c for ts in range(2) for c in k_group(0, ts)]
                + q_group(0, 0)
                + [c for t in range(4) for c in v_group(0, t)]
            ):
                cl()

            # filler for pass 0: rest of pass-0 K/Q/V, V quad 1, pass-1 K/Q.
            # ensure() marks make consumers wait for these.
            fill_q.extend(
                [c for t in range(4, 6) for c in v_group(0, t)]
                + [c for c in k_group(0, 2)]
                + [c for t in range(6, 8) for c in v_group(0, t)]
                + [c for c in k_group(0, 3)]
                + q_group(0, 1)
                + [c for t in range(8, 16) for c in v_group(0, t)]
                + q_group(0, 2)
                + q_group(0, 3)
                + wv_dma(1)
                + wkq_dma(1)
                + kt_alloc(1)
                + [c for ts in range(4) for c in k_group(1, ts)]
                + [c for qs in range(4) for c in q_group(1, qs)]
                + [c for t in range(16) for c in v_group(1, t)]
            )

            own_tiles = []
            wo_sbs = {}
            recv_tiles = {}

            def wo_dma(oh):
                def go():
                    wo_sbs[oh] = w_pool.tile(
                        [128, 12, 256], BF16, tag="wp", name=f"wo{oh}"
                    )
                    nc.sync.dma_start(
                        out=wo_sbs[oh], in_=wo_r[:, :, oh * 256 : (oh + 1) * 256]
                    )

                return [go]

            def recv_dma(p, g):
                def go():
                    rt = recv_pool.tile([128, TQ], BF16, tag="recv", name=f"rc{p}{g}")
                    nc.sync.dma_start(out=rt, in_=att_gout[p, g])
                    recv_tiles[(p, g)] = rt

                return [go]

            # ---- attention passes ----
            def emit_pass(p):
                qt_t = qt_tiles[p % 2]
                kt_t = kt_sbs[p]
                vq = 0 if p < 2 else 1
                vA, vB = 2 * (p % 2), 2 * (p % 2) + 1
                v_t = v_tiles[vq]

                own_t = own_pool.tile([128, TQ], BF16, tag="own", name=f"own{p}")
                own_tiles.append(own_t)
                gin_t = gin_pool.tile([128, TQ], BF16, tag="gin", name=f"gin{p}")

                for tqt in range(4):
                    ensure(("q", p, tqt))
                    att_A = ps_att.tile([E + 1, 512], FP32, tag="att")
                    att_B = ps_att.tile([E + 1, 512], FP32, tag="att")
                    for kt in range(16):
                        ensure(("k", p, kt // 4))
                        ensure((f"v{vq}", kt))
                        ps = ps_sc.tile([128, 2, 512], FP32, tag="sc")
                        nc.tensor.matmul(
                            out=ps[:, 0, :],
                            lhsT=kt_t[:, kt * 128 : (kt + 1) * 128],
                            rhs=qt_t[:, 0, tqt * 512 : (tqt + 1) * 512],
                            start=True,
                            stop=True,
                        )
                        nc.tensor.matmul(
                            out=ps[:, 1, :],
                            lhsT=kt_t[:, kt * 128 : (kt + 1) * 128],
                            rhs=qt_t[:, 1, tqt * 512 : (tqt + 1) * 512],
                            start=True,
                            stop=True,
                        )
                        ex = exp_pool.tile([128, 2, 512], BF16, tag="exp")
                        nc.scalar.activation(out=ex, in_=ps, func=AF.Exp, scale=SCALE)
                        nc.tensor.matmul(
                            out=att_A,
                            lhsT=v_t[:, kt, vA, :],
                            rhs=ex[:, 0, :],
                            start=(kt == 0),
                            stop=(kt == 15),
                        )
                        nc.tensor.matmul(
                            out=att_B,
                            lhsT=v_t[:, kt, vB, :],
                            rhs=ex[:, 1, :],
                            start=(kt == 0),
                            stop=(kt == 15),
                        )
                        step(2)

                    for hh, att_ps in ((0, att_A), (1, att_B)):
                        # fast PSUM evacuation, then normalize from SBUF
                        sums = small_pool.tile([1, 512], FP32, tag="sums", bufs=1)
                        nc.vector.tensor_copy(out=sums, in_=att_ps[E : E + 1, :])
                        a_sb = stage_pool.tile([64, 512], FP32, tag="astg")
                        nc.vector.tensor_copy(out=a_sb, in_=att_ps[:E, :])
                        recip = small_pool.tile([1, 512], FP32, tag="recip", bufs=1)
                        nc.vector.reciprocal_approx_fast(out=recip, in_=sums)
                        rb = rb_pool.tile([64, 512], FP32, tag="rbb")
                        nc.gpsimd.partition_broadcast(rb, recip)
                        if tqt < 2:
                            dest = own_t[
                                hh * 64 : (hh + 1) * 64,
                                tqt * 512 : (tqt + 1) * 512,
                            ]
                        else:
                            dest = gin_t[
                                hh * 64 : (hh + 1) * 64,
                                (tqt - 2) * 512 : (tqt - 1) * 512,
                            ]
                        nc.vector.tensor_mul(out=dest, in0=a_sb, in1=rb)

                # pair exchange of this pass's peer-token half (bf16)
                nc.sync.dma_start(out=att_gin[p], in_=gin_t)
                nc.gpsimd.collective_compute(
                    kind="AllGather",
                    op=mybir.AluOpType.bypass,
                    replica_groups=PAIRS,
                    ins=[att_gin[p]],
                    outs=[att_gout[p]],
                )

            def lhs_of(t):
                if t < 4:
                    return own_tiles[t]
                return recv_tiles[((t - 4) // 2, (t - 4) % 2)]

            # out-proj partials: blocks available before pass 3 ends
            # (own 0-2 + all of recv pass 0-2); finals add own3 + recv3.
            PARTIAL_BLOCKS = [0, 1, 2, 4, 5, 6, 7, 8, 9]
            FINAL_BLOCKS = [3, 10, 11]
            stg_tiles = {}

            def partial_group(oh, tokt):
                cell = {}

                def mm(i):
                    if i == 0:
                        cell["ps"] = ps_gen.tile(
                            [128, 256], FP32, tag="gen", name=f"pps{oh}_{tokt}"
                        )
                    t = PARTIAL_BLOCKS[i]
                    nc.tensor.matmul(
                        out=cell["ps"],
                        lhsT=lhs_of(t)[:, tokt * 128 : (tokt + 1) * 128],
                        rhs=wo_sbs[oh][:, t, :],
                        start=(i == 0),
                        stop=(i == len(PARTIAL_BLOCKS) - 1),
                    )

                def fin():
                    stg = stg_pool.tile(
                        [128, 256], FP32, tag="stg", name=f"stg{oh}_{tokt}"
                    )
                    nc.vector.tensor_add(
                        out=stg,
                        in0=cell["ps"],
                        in1=bo_bc[:, oh * 256 : (oh + 1) * 256],
                    )
                    stg_tiles[(oh, tokt)] = stg

                return [
                    lambda i=i: mm(i) for i in range(len(PARTIAL_BLOCKS))
                ] + [fin]

            emit_pass(0)
            fill_q.extend(
                wkq_dma(2)
                + kt_alloc(2)
                + [c for ts in range(4) for c in k_group(2, ts)]
                + [c for qs in range(4) for c in q_group(2, qs)]
            )
            emit_pass(1)
            fill_q.extend(
                wkq_dma(3)
                + kt_alloc(3)
                + [c for ts in range(4) for c in k_group(3, ts)]
                + [c for qs in range(4) for c in q_group(3, qs)]
                + recv_dma(0, 0)
                + recv_dma(0, 1)
                + [c for oh in range(4) for c in wo_dma(oh)]
            )
            emit_pass(2)
            fill_q.extend(recv_dma(1, 0) + recv_dma(1, 1))
            flush()
            # xT and K/Q weights are dead: release their 72KB for staging
            inner_es.close()
            stg_pool = _es.enter_context(tc.tile_pool(name="stg", bufs=32))

            fill_q.extend(
                recv_dma(2, 0)
                + recv_dma(2, 1)
                + [
                    c
                    for oh in range(4)
                    for tokt in range(8)
                    for c in partial_group(oh, tokt)
                ]
            )
            emit_pass(3)
            flush()
            for cl in recv_dma(3, 0) + recv_dma(3, 1):
                cl()

            # ---- finals: own3 + recv3 blocks on top of the staged partials
            for oh in range(4):
                for tokt in range(8):
                    ps = ps_gen.tile([128, 256], FP32, tag="gen")
                    for i, t in enumerate(FINAL_BLOCKS):
                        nc.tensor.matmul(
                            out=ps,
                            lhsT=lhs_of(t)[:, tokt * 128 : (tokt + 1) * 128],
                            rhs=wo_sbs[oh][:, t, :],
                            start=(i == 0),
                            stop=(i == len(FINAL_BLOCKS) - 1),
                        )
                    ostg = ostg_pool.tile([128, 256], FP32, tag="ostg")
                    nc.vector.tensor_add(
                        out=ostg, in0=ps, in1=stg_tiles[(oh, tokt)]
                    )
                    nc.sync.dma_start(
                        out=out[
                            tokt * 128 : (tokt + 1) * 128,
                            oh * 256 : (oh + 1) * 256,
                        ],
                        in_=ostg,
                    )

    nc.compile()
    return nc


def kernel(x, wq, bq, wk, bk, wv, bv, wo, bo, trace=False):
    x = np.asarray(x, dtype=np.float32)
    wq = np.asarray(wq, dtype=np.float32)
    bq = np.asarray(bq, dtype=np.float32)
    wk = np.asarray(wk, dtype=np.float32)
    bk = np.asarray(bk, dtype=np.float32)
    wv = np.asarray(wv, dtype=np.float32)
    bv = np.asarray(bv, dtype=np.float32)
    wo = np.asarray(wo, dtype=np.float32)
    bo = np.asarray(bo, dtype=np.float32)

    if "nc" not in _CACHE:
        _CACHE["nc"] = build_nc()
    nc = _CACHE["nc"]

    wq_f = wq.transpose(1, 0, 2).reshape(D, H * E)  # [D, heads*E] head-major cols
    wk_f = wk.transpose(1, 0, 2).reshape(D, H * E)
    wv_f = wv.transpose(1, 0, 2).reshape(D, H * E)
    wo_t = wo.T  # [in 1024, out 1024], in-dim = global head-major
    bo_row = np.ascontiguousarray(bo.reshape(1, D))

    in_maps = []
    for c in range(NCORES):
        b, hg = c // 2, c % 2
        cs = slice(hg * HL * E, (hg + 1) * HL * E)
        xT_b = x[b].T
        if hg == 0:
            xT_c = np.ascontiguousarray(xT_b)
        else:
            # own out-tokens first; K/V/Q all share this local token order
            xT_c = np.ascontiguousarray(
                np.concatenate([xT_b[:, TQ:], xT_b[:, :TQ]], axis=1)
            )
        # augmented wo: 4 own head-pair blocks, then (p, g) gathered blocks
        # with the loopback (g == hg) blocks zeroed
        wo_aug = np.zeros((12 * 128, D), dtype=np.float32)
        for p_ in range(4):
            wo_aug[p_ * 128 : (p_ + 1) * 128] = wo_t[
                hg * 512 + p_ * 128 : hg * 512 + (p_ + 1) * 128
            ]
        for p_ in range(4):
            for g_ in range(2):
                if g_ == hg:
                    continue
                t_ = 4 + p_ * 2 + g_
                wo_aug[t_ * 128 : (t_ + 1) * 128] = wo_t[
                    g_ * 512 + p_ * 128 : g_ * 512 + (p_ + 1) * 128
                ]
        m = {
            "xT": xT_c,
            "wq_t": np.ascontiguousarray(wq_f[:, cs]),
            "wk_t": np.ascontiguousarray(wk_f[:, cs]),
            "wv_t": np.ascontiguousarray(wv_f[:, cs]),
            "wo_b": np.ascontiguousarray(wo_aug).astype(ml_dtypes.bfloat16),
            "bqp": np.ascontiguousarray(
                bq.reshape(H * E)[cs].reshape(4, 128).T
            ),
            "bkp": np.ascontiguousarray(
                bk.reshape(H * E)[cs].reshape(4, 128).T
            ),
            "bv_row": np.ascontiguousarray(bv.reshape(1, H * E)[:, cs]),
            "bo_row": bo_row,
        }
        in_maps.append(m)

    res = run_bass_kernel_spmd(nc, in_maps, list(range(NCORES)), trace=trace)

    out = np.empty((B, S, D), dtype=np.float32)
    for c in range(NCORES):
        b, hg = c // 2, c % 2
        out[b, hg * TQ : (hg + 1) * TQ, :] = res.results[c]["out"]
    if trace:
        return out, res
    return out


# revision 12
# speedup vs baseline: 1.4070x; 1.0592x over previous
"""Multi-head attention (B=4, S=2048, D=1024, H=16, E=64) on 8 TRN2 NeuronCores.

Sharding: core c handles batch b=c//2 and head-group hg=c%2 (8 heads) over the
full 2048-token sequence; after each 2-head pass the peer-token half of the
attention output is exchanged pairwise (AllGather over [2c,2c+1]) and each
core runs the output projection for its own 1024 tokens over all 16 heads
(augmented wo layout with zeroed loopback blocks, as the program is SPMD).

Kernel structure (v2 — uniform 128x128 PE mode, exp/PE balanced pipeline):
  * All matmuls use the full 128-row PE config; the per-head scores matmuls
    (contraction = E = 64) are padded to 128 contraction rows via a
    zero-padded Q layout qt[128, 2, S]: strip 0 holds head A's Q in rows
    0-63 (rows 64-127 zero), strip 1 head B's in rows 64-127.  lhsT is the
    shared kt[:, kt-tile] so K rows of the other head multiply zeros.  This
    avoids the (64,128)<->(128,128) tiling-mode drains the PE would pay when
    interleaving scores and att@V matmuls.
  * Inner loop per (pass, tqt of 512 queries, kt of 128 keys): 2 scores
    matmuls (one per head, adjacent PSUM banks), one exp ACTIVATE over
    [128, 2, 512] (N=1024), 2 att@V accumulation matmuls (M=65: V plus a
    ones column that yields the softmax sums in PSUM row 64).
  * V is computed once (fp32r) and kept resident in SBUF as bf16
    [128 tok-part, 16 tok-tile, 4 head, 65] (two quads) — no DRAM spill.
  * K/Q are stored bf16 (scores matmul in bf16; PSUM accumulates fp32).
  * Softmax normalize: fast PSUM evacuation (sums row + 64 att rows copied
    to SBUF by DVE to free the bank), reciprocal_approx_fast, GpSimd
    partition broadcast, DVE multiply straight into the own-half tile or the
    exchange staging tile.
  * A filler queue interleaves the V/K/Q projection matmuls (and late DMAs)
    into the exp-bound attention loop at ~2 instructions per kt so the PE
    never idles (keeps the tensor-engine DVFS p-state at max clock).
"""

import numpy as np
import ml_dtypes

import concourse.bass as bass
import concourse.mybir as mybir
import concourse.tile as tile
from concourse import bacc
from concourse.bass_utils import run_bass_kernel_spmd

FP32 = mybir.dt.float32
FP32R = mybir.dt.float32r
BF16 = mybir.dt.bfloat16
AF = mybir.ActivationFunctionType

B, S, D, H, E = 4, 2048, 1024, 16, 64
NCORES = 8
TQ = S // 2  # tokens per core for the output projection
HL = H // 2  # local heads per core
SCALE = 1.0 / float(np.sqrt(E))
PAIRS = [[0, 1], [2, 3], [4, 5], [6, 7]]

_CACHE = {}


def build_nc():
    nc = bacc.Bacc("TRN2", target_bir_lowering=False)

    xT = nc.dram_tensor("xT", [D, S], BF16, kind="ExternalInput")
    wq_t = nc.dram_tensor("wq_t", [D, HL * E], BF16, kind="ExternalInput")
    wk_t = nc.dram_tensor("wk_t", [D, HL * E], BF16, kind="ExternalInput")
    wv_t = nc.dram_tensor("wv_t", [D, HL * E], BF16, kind="ExternalInput")
    wo_b = nc.dram_tensor("wo_b", [12 * 128, D], BF16, kind="ExternalInput")
    bqp = nc.dram_tensor("bqp", [128, 4], FP32, kind="ExternalInput")
    bkp = nc.dram_tensor("bkp", [128, 4], FP32, kind="ExternalInput")
    bv_row = nc.dram_tensor("bv_row", [1, HL * E], FP32R, kind="ExternalInput")
    bo_row = nc.dram_tensor("bo_row", [1, D], FP32R, kind="ExternalInput")
    out = nc.dram_tensor("out", [TQ, D], FP32, kind="ExternalOutput")
    att_gin = nc.dram_tensor("att_gin", [4, 128, TQ], BF16)
    att_gout = nc.dram_tensor("att_gout", [4, 2, 128, TQ], BF16)

    xT_r = xT.rearrange("(t p) s -> p t s", p=128)  # [128, 8, 2048]
    wq_r = wq_t.rearrange("(t p) m -> p t m", p=128)  # [128, 8, 512]
    wk_r = wk_t.rearrange("(t p) m -> p t m", p=128)
    wv_r = wv_t.rearrange("(t p) m -> p t m", p=128)
    wo_r = wo_b.rearrange("(t p) m -> p t m", p=128)  # [128, 12, 1024]

    from contextlib import ExitStack

    with tile.TileContext(nc) as tc:
        with ExitStack() as _es:
            qt_pool = _es.enter_context(tc.tile_pool(name="qtp", bufs=2))
            kt_pool = _es.enter_context(tc.tile_pool(name="ktp", bufs=2))
            v_pool = _es.enter_context(tc.tile_pool(name="vp", bufs=2))
            w_pool = _es.enter_context(tc.tile_pool(name="wp", bufs=4))
            exp_pool = _es.enter_context(tc.tile_pool(name="expp", bufs=4))
            own_pool = _es.enter_context(tc.tile_pool(name="own", bufs=4))
            gin_pool = _es.enter_context(tc.tile_pool(name="gin", bufs=2))
            recv_pool = _es.enter_context(tc.tile_pool(name="recv", bufs=8))
            stage_pool = _es.enter_context(tc.tile_pool(name="astg", bufs=2))
            ostg_pool = _es.enter_context(tc.tile_pool(name="ostg", bufs=2))
            small_pool = _es.enter_context(tc.tile_pool(name="small", bufs=2))
            rb_pool = _es.enter_context(tc.tile_pool(name="rbp", bufs=2))
            ones_pool = _es.enter_context(tc.tile_pool(name="ones", bufs=1))
            ps_sc = _es.enter_context(tc.tile_pool(name="ps_s", bufs=2, space="PSUM"))
            ps_att = _es.enter_context(tc.tile_pool(name="ps_a", bufs=2, space="PSUM"))
            ps_gen = _es.enter_context(tc.tile_pool(name="ps_g", bufs=2, space="PSUM"))

            # inner scope: released after pass 2 so the staging pool can
            # reuse the 72KB (xt + K/Q weights are dead by then)
            inner_es = ExitStack()
            xt_pool = inner_es.enter_context(tc.tile_pool(name="xt", bufs=1))
            wkq_pool = inner_es.enter_context(tc.tile_pool(name="wkq", bufs=2))

            # ---- persistent tiles ----
            xt_sb = xt_pool.tile([128, 8, S], BF16, tag="xt")  # 32KB/part

            ones_col_f = ones_pool.tile([128, 4], FP32, tag="onescf")
            nc.vector.memset(ones_col_f, 1.0)
            ones_col = ones_pool.tile([128, 4], BF16, tag="onescol")
            nc.vector.tensor_copy(out=ones_col, in_=ones_col_f)
            # tiny dummy exp: preload the ACT exp table during the xT DMA
            exp_warm = ones_pool.tile([1, 4], FP32, tag="expwarm")
            nc.scalar.activation(
                out=exp_warm, in_=ones_col_f[0:1, :], func=AF.Exp, scale=1.0
            )
            bq_sb = ones_pool.tile([128, 4], FP32, tag="bq")
            bk_sb = ones_pool.tile([128, 4], FP32, tag="bk")
            nc.sync.dma_start(out=bq_sb, in_=bqp[:, :])
            nc.sync.dma_start(out=bk_sb, in_=bkp[:, :])

            bv_bc = ones_pool.tile([128, HL * E], FP32R, tag="bvbc")
            bo_bc = ones_pool.tile([128, D], FP32R, tag="bobc")
            nc.sync.dma_start(
                out=bv_bc, in_=bv_row[:, :].to_broadcast([128, HL * E])
            )
            nc.sync.dma_start(out=bo_bc, in_=bo_row[:, :].to_broadcast([128, D]))

            # V resident in SBUF: quad q holds heads 4q..4q+3:
            # [tok-in-tile(part), tok-tile, head, E+1]
            v_tiles = [
                v_pool.tile([128, 16, 4, E + 1], BF16, tag="vsb", name=f"v{q}")
                for q in range(2)
            ]
            # zero-padded Q, ping-pong across passes: strip 0 = head A rows
            # 0-63 (rest zero), strip 1 = head B rows 64-127 (rest zero)
            qt_tiles = [
                qt_pool.tile([128, 2, S], BF16, tag="qt", name=f"qt{i}")
                for i in range(2)
            ]
            for i in range(2):
                nc.vector.memset(qt_tiles[i][64:128, 0, :], 0.0)
                nc.vector.memset(qt_tiles[i][0:64, 1, :], 0.0)

            # ---- filler queue ----
            fill_q = []

            def step(n=2):
                if len(fill_q) > 120:
                    n += 1
                for _ in range(n):
                    if fill_q:
                        fill_q.pop(0)()

            def flush():
                while fill_q:
                    fill_q.pop(0)()

            done_marks = set()

            def mark(tag):
                def m():
                    done_marks.add(tag)

                return [m]

            def ensure(tag):
                while tag not in done_marks and fill_q:
                    fill_q.pop(0)()

            # ---- projection group emitters (closure lists) ----
            wv_sbs = {}

            def wv_dma(q):
                def go():
                    wv_sbs[q] = w_pool.tile(
                        [128, 8, 256], BF16, tag="wp", name=f"wv{q}"
                    )
                    nc.sync.dma_start(
                        out=wv_sbs[q], in_=wv_r[:, :, q * 256 : (q + 1) * 256]
                    )

                return [go]

            def v_group(q, tokt):
                cell = {}

                def mm(k):
                    if k == 0:
                        cell["ps"] = ps_gen.tile(
                            [128, 256], FP32, tag="gen", name=f"vps{q}_{tokt}"
                        )
                    nc.tensor.matmul(
                        out=cell["ps"],
                        lhsT=xt_sb[:, k, tokt * 128 : (tokt + 1) * 128],
                        rhs=wv_sbs[q][:, k, :],
                        start=(k == 0),
                        stop=(k == 7),
                    )

                def fin():
                    ps = cell["ps"]
                    nc.vector.tensor_add(
                        out=v_tiles[q][:, tokt, :, :E],
                        in0=ps.rearrange("p (h e) -> p h e", e=E),
                        in1=bv_bc[:, q * 256 : (q + 1) * 256].rearrange(
                            "p (h e) -> p h e", e=E
                        ),
                    )
                    nc.vector.tensor_copy(
                        out=v_tiles[q][:, tokt, :, E : E + 1],
                        in_=ones_col[:, :4].unsqueeze(2),
                    )

                return (
                    [lambda k=k: mm(k) for k in range(8)]
                    + [fin]
                    + mark((f"v{q}", tokt))
                )

            wkq_sbs = {}

            def wkq_dma(p):
                def go():
                    wk_sb = wkq_pool.tile(
                        [128, 8, 128], BF16, tag="wk", name=f"wk{p}"
                    )
                    wq_sb = wkq_pool.tile(
                        [128, 8, 128], BF16, tag="wq", name=f"wq{p}"
                    )
                    nc.sync.dma_start(
                        out=wk_sb, in_=wk_r[:, :, p * 128 : (p + 1) * 128]
                    )
                    nc.sync.dma_start(
                        out=wq_sb, in_=wq_r[:, :, p * 128 : (p + 1) * 128]
                    )
                    wkq_sbs[p] = (wk_sb, wq_sb)

                return [go]

            kt_sbs = {}

            def kt_alloc(p):
                def go():
                    kt_sbs[p] = kt_pool.tile([128, S], BF16, tag="kt", name=f"kt{p}")

                return [go]

            def k_group(p, ts):
                cell = {}

                def mm(k):
                    if k == 0:
                        cell["ps"] = ps_gen.tile(
                            [128, 512], FP32, tag="gen", name=f"kps{p}_{ts}"
                        )
                    nc.tensor.matmul(
                        out=cell["ps"],
                        lhsT=wkq_sbs[p][0][:, k, :],
                        rhs=xt_sb[:, k, ts * 512 : (ts + 1) * 512],
                        start=(k == 0),
                        stop=(k == 7),
                    )

                def fin():
                    nc.vector.tensor_scalar_add(
                        out=kt_sbs[p][:, ts * 512 : (ts + 1) * 512],
                        in0=cell["ps"],
                        scalar1=bk_sb[:, p : p + 1],
                    )

                return (
                    [lambda k=k: mm(k) for k in range(8)]
                    + [fin]
                    + mark(("k", p, ts))
                )

            def q_group(p, qs):
                cell = {}
                qt_t = qt_tiles[p % 2]

                def mm(k):
                    if k == 0:
                        cell["ps"] = ps_gen.tile(
                            [128, 512], FP32, tag="gen", name=f"qps{p}_{qs}"
                        )
                    nc.tensor.matmul(
                        out=cell["ps"],
                        lhsT=wkq_sbs[p][1][:, k, :],
                        rhs=xt_sb[:, k, qs * 512 : (qs + 1) * 512],
                        start=(k == 0),
                        stop=(k == 7),
                    )

                def fin():
                    ps = cell["ps"]
                    nc.vector.tensor_scalar_add(
                        out=qt_t[0:64, 0, qs * 512 : (qs + 1) * 512],
                        in0=ps[0:64, :],
                        scalar1=bq_sb[0:64, p : p + 1],
                    )
                    nc.vector.tensor_scalar_add(
                        out=qt_t[64:128, 1, qs * 512 : (qs + 1) * 512],
                        in0=ps[64:128, :],
                        scalar1=bq_sb[64:128, p : p + 1],
                    )

                return (
                    [lambda k=k: mm(k) for k in range(8)]
                    + [fin]
                    + mark(("q", p, qs))
                )

            # ---- upfront: weights first (small DMAs ahead of the 4MB xT),
            # then just enough compute for pass-0 tqt0's first key tiles ----
            for cl in wv_dma(0) + wkq_dma(0):
                cl()
            for k in range(8):
                nc.sync.dma_start(out=xt_sb[:, k, :], in_=xT_r[:, k, :])
            for cl in (
                kt_alloc(0)
                + [c for ts in range(2) for c in k_group(0, ts)]
                + q_group(0, 0)
                + [c for t in range(4) for c in v_group(0, t)]
            ):
                cl()

            # filler for pass 0: rest of pass-0 K/Q/V, V quad 1, pass-1 K/Q.
            # ensure() marks make consumers wait for these.
            fill_q.extend(
                [c for t in range(4, 6) for c in v_group(0, t)]
                + [c for c in k_group(0, 2)]
                + [c for t in range(6, 8) for c in v_group(0, t)]
                + [c for c in k_group(0, 3)]
                + q_group(0, 1)
                + [c for t in range(8, 16) for c in v_group(0, t)]
                + q_group(0, 2)
                + q_group(0, 3)
                + wv_dma(1)
                + wkq_dma(1)
                + kt_alloc(1)
                + [c for ts in range(4) for c in k_group(1, ts)]
                + [c for qs in range(4) for c in q_group(1, qs)]
                + [c for t in range(16) for c in v_group(1, t)]
            )

            own_tiles = []
            wo_sbs = {}
            recv_tiles = {}

            def wo_dma(oh):
                def go():
                    wo_sbs[oh] = w_pool.tile(
                        [128, 12, 512], BF16, tag="wp", name=f"wo{oh}"
                    )
                    nc.sync.dma_start(
                        out=wo_sbs[oh], in_=wo_r[:, :, oh * 512 : (oh + 1) * 512]
                    )

                return [go]

            def recv_dma(p, g):
                def go():
                    rt = recv_pool.tile([128, TQ], BF16, tag="recv", name=f"rc{p}{g}")
                    nc.sync.dma_start(out=rt, in_=att_gout[p, g])
                    recv_tiles[(p, g)] = rt

                return [go]

            # ---- attention passes ----
            def emit_pass(p):
                qt_t = qt_tiles[p % 2]
                kt_t = kt_sbs[p]
                vq = 0 if p < 2 else 1
                vA, vB = 2 * (p % 2), 2 * (p % 2) + 1
                v_t = v_tiles[vq]

                own_t = own_pool.tile([128, TQ], BF16, tag="own", name=f"own{p}")
                own_tiles.append(own_t)
                gin_t = gin_pool.tile([128, TQ], BF16, tag="gin", name=f"gin{p}")

                for tqt in range(4):
                    ensure(("q", p, tqt))
                    att_A = ps_att.tile([E + 1, 512], FP32, tag="att")
                    att_B = ps_att.tile([E + 1, 512], FP32, tag="att")
                    for kt in range(16):
                        ensure(("k", p, kt // 4))
                        ensure((f"v{vq}", kt))
                        ps = ps_sc.tile([128, 2, 512], FP32, tag="sc")
                        nc.tensor.matmul(
                            out=ps[:, 0, :],
                            lhsT=kt_t[:, kt * 128 : (kt + 1) * 128],
                            rhs=qt_t[:, 0, tqt * 512 : (tqt + 1) * 512],
                            start=True,
                            stop=True,
                        )
                        nc.tensor.matmul(
                            out=ps[:, 1, :],
                            lhsT=kt_t[:, kt * 128 : (kt + 1) * 128],
                            rhs=qt_t[:, 1, tqt * 512 : (tqt + 1) * 512],
                            start=True,
                            stop=True,
                        )
                        ex = exp_pool.tile([128, 2, 512], BF16, tag="exp")
                        nc.scalar.activation(out=ex, in_=ps, func=AF.Exp, scale=SCALE)
                        nc.tensor.matmul(
                            out=att_A,
                            lhsT=v_t[:, kt, vA, :],
                            rhs=ex[:, 0, :],
                            start=(kt == 0),
                            stop=(kt == 15),
                        )
                        nc.tensor.matmul(
                            out=att_B,
                            lhsT=v_t[:, kt, vB, :],
                            rhs=ex[:, 1, :],
                            start=(kt == 0),
                            stop=(kt == 15),
                        )
                        step(2)

                    for hh, att_ps in ((0, att_A), (1, att_B)):
                        # fast PSUM evacuation, then normalize from SBUF
                        sums = small_pool.tile([1, 512], FP32, tag="sums", bufs=1)
                        nc.vector.tensor_copy(out=sums, in_=att_ps[E : E + 1, :])
                        a_sb = stage_pool.tile([64, 512], FP32, tag="astg")
                        nc.vector.tensor_copy(out=a_sb, in_=att_ps[:E, :])
                        recip = small_pool.tile([1, 512], FP32, tag="recip", bufs=1)
                        nc.vector.reciprocal_approx_fast(out=recip, in_=sums)
                        rb = rb_pool.tile([64, 512], FP32, tag="rbb")
                        nc.gpsimd.partition_broadcast(rb, recip)
                        if tqt < 2:
                            dest = own_t[
                                hh * 64 : (hh + 1) * 64,
                                tqt * 512 : (tqt + 1) * 512,
                            ]
                        else:
                            dest = gin_t[
                                hh * 64 : (hh + 1) * 64,
                                (tqt - 2) * 512 : (tqt - 1) * 512,
                            ]
                        nc.vector.tensor_mul(out=dest, in0=a_sb, in1=rb)

                # pair exchange of this pass's peer-token half (bf16)
                nc.sync.dma_start(out=att_gin[p], in_=gin_t)
                nc.gpsimd.collective_compute(
                    kind="AllGather",
                    op=mybir.AluOpType.bypass,
                    replica_groups=PAIRS,
                    ins=[att_gin[p]],
                    outs=[att_gout[p]],
                )

            def lhs_of(t):
                if t < 4:
                    return own_tiles[t]
                return recv_tiles[((t - 4) // 2, (t - 4) % 2)]

            # out-proj partials: blocks available before pass 3 ends
            # (own 0-2 + all of recv pass 0-2); finals add own3 + recv3.
            PARTIAL_BLOCKS = [0, 1, 2, 4, 5, 6, 7, 8, 9]
            FINAL_BLOCKS = [3, 10, 11]
            stg_tiles = {}

            def partial_group(oh, tokt):
                cell = {}

                def mm(i):
                    if i == 0:
                        cell["ps"] = ps_gen.tile(
                            [128, 512], FP32, tag="gen", name=f"pps{oh}_{tokt}"
                        )
                    t = PARTIAL_BLOCKS[i]
                    nc.tensor.matmul(
                        out=cell["ps"],
                        lhsT=lhs_of(t)[:, tokt * 128 : (tokt + 1) * 128],
                        rhs=wo_sbs[oh][:, t, :],
                        start=(i == 0),
                        stop=(i == len(PARTIAL_BLOCKS) - 1),
                    )

                def fin():
                    stg = stg_pool.tile(
                        [128, 512], FP32, tag="stg", name=f"stg{oh}_{tokt}"
                    )
                    nc.vector.tensor_add(
                        out=stg,
                        in0=cell["ps"],
                        in1=bo_bc[:, oh * 512 : (oh + 1) * 512],
                    )
                    stg_tiles[(oh, tokt)] = stg

                return [
                    lambda i=i: mm(i) for i in range(len(PARTIAL_BLOCKS))
                ] + [fin]

            emit_pass(0)
            fill_q.extend(
                wkq_dma(2)
                + kt_alloc(2)
                + [c for ts in range(4) for c in k_group(2, ts)]
                + [c for qs in range(4) for c in q_group(2, qs)]
            )
            emit_pass(1)
            fill_q.extend(
                wkq_dma(3)
                + kt_alloc(3)
                + [c for ts in range(4) for c in k_group(3, ts)]
                + [c for qs in range(4) for c in q_group(3, qs)]
                + recv_dma(0, 0)
                + recv_dma(0, 1)
                + [c for oh in range(2) for c in wo_dma(oh)]
            )
            emit_pass(2)
            fill_q.extend(recv_dma(1, 0) + recv_dma(1, 1))
            flush()
            # xT and K/Q weights are dead: release their 72KB for staging
            inner_es.close()
            stg_pool = _es.enter_context(tc.tile_pool(name="stg", bufs=16))

            fill_q.extend(
                recv_dma(2, 0)
                + recv_dma(2, 1)
                + [
                    c
                    for oh in range(2)
                    for tokt in range(8)
                    for c in partial_group(oh, tokt)
                ]
            )
            emit_pass(3)
            flush()
            for cl in recv_dma(3, 0) + recv_dma(3, 1):
                cl()

            # ---- finals: own3 + recv3 blocks on top of the staged partials
            for oh in range(2):
                for tokt in range(8):
                    ps = ps_gen.tile([128, 512], FP32, tag="gen")
                    for i, t in enumerate(FINAL_BLOCKS):
                        nc.tensor.matmul(
                            out=ps,
                            lhsT=lhs_of(t)[:, tokt * 128 : (tokt + 1) * 128],
                            rhs=wo_sbs[oh][:, t, :],
                            start=(i == 0),
                            stop=(i == len(FINAL_BLOCKS) - 1),
                        )
                    ostg = ostg_pool.tile([128, 512], FP32, tag="ostg")
                    nc.vector.tensor_add(
                        out=ostg, in0=ps, in1=stg_tiles[(oh, tokt)]
                    )
                    nc.sync.dma_start(
                        out=out[
                            tokt * 128 : (tokt + 1) * 128,
                            oh * 512 : (oh + 1) * 512,
                        ],
                        in_=ostg,
                    )

    nc.compile()
    return nc


def kernel(x, wq, bq, wk, bk, wv, bv, wo, bo, trace=False):
    x = np.asarray(x, dtype=np.float32)
    wq = np.asarray(wq, dtype=np.float32)
    bq = np.asarray(bq, dtype=np.float32)
    wk = np.asarray(wk, dtype=np.float32)
    bk = np.asarray(bk, dtype=np.float32)
    wv = np.asarray(wv, dtype=np.float32)
    bv = np.asarray(bv, dtype=np.float32)
    wo = np.asarray(wo, dtype=np.float32)
    bo = np.asarray(bo, dtype=np.float32)

    if "nc" not in _CACHE:
        _CACHE["nc"] = build_nc()
    nc = _CACHE["nc"]

    wq_f = wq.transpose(1, 0, 2).reshape(D, H * E)  # [D, heads*E] head-major cols
    wk_f = wk.transpose(1, 0, 2).reshape(D, H * E)
    wv_f = wv.transpose(1, 0, 2).reshape(D, H * E)
    wo_t = wo.T  # [in 1024, out 1024], in-dim = global head-major
    bo_row = np.ascontiguousarray(bo.reshape(1, D))

    in_maps = []
    for c in range(NCORES):
        b, hg = c // 2, c % 2
        cs = slice(hg * HL * E, (hg + 1) * HL * E)
        xT_b = x[b].T
        if hg == 0:
            xT_c = np.ascontiguousarray(xT_b)
        else:
            # own out-tokens first; K/V/Q all share this local token order
            xT_c = np.ascontiguousarray(
                np.concatenate([xT_b[:, TQ:], xT_b[:, :TQ]], axis=1)
            )
        # augmented wo: 4 own head-pair blocks, then (p, g) gathered blocks
        # with the loopback (g == hg) blocks zeroed
        wo_aug = np.zeros((12 * 128, D), dtype=np.float32)
        for p_ in range(4):
            wo_aug[p_ * 128 : (p_ + 1) * 128] = wo_t[
                hg * 512 + p_ * 128 : hg * 512 + (p_ + 1) * 128
            ]
        for p_ in range(4):
            for g_ in range(2):
                if g_ == hg:
                    continue
                t_ = 4 + p_ * 2 + g_
                wo_aug[t_ * 128 : (t_ + 1) * 128] = wo_t[
                    g_ * 512 + p_ * 128 : g_ * 512 + (p_ + 1) * 128
                ]
        m = {
            "xT": xT_c.astype(ml_dtypes.bfloat16),
            "wq_t": np.ascontiguousarray(wq_f[:, cs]).astype(ml_dtypes.bfloat16),
            "wk_t": np.ascontiguousarray(wk_f[:, cs]).astype(ml_dtypes.bfloat16),
            "wv_t": np.ascontiguousarray(wv_f[:, cs]).astype(ml_dtypes.bfloat16),
            "wo_b": np.ascontiguousarray(wo_aug).astype(ml_dtypes.bfloat16),
            "bqp": np.ascontiguousarray(
                bq.reshape(H * E)[cs].reshape(4, 128).T
            ),
            "bkp": np.ascontiguousarray(
                bk.reshape(H * E)[cs].reshape(4, 128).T
            ),
            "bv_row": np.ascontiguousarray(bv.reshape(1, H * E)[:, cs]),
            "bo_row": bo_row,
        }
        in_maps.append(m)

    res = run_bass_kernel_spmd(nc, in_maps, list(range(NCORES)), trace=trace)

    out = np.empty((B, S, D), dtype=np.float32)
    for c in range(NCORES):
        b, hg = c // 2, c % 2
        out[b, hg * TQ : (hg + 1) * TQ, :] = res.results[c]["out"]
    if trace:
        return out, res
    return out


# revision 13
# speedup vs baseline: 1.4423x; 1.0251x over previous
"""Multi-head attention (B=4, S=2048, D=1024, H=16, E=64) on 8 TRN2 NeuronCores.

Sharding: core c handles batch b=c//2 and head-group hg=c%2 (8 heads) over the
full 2048-token sequence; after each 2-head pass the peer-token half of the
attention output is exchanged pairwise (AllGather over [2c,2c+1]) and each
core runs the output projection for its own 1024 tokens over all 16 heads
(augmented wo layout with zeroed loopback blocks, as the program is SPMD).

Kernel structure (v2 — uniform 128x128 PE mode, exp/PE balanced pipeline):
  * All matmuls use the full 128-row PE config; the per-head scores matmuls
    (contraction = E = 64) are padded to 128 contraction rows via a
    zero-padded Q layout qt[128, 2, S]: strip 0 holds head A's Q in rows
    0-63 (rows 64-127 zero), strip 1 head B's in rows 64-127.  lhsT is the
    shared kt[:, kt-tile] so K rows of the other head multiply zeros.  This
    avoids the (64,128)<->(128,128) tiling-mode drains the PE would pay when
    interleaving scores and att@V matmuls.
  * Inner loop per (pass, tqt of 512 queries, kt of 128 keys): 2 scores
    matmuls (one per head, adjacent PSUM banks), one exp ACTIVATE over
    [128, 2, 512] (N=1024), 2 att@V accumulation matmuls (M=65: V plus a
    ones column that yields the softmax sums in PSUM row 64).
  * V is computed once (fp32r) and kept resident in SBUF as bf16
    [128 tok-part, 16 tok-tile, 4 head, 65] (two quads) — no DRAM spill.
  * K/Q are stored bf16 (scores matmul in bf16; PSUM accumulates fp32).
  * Softmax normalize: fast PSUM evacuation (sums row + 64 att rows copied
    to SBUF by DVE to free the bank), reciprocal_approx_fast, GpSimd
    partition broadcast, DVE multiply straight into the own-half tile or the
    exchange staging tile.
  * A filler queue interleaves the V/K/Q projection matmuls (and late DMAs)
    into the exp-bound attention loop at ~2 instructions per kt so the PE
    never idles (keeps the tensor-engine DVFS p-state at max clock).
"""

import numpy as np
import ml_dtypes

import concourse.bass as bass
import concourse.mybir as mybir
import concourse.tile as tile
from concourse import bacc
from concourse.bass_utils import run_bass_kernel_spmd

FP32 = mybir.dt.float32
FP32R = mybir.dt.float32r
BF16 = mybir.dt.bfloat16
AF = mybir.ActivationFunctionType

B, S, D, H, E = 4, 2048, 1024, 16, 64
NCORES = 8
TQ = S // 2  # tokens per core for the output projection
HL = H // 2  # local heads per core
SCALE = 1.0 / float(np.sqrt(E))
PAIRS = [[0, 1], [2, 3], [4, 5], [6, 7]]

_CACHE = {}


def build_nc():
    nc = bacc.Bacc("TRN2", target_bir_lowering=False)

    # inputs arrive pre-transposed to the SBUF layout: partition-major so
    # each DMA is 128 large contiguous runs (descriptor-rate matters)
    xT = nc.dram_tensor("xT", [128, 8, S], BF16, kind="ExternalInput")
    wq_t = nc.dram_tensor("wq_t", [128, 8, HL * E], BF16, kind="ExternalInput")
    wk_t = nc.dram_tensor("wk_t", [128, 8, HL * E], BF16, kind="ExternalInput")
    wv_t = nc.dram_tensor("wv_t", [128, 8, HL * E], BF16, kind="ExternalInput")
    wo_b = nc.dram_tensor("wo_b", [128, 12, D], BF16, kind="ExternalInput")
    bqp = nc.dram_tensor("bqp", [128, 4], FP32, kind="ExternalInput")
    bkp = nc.dram_tensor("bkp", [128, 4], FP32, kind="ExternalInput")
    bv_row = nc.dram_tensor("bv_row", [1, HL * E], FP32R, kind="ExternalInput")
    bo_row = nc.dram_tensor("bo_row", [1, D], FP32R, kind="ExternalInput")
    out = nc.dram_tensor("out", [TQ, D], FP32, kind="ExternalOutput")
    wq_r4 = wq_t.rearrange("p k (t m) -> p t k m", m=128)  # [128, 4, 8, 128]
    wk_r4 = wk_t.rearrange("p k (t m) -> p t k m", m=128)
    att_gin = nc.dram_tensor("att_gin", [4, 2, 128, 512], BF16)
    att_gout = nc.dram_tensor("att_gout", [4, 2, 2, 128, 512], BF16)


    from contextlib import ExitStack

    with tile.TileContext(nc) as tc:
        with ExitStack() as _es:
            qt_pool = _es.enter_context(tc.tile_pool(name="qtp", bufs=2))
            kt_pool = _es.enter_context(tc.tile_pool(name="ktp", bufs=2))
            v_pool = _es.enter_context(tc.tile_pool(name="vp", bufs=2))
            w_pool = _es.enter_context(tc.tile_pool(name="wp", bufs=4))
            exp_pool = _es.enter_context(tc.tile_pool(name="expp", bufs=4))
            own_pool = _es.enter_context(tc.tile_pool(name="own", bufs=4))
            gin_pool = _es.enter_context(tc.tile_pool(name="gin", bufs=2))
            recv_pool = _es.enter_context(tc.tile_pool(name="recv", bufs=8))
            stage_pool = _es.enter_context(tc.tile_pool(name="astg", bufs=2))
            ostg_pool = _es.enter_context(tc.tile_pool(name="ostg", bufs=2))
            small_pool = _es.enter_context(tc.tile_pool(name="small", bufs=2))
            rb_pool = _es.enter_context(tc.tile_pool(name="rbp", bufs=2))
            ones_pool = _es.enter_context(tc.tile_pool(name="ones", bufs=1))
            ps_sc = _es.enter_context(tc.tile_pool(name="ps_s", bufs=2, space="PSUM"))
            ps_att = _es.enter_context(tc.tile_pool(name="ps_a", bufs=2, space="PSUM"))
            ps_gen = _es.enter_context(tc.tile_pool(name="ps_g", bufs=2, space="PSUM"))

            # inner scope: released after pass 2 so the staging pool can
            # reuse the 72KB (xt + K/Q weights are dead by then)
            inner_es = ExitStack()
            xt_pool = inner_es.enter_context(tc.tile_pool(name="xt", bufs=1))
            wkq_pool = inner_es.enter_context(tc.tile_pool(name="wkq", bufs=2))

            # ---- persistent tiles ----
            xt_sb = xt_pool.tile([128, 8, S], BF16, tag="xt")  # 32KB/part

            ones_col_f = ones_pool.tile([128, 4], FP32, tag="onescf")
            nc.vector.memset(ones_col_f, 1.0)
            ones_col = ones_pool.tile([128, 4], BF16, tag="onescol")
            nc.vector.tensor_copy(out=ones_col, in_=ones_col_f)
            # tiny dummy exp: preload the ACT exp table during the xT DMA
            exp_warm = ones_pool.tile([1, 4], FP32, tag="expwarm")
            nc.scalar.activation(
                out=exp_warm, in_=ones_col_f[0:1, :], func=AF.Exp, scale=1.0
            )
            bq_sb = ones_pool.tile([128, 4], FP32, tag="bq")
            bk_sb = ones_pool.tile([128, 4], FP32, tag="bk")
            nc.sync.dma_start(out=bq_sb, in_=bqp[:, :])
            nc.sync.dma_start(out=bk_sb, in_=bkp[:, :])

            bv_bc = ones_pool.tile([128, HL * E], FP32R, tag="bvbc")
            bo_bc = ones_pool.tile([128, D], FP32R, tag="bobc")
            nc.sync.dma_start(
                out=bv_bc, in_=bv_row[:, :].to_broadcast([128, HL * E])
            )
            nc.sync.dma_start(out=bo_bc, in_=bo_row[:, :].to_broadcast([128, D]))

            # V resident in SBUF: quad q holds heads 4q..4q+3:
            # [tok-in-tile(part), tok-tile, head, E+1]
            v_tiles = [
                v_pool.tile([128, 16, 4, E + 1], BF16, tag="vsb", name=f"v{q}")
                for q in range(2)
            ]
            # zero-padded Q, ping-pong across passes: strip 0 = head A rows
            # 0-63 (rest zero), strip 1 = head B rows 64-127 (rest zero)
            qt_tiles = [
                qt_pool.tile([128, 2, S], BF16, tag="qt", name=f"qt{i}")
                for i in range(2)
            ]
            for i in range(2):
                nc.vector.memset(qt_tiles[i][64:128, 0, :], 0.0)
                nc.vector.memset(qt_tiles[i][0:64, 1, :], 0.0)

            # ---- filler queue ----
            fill_q = []

            def step(n=2):
                if len(fill_q) > 120:
                    n += 1
                for _ in range(n):
                    if fill_q:
                        fill_q.pop(0)()

            def flush():
                while fill_q:
                    fill_q.pop(0)()

            done_marks = set()

            def mark(tag):
                def m():
                    done_marks.add(tag)

                return [m]

            def ensure(tag):
                while tag not in done_marks and fill_q:
                    fill_q.pop(0)()

            # ---- projection group emitters (closure lists) ----
            wv_sbs = {}

            def wv_dma(q):
                def go():
                    if "wv" not in wv_sbs:
                        wv_sbs["wv"] = w_pool.tile(
                            [128, 8, HL * E], BF16, tag="wp", name="wv"
                        )
                        nc.sync.dma_start(out=wv_sbs["wv"], in_=wv_t[:, :, :])
                    wv_sbs[q] = wv_sbs["wv"]

                return [go]

            def v_group(q, tokt):
                cell = {}

                def mm(k):
                    if k == 0:
                        cell["ps"] = ps_gen.tile(
                            [128, 256], FP32, tag="gen", name=f"vps{q}_{tokt}"
                        )
                    nc.tensor.matmul(
                        out=cell["ps"],
                        lhsT=xt_sb[:, k, tokt * 128 : (tokt + 1) * 128],
                        rhs=wv_sbs[q][:, k, q * 256 : (q + 1) * 256],
                        start=(k == 0),
                        stop=(k == 7),
                    )

                def fin():
                    ps = cell["ps"]
                    nc.vector.tensor_add(
                        out=v_tiles[q][:, tokt, :, :E],
                        in0=ps.rearrange("p (h e) -> p h e", e=E),
                        in1=bv_bc[:, q * 256 : (q + 1) * 256].rearrange(
                            "p (h e) -> p h e", e=E
                        ),
                    )
                    nc.vector.tensor_copy(
                        out=v_tiles[q][:, tokt, :, E : E + 1],
                        in_=ones_col[:, :4].unsqueeze(2),
                    )

                return (
                    [lambda k=k: mm(k) for k in range(8)]
                    + [fin]
                    + mark((f"v{q}", tokt))
                )

            wkq_sbs = {}

            def wkq_dma(p):
                def go():
                    wk_sb = wkq_pool.tile(
                        [128, 8, 128], BF16, tag="wk", name=f"wk{p}"
                    )
                    wq_sb = wkq_pool.tile(
                        [128, 8, 128], BF16, tag="wq", name=f"wq{p}"
                    )
                    nc.sync.dma_start(
                        out=wk_sb, in_=wk_r4[:, p, :, :]
                    )
                    nc.sync.dma_start(
                        out=wq_sb, in_=wq_r4[:, p, :, :]
                    )
                    wkq_sbs[p] = (wk_sb, wq_sb)

                return [go]

            kt_sbs = {}

            def kt_alloc(p):
                def go():
                    kt_sbs[p] = kt_pool.tile([128, S], BF16, tag="kt", name=f"kt{p}")

                return [go]

            def k_group(p, ts):
                cell = {}

                def mm(k):
                    if k == 0:
                        cell["ps"] = ps_gen.tile(
                            [128, 512], FP32, tag="gen", name=f"kps{p}_{ts}"
                        )
                    nc.tensor.matmul(
                        out=cell["ps"],
                        lhsT=wkq_sbs[p][0][:, k, :],
                        rhs=xt_sb[:, k, ts * 512 : (ts + 1) * 512],
                        start=(k == 0),
                        stop=(k == 7),
                    )

                def fin():
                    nc.vector.tensor_scalar_add(
                        out=kt_sbs[p][:, ts * 512 : (ts + 1) * 512],
                        in0=cell["ps"],
                        scalar1=bk_sb[:, p : p + 1],
                    )

                return (
                    [lambda k=k: mm(k) for k in range(8)]
                    + [fin]
                    + mark(("k", p, ts))
                )

            def q_group(p, qs):
                cell = {}
                qt_t = qt_tiles[p % 2]

                def mm(k):
                    if k == 0:
                        cell["ps"] = ps_gen.tile(
                            [128, 512], FP32, tag="gen", name=f"qps{p}_{qs}"
                        )
                    nc.tensor.matmul(
                        out=cell["ps"],
                        lhsT=wkq_sbs[p][1][:, k, :],
                        rhs=xt_sb[:, k, qs * 512 : (qs + 1) * 512],
                        start=(k == 0),
                        stop=(k == 7),
                    )

                def fin():
                    ps = cell["ps"]
                    nc.vector.tensor_scalar_add(
                        out=qt_t[0:64, 0, qs * 512 : (qs + 1) * 512],
                        in0=ps[0:64, :],
                        scalar1=bq_sb[0:64, p : p + 1],
                    )
                    nc.vector.tensor_scalar_add(
                        out=qt_t[64:128, 1, qs * 512 : (qs + 1) * 512],
                        in0=ps[64:128, :],
                        scalar1=bq_sb[64:128, p : p + 1],
                    )

                return (
                    [lambda k=k: mm(k) for k in range(8)]
                    + [fin]
                    + mark(("q", p, qs))
                )

            # ---- upfront: weights first (small DMAs ahead of the 4MB xT),
            # then just enough compute for pass-0 tqt0's first key tiles ----
            for cl in wv_dma(0) + wkq_dma(0):
                cl()
            for k in range(8):
                nc.sync.dma_start(out=xt_sb[:, k, :], in_=xT[:, k, :])
            for cl in (
                kt_alloc(0)
                + [c for ts in range(2) for c in k_group(0, ts)]
                + q_group(0, 0)
                + [c for t in range(4) for c in v_group(0, t)]
            ):
                cl()

            # filler for pass 0: rest of pass-0 K/Q/V, V quad 1, pass-1 K/Q.
            # ensure() marks make consumers wait for these.
            fill_q.extend(
                [c for t in range(4, 6) for c in v_group(0, t)]
                + [c for c in k_group(0, 2)]
                + [c for t in range(6, 8) for c in v_group(0, t)]
                + [c for c in k_group(0, 3)]
                + q_group(0, 1)
                + [c for t in range(8, 16) for c in v_group(0, t)]
                + q_group(0, 2)
                + q_group(0, 3)
                + wv_dma(1)
                + wkq_dma(1)
                + kt_alloc(1)
                + [c for ts in range(4) for c in k_group(1, ts)]
                + [c for qs in range(4) for c in q_group(1, qs)]
                + [c for t in range(16) for c in v_group(1, t)]
            )

            own_tiles = []
            wo_sbs = {}
            recv_tiles = {}

            def wo_dma(oh):
                def go():
                    wo_sbs[oh] = w_pool.tile(
                        [128, 12, 512], BF16, tag="wp", name=f"wo{oh}"
                    )
                    nc.sync.dma_start(
                        out=wo_sbs[oh], in_=wo_b[:, :, oh * 512 : (oh + 1) * 512]
                    )

                return [go]

            def recv_dma(p, g, halves=(0, 1)):
                def go():
                    if (p, g) not in recv_tiles:
                        recv_tiles[(p, g)] = recv_pool.tile(
                            [128, TQ], BF16, tag="recv", name=f"rc{p}{g}"
                        )
                    rt = recv_tiles[(p, g)]
                    for h in halves:
                        nc.sync.dma_start(
                            out=rt[:, h * 512 : (h + 1) * 512],
                            in_=att_gout[p, h, g],
                        )

                return [go]

            # ---- attention passes ----
            def emit_pass(p):
                qt_t = qt_tiles[p % 2]
                kt_t = kt_sbs[p]
                vq = 0 if p < 2 else 1
                vA, vB = 2 * (p % 2), 2 * (p % 2) + 1
                v_t = v_tiles[vq]

                own_t = own_pool.tile([128, TQ], BF16, tag="own", name=f"own{p}")
                own_tiles.append(own_t)
                gin_t = gin_pool.tile([128, TQ], BF16, tag="gin", name=f"gin{p}")

                for tqt in range(4):
                    ensure(("q", p, tqt))
                    att_A = ps_att.tile([E + 1, 512], FP32, tag="att")
                    att_B = ps_att.tile([E + 1, 512], FP32, tag="att")
                    for kt in range(16):
                        ensure(("k", p, kt // 4))
                        ensure((f"v{vq}", kt))
                        ps = ps_sc.tile([128, 2, 512], FP32, tag="sc")
                        nc.tensor.matmul(
                            out=ps[:, 0, :],
                            lhsT=kt_t[:, kt * 128 : (kt + 1) * 128],
                            rhs=qt_t[:, 0, tqt * 512 : (tqt + 1) * 512],
                            start=True,
                            stop=True,
                        )
                        nc.tensor.matmul(
                            out=ps[:, 1, :],
                            lhsT=kt_t[:, kt * 128 : (kt + 1) * 128],
                            rhs=qt_t[:, 1, tqt * 512 : (tqt + 1) * 512],
                            start=True,
                            stop=True,
                        )
                        ex = exp_pool.tile([128, 2, 512], BF16, tag="exp")
                        nc.scalar.activation(out=ex, in_=ps, func=AF.Exp, scale=SCALE)
                        nc.tensor.matmul(
                            out=att_A,
                            lhsT=v_t[:, kt, vA, :],
                            rhs=ex[:, 0, :],
                            start=(kt == 0),
                            stop=(kt == 15),
                        )
                        nc.tensor.matmul(
                            out=att_B,
                            lhsT=v_t[:, kt, vB, :],
                            rhs=ex[:, 1, :],
                            start=(kt == 0),
                            stop=(kt == 15),
                        )
                        step(2)

                    for hh, att_ps in ((0, att_A), (1, att_B)):
                        # fast PSUM evacuation, then normalize from SBUF
                        sums = small_pool.tile([1, 512], FP32, tag="sums", bufs=1)
                        nc.vector.tensor_copy(out=sums, in_=att_ps[E : E + 1, :])
                        a_sb = stage_pool.tile([64, 512], FP32, tag="astg")
                        nc.vector.tensor_copy(out=a_sb, in_=att_ps[:E, :])
                        recip = small_pool.tile([1, 512], FP32, tag="recip", bufs=1)
                        nc.vector.reciprocal_approx_fast(out=recip, in_=sums)
                        rb = rb_pool.tile([64, 512], FP32, tag="rbb")
                        nc.gpsimd.partition_broadcast(rb, recip)
                        if tqt < 2:
                            dest = own_t[
                                hh * 64 : (hh + 1) * 64,
                                tqt * 512 : (tqt + 1) * 512,
                            ]
                        else:
                            dest = gin_t[
                                hh * 64 : (hh + 1) * 64,
                                (tqt - 2) * 512 : (tqt - 1) * 512,
                            ]
                        nc.vector.tensor_mul(out=dest, in0=a_sb, in1=rb)

                    # pair-exchange each 512-token half as soon as its
                    # drains land, so the last collective hides in pass 3
                    if tqt >= 2:
                        h = tqt - 2
                        nc.sync.dma_start(
                            out=att_gin[p, h],
                            in_=gin_t[:, h * 512 : (h + 1) * 512],
                        )
                        nc.gpsimd.collective_compute(
                            kind="AllGather",
                            op=mybir.AluOpType.bypass,
                            replica_groups=PAIRS,
                            ins=[att_gin[p, h]],
                            outs=[att_gout[p, h]],
                        )
                        if p == 3:
                            for cl in recv_dma(3, 0, (h,)) + recv_dma(
                                3, 1, (h,)
                            ):
                                cl()

            def lhs_of(t):
                if t < 4:
                    return own_tiles[t]
                return recv_tiles[((t - 4) // 2, (t - 4) % 2)]

            # out-proj partials: blocks available before pass 3 ends
            # (own 0-2 + all of recv pass 0-2); finals add own3 + recv3.
            PARTIAL_BLOCKS = [0, 1, 2, 4, 5, 6, 7, 8, 9]
            FINAL_BLOCKS = [3, 10, 11]
            stg_tiles = {}

            def partial_group(oh, tokt):
                cell = {}

                def mm(i):
                    if i == 0:
                        cell["ps"] = ps_gen.tile(
                            [128, 512], FP32, tag="gen", name=f"pps{oh}_{tokt}"
                        )
                    t = PARTIAL_BLOCKS[i]
                    nc.tensor.matmul(
                        out=cell["ps"],
                        lhsT=lhs_of(t)[:, tokt * 128 : (tokt + 1) * 128],
                        rhs=wo_sbs[oh][:, t, :],
                        start=(i == 0),
                        stop=(i == len(PARTIAL_BLOCKS) - 1),
                    )

                def fin():
                    stg = stg_pool.tile(
                        [128, 512], FP32, tag="stg", name=f"stg{oh}_{tokt}"
                    )
                    nc.vector.tensor_add(
                        out=stg,
                        in0=cell["ps"],
                        in1=bo_bc[:, oh * 512 : (oh + 1) * 512],
                    )
                    stg_tiles[(oh, tokt)] = stg

                return [
                    lambda i=i: mm(i) for i in range(len(PARTIAL_BLOCKS))
                ] + [fin]

            emit_pass(0)
            fill_q.extend(
                wkq_dma(2)
                + kt_alloc(2)
                + [c for ts in range(4) for c in k_group(2, ts)]
                + [c for qs in range(4) for c in q_group(2, qs)]
            )
            emit_pass(1)
            fill_q.extend(
                wkq_dma(3)
                + kt_alloc(3)
                + [c for ts in range(4) for c in k_group(3, ts)]
                + [c for qs in range(4) for c in q_group(3, qs)]
                + recv_dma(0, 0)
                + recv_dma(0, 1)
                + [c for oh in range(2) for c in wo_dma(oh)]
            )
            emit_pass(2)
            fill_q.extend(recv_dma(1, 0) + recv_dma(1, 1))
            flush()
            # xT and K/Q weights are dead: release their 72KB for staging
            inner_es.close()
            stg_pool = _es.enter_context(tc.tile_pool(name="stg", bufs=16))

            fill_q.extend(
                recv_dma(2, 0)
                + recv_dma(2, 1)
                + [
                    c
                    for oh in range(2)
                    for tokt in range(8)
                    for c in partial_group(oh, tokt)
                ]
            )
            emit_pass(3)
            flush()

            # ---- finals: own3 + recv3 blocks on top of the staged partials
            for oh in range(2):
                for tokt in range(8):
                    ps = ps_gen.tile([128, 512], FP32, tag="gen")
                    for i, t in enumerate(FINAL_BLOCKS):
                        nc.tensor.matmul(
                            out=ps,
                            lhsT=lhs_of(t)[:, tokt * 128 : (tokt + 1) * 128],
                            rhs=wo_sbs[oh][:, t, :],
                            start=(i == 0),
                            stop=(i == len(FINAL_BLOCKS) - 1),
                        )
                    ostg = ostg_pool.tile([128, 512], FP32, tag="ostg")
                    nc.vector.tensor_add(
                        out=ostg, in0=ps, in1=stg_tiles[(oh, tokt)]
                    )
                    nc.sync.dma_start(
                        out=out[
                            tokt * 128 : (tokt + 1) * 128,
                            oh * 512 : (oh + 1) * 512,
                        ],
                        in_=ostg,
                    )

    nc.compile()
    return nc


def kernel(x, wq, bq, wk, bk, wv, bv, wo, bo, trace=False):
    x = np.asarray(x, dtype=np.float32)
    wq = np.asarray(wq, dtype=np.float32)
    bq = np.asarray(bq, dtype=np.float32)
    wk = np.asarray(wk, dtype=np.float32)
    bk = np.asarray(bk, dtype=np.float32)
    wv = np.asarray(wv, dtype=np.float32)
    bv = np.asarray(bv, dtype=np.float32)
    wo = np.asarray(wo, dtype=np.float32)
    bo = np.asarray(bo, dtype=np.float32)

    if "nc" not in _CACHE:
        _CACHE["nc"] = build_nc()
    nc = _CACHE["nc"]

    wq_f = wq.transpose(1, 0, 2).reshape(D, H * E)  # [D, heads*E] head-major cols
    wk_f = wk.transpose(1, 0, 2).reshape(D, H * E)
    wv_f = wv.transpose(1, 0, 2).reshape(D, H * E)
    wo_t = wo.T  # [in 1024, out 1024], in-dim = global head-major
    bo_row = np.ascontiguousarray(bo.reshape(1, D))

    in_maps = []
    for c in range(NCORES):
        b, hg = c // 2, c % 2
        cs = slice(hg * HL * E, (hg + 1) * HL * E)
        xT_b = x[b].T
        if hg == 0:
            xT_c = np.ascontiguousarray(xT_b)
        else:
            # own out-tokens first; K/V/Q all share this local token order
            xT_c = np.ascontiguousarray(
                np.concatenate([xT_b[:, TQ:], xT_b[:, :TQ]], axis=1)
            )
        # augmented wo: 4 own head-pair blocks, then (p, g) gathered blocks
        # with the loopback (g == hg) blocks zeroed
        wo_aug = np.zeros((12 * 128, D), dtype=np.float32)
        for p_ in range(4):
            wo_aug[p_ * 128 : (p_ + 1) * 128] = wo_t[
                hg * 512 + p_ * 128 : hg * 512 + (p_ + 1) * 128
            ]
        for p_ in range(4):
            for g_ in range(2):
                if g_ == hg:
                    continue
                t_ = 4 + p_ * 2 + g_
                wo_aug[t_ * 128 : (t_ + 1) * 128] = wo_t[
                    g_ * 512 + p_ * 128 : g_ * 512 + (p_ + 1) * 128
                ]
        def pmajor(a, nt):
            # [nt*128, m] -> [128, nt, m] partition-major contiguous
            return np.ascontiguousarray(
                a.reshape(nt, 128, a.shape[1]).transpose(1, 0, 2)
            )

        m = {
            "xT": pmajor(xT_c, 8).astype(ml_dtypes.bfloat16),
            "wq_t": pmajor(wq_f[:, cs], 8).astype(ml_dtypes.bfloat16),
            "wk_t": pmajor(wk_f[:, cs], 8).astype(ml_dtypes.bfloat16),
            "wv_t": pmajor(wv_f[:, cs], 8).astype(ml_dtypes.bfloat16),
            "wo_b": pmajor(wo_aug, 12).astype(ml_dtypes.bfloat16),
            "bqp": np.ascontiguousarray(
                bq.reshape(H * E)[cs].reshape(4, 128).T
            ),
            "bkp": np.ascontiguousarray(
                bk.reshape(H * E)[cs].reshape(4, 128).T
            ),
            "bv_row": np.ascontiguousarray(bv.reshape(1, H * E)[:, cs]),
            "bo_row": bo_row,
        }
        in_maps.append(m)

    res = run_bass_kernel_spmd(nc, in_maps, list(range(NCORES)), trace=trace)

    out = np.empty((B, S, D), dtype=np.float32)
    for c in range(NCORES):
        b, hg = c // 2, c % 2
        out[b, hg * TQ : (hg + 1) * TQ, :] = res.results[c]["out"]
    if trace:
        return out, res
    return out


# revision 14
# speedup vs baseline: 1.4990x; 1.0393x over previous
"""Multi-head attention (B=4, S=2048, D=1024, H=16, E=64) on 8 TRN2 NeuronCores.

Sharding: core c handles batch b=c//2 and head-group hg=c%2 (8 heads) over the
full 2048-token sequence; after each 2-head pass the peer-token half of the
attention output is exchanged pairwise (AllGather over [2c,2c+1]) and each
core runs the output projection for its own 1024 tokens over all 16 heads
(augmented wo layout with zeroed loopback blocks, as the program is SPMD).

Kernel structure (v2 — uniform 128x128 PE mode, exp/PE balanced pipeline):
  * All matmuls use the full 128-row PE config; the per-head scores matmuls
    (contraction = E = 64) are padded to 128 contraction rows via a
    zero-padded Q layout qt[128, 2, S]: strip 0 holds head A's Q in rows
    0-63 (rows 64-127 zero), strip 1 head B's in rows 64-127.  lhsT is the
    shared kt[:, kt-tile] so K rows of the other head multiply zeros.  This
    avoids the (64,128)<->(128,128) tiling-mode drains the PE would pay when
    interleaving scores and att@V matmuls.
  * Inner loop per (pass, tqt of 512 queries, kt of 128 keys): 2 scores
    matmuls (one per head, adjacent PSUM banks), one exp ACTIVATE over
    [128, 2, 512] (N=1024), 2 att@V accumulation matmuls (M=65: V plus a
    ones column that yields the softmax sums in PSUM row 64).
  * V is computed once (fp32r) and kept resident in SBUF as bf16
    [128 tok-part, 16 tok-tile, 4 head, 65] (two quads) — no DRAM spill.
  * K/Q are stored bf16 (scores matmul in bf16; PSUM accumulates fp32).
  * Softmax normalize: fast PSUM evacuation (sums row + 64 att rows copied
    to SBUF by DVE to free the bank), reciprocal_approx_fast, GpSimd
    partition broadcast, DVE multiply straight into the own-half tile or the
    exchange staging tile.
  * A filler queue interleaves the V/K/Q projection matmuls (and late DMAs)
    into the exp-bound attention loop at ~2 instructions per kt so the PE
    never idles (keeps the tensor-engine DVFS p-state at max clock).
"""

import numpy as np
import ml_dtypes

import concourse.bass as bass
import concourse.mybir as mybir
import concourse.tile as tile
from concourse import bacc
from concourse.bass_utils import run_bass_kernel_spmd

FP32 = mybir.dt.float32
FP32R = mybir.dt.float32r
BF16 = mybir.dt.bfloat16
AF = mybir.ActivationFunctionType

B, S, D, H, E = 4, 2048, 1024, 16, 64
NCORES = 8
TQ = S // 2  # tokens per core for the output projection
HL = H // 2  # local heads per core
SCALE = 1.0 / float(np.sqrt(E))
PAIRS = [[0, 1], [2, 3], [4, 5], [6, 7]]

_CACHE = {}


def build_nc():
    nc = bacc.Bacc("TRN2", target_bir_lowering=False)

    # inputs arrive pre-transposed to the SBUF layout: partition-major so
    # each DMA is 128 large contiguous runs (descriptor-rate matters)
    xT = nc.dram_tensor("xT", [128, 8, S], BF16, kind="ExternalInput")
    wq_t = nc.dram_tensor("wq_t", [128, 8, HL * E], BF16, kind="ExternalInput")
    wk_t = nc.dram_tensor("wk_t", [128, 8, HL * E], BF16, kind="ExternalInput")
    wv_t = nc.dram_tensor("wv_t", [128, 8, HL * E], BF16, kind="ExternalInput")
    wo_b = nc.dram_tensor("wo_b", [128, 12, D], BF16, kind="ExternalInput")
    bqp = nc.dram_tensor("bqp", [128, 4], FP32, kind="ExternalInput")
    bkp = nc.dram_tensor("bkp", [128, 4], FP32, kind="ExternalInput")
    bv_row = nc.dram_tensor("bv_row", [1, HL * E], FP32R, kind="ExternalInput")
    bo_row = nc.dram_tensor("bo_row", [1, D], FP32R, kind="ExternalInput")
    out = nc.dram_tensor("out", [TQ, D], FP32, kind="ExternalOutput")
    wq_r4 = wq_t.rearrange("p k (t m) -> p t k m", m=128)  # [128, 4, 8, 128]
    wk_r4 = wk_t.rearrange("p k (t m) -> p t k m", m=128)
    att_gin = nc.dram_tensor("att_gin", [4, 2, 128, 512], BF16)
    att_gout = nc.dram_tensor("att_gout", [4, 2, 2, 128, 512], BF16)


    from contextlib import ExitStack

    with tile.TileContext(nc) as tc:
        with ExitStack() as _es:
            qt_pool = _es.enter_context(tc.tile_pool(name="qtp", bufs=2))
            kt_pool = _es.enter_context(tc.tile_pool(name="ktp", bufs=2))
            v_pool = _es.enter_context(tc.tile_pool(name="vp", bufs=2))
            w_pool = _es.enter_context(tc.tile_pool(name="wp", bufs=4))
            exp_pool = _es.enter_context(tc.tile_pool(name="expp", bufs=4))
            own_pool = _es.enter_context(tc.tile_pool(name="own", bufs=4))
            gin_pool = _es.enter_context(tc.tile_pool(name="gin", bufs=2))
            recv_pool = _es.enter_context(tc.tile_pool(name="recv", bufs=8))
            stage_pool = _es.enter_context(tc.tile_pool(name="astg", bufs=2))
            ostg_pool = _es.enter_context(tc.tile_pool(name="ostg", bufs=2))
            small_pool = _es.enter_context(tc.tile_pool(name="small", bufs=2))
            rb_pool = _es.enter_context(tc.tile_pool(name="rbp", bufs=2))
            ones_pool = _es.enter_context(tc.tile_pool(name="ones", bufs=1))
            ps_sc = _es.enter_context(tc.tile_pool(name="ps_s", bufs=2, space="PSUM"))
            ps_att = _es.enter_context(tc.tile_pool(name="ps_a", bufs=2, space="PSUM"))
            ps_gen = _es.enter_context(tc.tile_pool(name="ps_g", bufs=2, space="PSUM"))

            # inner scope: released after pass 2 so the staging pool can
            # reuse the 72KB (xt + K/Q weights are dead by then)
            inner_es = ExitStack()
            xt_pool = inner_es.enter_context(tc.tile_pool(name="xt", bufs=1))
            wkq_pool = inner_es.enter_context(tc.tile_pool(name="wkq", bufs=2))

            # ---- persistent tiles ----
            xt_sb = xt_pool.tile([128, 8, S], BF16, tag="xt")  # 32KB/part

            ones_col_f = ones_pool.tile([128, 4], FP32, tag="onescf")
            nc.vector.memset(ones_col_f, 1.0)
            ones_col = ones_pool.tile([128, 4], BF16, tag="onescol")
            nc.vector.tensor_copy(out=ones_col, in_=ones_col_f)
            # tiny dummy exp: preload the ACT exp table during the xT DMA
            exp_warm = ones_pool.tile([1, 4], FP32, tag="expwarm")
            nc.scalar.activation(
                out=exp_warm, in_=ones_col_f[0:1, :], func=AF.Exp, scale=1.0
            )
            bq_sb = ones_pool.tile([128, 4], FP32, tag="bq")
            bk_sb = ones_pool.tile([128, 4], FP32, tag="bk")
            nc.sync.dma_start(out=bq_sb, in_=bqp[:, :])
            nc.sync.dma_start(out=bk_sb, in_=bkp[:, :])

            bv_sb = ones_pool.tile([1, HL * E], FP32R, tag="bvrow")
            bo_sb = ones_pool.tile([1, D], FP32R, tag="borow")
            nc.sync.dma_start(out=bv_sb, in_=bv_row[:, :])
            nc.sync.dma_start(out=bo_sb, in_=bo_row[:, :])
            bv_bc = ones_pool.tile([128, HL * E], FP32R, tag="bvbc")
            bo_bc = ones_pool.tile([128, D], FP32R, tag="bobc")
            nc.gpsimd.partition_broadcast(bv_bc, bv_sb)
            nc.gpsimd.partition_broadcast(bo_bc, bo_sb)

            # V resident in SBUF: quad q holds heads 4q..4q+3:
            # [tok-in-tile(part), tok-tile, head, E+1]
            v_tiles = [
                v_pool.tile([128, 16, 4, E + 1], BF16, tag="vsb", name=f"v{q}")
                for q in range(2)
            ]
            # zero-padded Q, ping-pong across passes: strip 0 = head A rows
            # 0-63 (rest zero), strip 1 = head B rows 64-127 (rest zero)
            qt_tiles = [
                qt_pool.tile([128, 2, S], BF16, tag="qt", name=f"qt{i}")
                for i in range(2)
            ]
            for i in range(2):
                nc.vector.memset(qt_tiles[i][64:128, 0, :], 0.0)
                nc.vector.memset(qt_tiles[i][0:64, 1, :], 0.0)

            # ---- filler queue ----
            fill_q = []

            def step(n=2):
                if len(fill_q) > 120:
                    n += 1
                for _ in range(n):
                    if fill_q:
                        fill_q.pop(0)()

            def flush():
                while fill_q:
                    fill_q.pop(0)()

            done_marks = set()

            def mark(tag):
                def m():
                    done_marks.add(tag)

                return [m]

            def ensure(tag):
                while tag not in done_marks and fill_q:
                    fill_q.pop(0)()

            # ---- projection group emitters (closure lists) ----
            wv_sbs = {}

            def wv_dma(q):
                def go():
                    if "wv" not in wv_sbs:
                        wv_sbs["wv"] = w_pool.tile(
                            [128, 8, HL * E], BF16, tag="wp", name="wv"
                        )
                        nc.sync.dma_start(out=wv_sbs["wv"], in_=wv_t[:, :, :])
                    wv_sbs[q] = wv_sbs["wv"]

                return [go]

            def v_group(q, tokt):
                cell = {}

                def mm(k):
                    if k == 0:
                        cell["ps"] = ps_gen.tile(
                            [128, 256], FP32, tag="gen", name=f"vps{q}_{tokt}"
                        )
                    nc.tensor.matmul(
                        out=cell["ps"],
                        lhsT=xt_sb[:, k, tokt * 128 : (tokt + 1) * 128],
                        rhs=wv_sbs[q][:, k, q * 256 : (q + 1) * 256],
                        start=(k == 0),
                        stop=(k == 7),
                    )

                def fin():
                    ps = cell["ps"]
                    nc.vector.tensor_add(
                        out=v_tiles[q][:, tokt, :, :E],
                        in0=ps.rearrange("p (h e) -> p h e", e=E),
                        in1=bv_bc[:, q * 256 : (q + 1) * 256].rearrange(
                            "p (h e) -> p h e", e=E
                        ),
                    )
                    nc.vector.tensor_copy(
                        out=v_tiles[q][:, tokt, :, E : E + 1],
                        in_=ones_col[:, :4].unsqueeze(2),
                    )

                return (
                    [lambda k=k: mm(k) for k in range(8)]
                    + [fin]
                    + mark((f"v{q}", tokt))
                )

            wkq_sbs = {}

            def wkq_dma(p):
                def go():
                    wk_sb = wkq_pool.tile(
                        [128, 8, 128], BF16, tag="wk", name=f"wk{p}"
                    )
                    wq_sb = wkq_pool.tile(
                        [128, 8, 128], BF16, tag="wq", name=f"wq{p}"
                    )
                    nc.sync.dma_start(
                        out=wk_sb, in_=wk_r4[:, p, :, :]
                    )
                    nc.sync.dma_start(
                        out=wq_sb, in_=wq_r4[:, p, :, :]
                    )
                    wkq_sbs[p] = (wk_sb, wq_sb)

                return [go]

            kt_sbs = {}

            def kt_alloc(p):
                def go():
                    kt_sbs[p] = kt_pool.tile([128, S], BF16, tag="kt", name=f"kt{p}")

                return [go]

            def k_group(p, ts):
                cell = {}

                def mm(k):
                    if k == 0:
                        cell["ps"] = ps_gen.tile(
                            [128, 512], FP32, tag="gen", name=f"kps{p}_{ts}"
                        )
                    nc.tensor.matmul(
                        out=cell["ps"],
                        lhsT=wkq_sbs[p][0][:, k, :],
                        rhs=xt_sb[:, k, ts * 512 : (ts + 1) * 512],
                        start=(k == 0),
                        stop=(k == 7),
                    )

                def fin():
                    nc.vector.tensor_scalar_add(
                        out=kt_sbs[p][:, ts * 512 : (ts + 1) * 512],
                        in0=cell["ps"],
                        scalar1=bk_sb[:, p : p + 1],
                    )

                return (
                    [lambda k=k: mm(k) for k in range(8)]
                    + [fin]
                    + mark(("k", p, ts))
                )

            def q_group(p, qs):
                cell = {}
                qt_t = qt_tiles[p % 2]

                def mm(k):
                    if k == 0:
                        cell["ps"] = ps_gen.tile(
                            [128, 512], FP32, tag="gen", name=f"qps{p}_{qs}"
                        )
                    nc.tensor.matmul(
                        out=cell["ps"],
                        lhsT=wkq_sbs[p][1][:, k, :],
                        rhs=xt_sb[:, k, qs * 512 : (qs + 1) * 512],
                        start=(k == 0),
                        stop=(k == 7),
                    )

                def fin():
                    ps = cell["ps"]
                    nc.vector.tensor_scalar_add(
                        out=qt_t[0:64, 0, qs * 512 : (qs + 1) * 512],
                        in0=ps[0:64, :],
                        scalar1=bq_sb[0:64, p : p + 1],
                    )
                    nc.vector.tensor_scalar_add(
                        out=qt_t[64:128, 1, qs * 512 : (qs + 1) * 512],
                        in0=ps[64:128, :],
                        scalar1=bq_sb[64:128, p : p + 1],
                    )

                return (
                    [lambda k=k: mm(k) for k in range(8)]
                    + [fin]
                    + mark(("q", p, qs))
                )

            # ---- upfront: weights first (small DMAs ahead of the 4MB xT),
            # then just enough compute for pass-0 tqt0's first key tiles ----
            for cl in wv_dma(0) + wkq_dma(0):
                cl()
            for k in range(8):
                nc.sync.dma_start(out=xt_sb[:, k, :], in_=xT[:, k, :])
            for cl in (
                kt_alloc(0)
                + [c for ts in range(2) for c in k_group(0, ts)]
                + q_group(0, 0)
                + [c for t in range(4) for c in v_group(0, t)]
            ):
                cl()

            # filler for pass 0: rest of pass-0 K/Q/V, V quad 1, pass-1 K/Q.
            # ensure() marks make consumers wait for these.
            fill_q.extend(
                [c for t in range(4, 6) for c in v_group(0, t)]
                + [c for c in k_group(0, 2)]
                + [c for t in range(6, 8) for c in v_group(0, t)]
                + [c for c in k_group(0, 3)]
                + q_group(0, 1)
                + [c for t in range(8, 16) for c in v_group(0, t)]
                + q_group(0, 2)
                + q_group(0, 3)
                + wv_dma(1)
                + wkq_dma(1)
                + kt_alloc(1)
                + [c for ts in range(4) for c in k_group(1, ts)]
                + [c for qs in range(4) for c in q_group(1, qs)]
                + [c for t in range(16) for c in v_group(1, t)]
            )

            own_tiles = []
            wo_sbs = {}
            recv_tiles = {}

            def wo_dma(oh):
                def go():
                    wo_sbs[oh] = w_pool.tile(
                        [128, 12, 512], BF16, tag="wp", name=f"wo{oh}"
                    )
                    nc.sync.dma_start(
                        out=wo_sbs[oh], in_=wo_b[:, :, oh * 512 : (oh + 1) * 512]
                    )

                return [go]

            def recv_dma(p, g, halves=(0, 1)):
                def go():
                    if (p, g) not in recv_tiles:
                        recv_tiles[(p, g)] = recv_pool.tile(
                            [128, TQ], BF16, tag="recv", name=f"rc{p}{g}"
                        )
                    rt = recv_tiles[(p, g)]
                    for h in halves:
                        nc.sync.dma_start(
                            out=rt[:, h * 512 : (h + 1) * 512],
                            in_=att_gout[p, h, g],
                        )

                return [go]

            # ---- attention passes ----
            def emit_pass(p):
                qt_t = qt_tiles[p % 2]
                kt_t = kt_sbs[p]
                vq = 0 if p < 2 else 1
                vA, vB = 2 * (p % 2), 2 * (p % 2) + 1
                v_t = v_tiles[vq]

                own_t = own_pool.tile([128, TQ], BF16, tag="own", name=f"own{p}")
                own_tiles.append(own_t)
                gin_t = gin_pool.tile([128, TQ], BF16, tag="gin", name=f"gin{p}")

                for tqt in range(4):
                    ensure(("q", p, tqt))
                    att_A = ps_att.tile([E + 1, 512], FP32, tag="att")
                    att_B = ps_att.tile([E + 1, 512], FP32, tag="att")
                    for kt in range(16):
                        ensure(("k", p, kt // 4))
                        ensure((f"v{vq}", kt))
                        ps = ps_sc.tile([128, 2, 512], FP32, tag="sc")
                        nc.tensor.matmul(
                            out=ps[:, 0, :],
                            lhsT=kt_t[:, kt * 128 : (kt + 1) * 128],
                            rhs=qt_t[:, 0, tqt * 512 : (tqt + 1) * 512],
                            start=True,
                            stop=True,
                        )
                        nc.tensor.matmul(
                            out=ps[:, 1, :],
                            lhsT=kt_t[:, kt * 128 : (kt + 1) * 128],
                            rhs=qt_t[:, 1, tqt * 512 : (tqt + 1) * 512],
                            start=True,
                            stop=True,
                        )
                        ex = exp_pool.tile([128, 2, 512], BF16, tag="exp")
                        nc.scalar.activation(out=ex, in_=ps, func=AF.Exp, scale=SCALE)
                        nc.tensor.matmul(
                            out=att_A,
                            lhsT=v_t[:, kt, vA, :],
                            rhs=ex[:, 0, :],
                            start=(kt == 0),
                            stop=(kt == 15),
                        )
                        nc.tensor.matmul(
                            out=att_B,
                            lhsT=v_t[:, kt, vB, :],
                            rhs=ex[:, 1, :],
                            start=(kt == 0),
                            stop=(kt == 15),
                        )
                        step(2)

                    for hh, att_ps in ((0, att_A), (1, att_B)):
                        # fast PSUM evacuation, then normalize from SBUF
                        sums = small_pool.tile([1, 512], FP32, tag="sums", bufs=1)
                        nc.vector.tensor_copy(out=sums, in_=att_ps[E : E + 1, :])
                        a_sb = stage_pool.tile([64, 512], FP32, tag="astg")
                        nc.vector.tensor_copy(out=a_sb, in_=att_ps[:E, :])
                        recip = small_pool.tile([1, 512], FP32, tag="recip", bufs=1)
                        nc.vector.reciprocal_approx_fast(out=recip, in_=sums)
                        rb = rb_pool.tile([64, 512], FP32, tag="rbb")
                        nc.gpsimd.partition_broadcast(rb, recip)
                        if tqt < 2:
                            dest = own_t[
                                hh * 64 : (hh + 1) * 64,
                                tqt * 512 : (tqt + 1) * 512,
                            ]
                        else:
                            dest = gin_t[
                                hh * 64 : (hh + 1) * 64,
                                (tqt - 2) * 512 : (tqt - 1) * 512,
                            ]
                        nc.vector.tensor_mul(out=dest, in0=a_sb, in1=rb)

                    # pair-exchange each 512-token half as soon as its
                    # drains land, so the last collective hides in pass 3
                    if tqt >= 2:
                        h = tqt - 2
                        nc.sync.dma_start(
                            out=att_gin[p, h],
                            in_=gin_t[:, h * 512 : (h + 1) * 512],
                        )
                        nc.gpsimd.collective_compute(
                            kind="AllGather",
                            op=mybir.AluOpType.bypass,
                            replica_groups=PAIRS,
                            ins=[att_gin[p, h]],
                            outs=[att_gout[p, h]],
                        )
                        if p == 3:
                            for cl in recv_dma(3, 0, (h,)) + recv_dma(
                                3, 1, (h,)
                            ):
                                cl()

            def lhs_of(t):
                if t < 4:
                    return own_tiles[t]
                return recv_tiles[((t - 4) // 2, (t - 4) % 2)]

            # out-proj partials: blocks available before pass 3 ends
            # (own 0-2 + all of recv pass 0-2); finals add own3 + recv3.
            PARTIAL_BLOCKS = [0, 1, 2, 4, 5, 6, 7, 8, 9]
            FINAL_BLOCKS = [3, 10, 11]
            stg_tiles = {}

            def partial_group(oh, tokt):
                cell = {}

                def mm(i):
                    if i == 0:
                        cell["ps"] = ps_gen.tile(
                            [128, 512], FP32, tag="gen", name=f"pps{oh}_{tokt}"
                        )
                    t = PARTIAL_BLOCKS[i]
                    nc.tensor.matmul(
                        out=cell["ps"],
                        lhsT=lhs_of(t)[:, tokt * 128 : (tokt + 1) * 128],
                        rhs=wo_sbs[oh][:, t, :],
                        start=(i == 0),
                        stop=(i == len(PARTIAL_BLOCKS) - 1),
                    )

                def fin():
                    stg = stg_pool.tile(
                        [128, 512], FP32, tag="stg", name=f"stg{oh}_{tokt}"
                    )
                    nc.vector.tensor_add(
                        out=stg,
                        in0=cell["ps"],
                        in1=bo_bc[:, oh * 512 : (oh + 1) * 512],
                    )
                    stg_tiles[(oh, tokt)] = stg

                return [
                    lambda i=i: mm(i) for i in range(len(PARTIAL_BLOCKS))
                ] + [fin]

            emit_pass(0)
            fill_q.extend(
                wkq_dma(2)
                + kt_alloc(2)
                + [c for ts in range(4) for c in k_group(2, ts)]
                + [c for qs in range(4) for c in q_group(2, qs)]
            )
            emit_pass(1)
            fill_q.extend(
                wkq_dma(3)
                + kt_alloc(3)
                + [c for ts in range(4) for c in k_group(3, ts)]
                + [c for qs in range(4) for c in q_group(3, qs)]
                + recv_dma(0, 0)
                + recv_dma(0, 1)
                + [c for oh in range(2) for c in wo_dma(oh)]
            )
            emit_pass(2)
            fill_q.extend(recv_dma(1, 0) + recv_dma(1, 1))
            flush()
            # xT and K/Q weights are dead: release their 72KB for staging
            inner_es.close()
            stg_pool = _es.enter_context(tc.tile_pool(name="stg", bufs=16))

            fill_q.extend(
                recv_dma(2, 0)
                + recv_dma(2, 1)
                + [
                    c
                    for oh in range(2)
                    for tokt in range(8)
                    for c in partial_group(oh, tokt)
                ]
            )
            emit_pass(3)
            flush()

            # ---- finals: own3 + recv3 blocks on top of the staged partials
            for oh in range(2):
                for tokt in range(8):
                    ps = ps_gen.tile([128, 512], FP32, tag="gen")
                    for i, t in enumerate(FINAL_BLOCKS):
                        nc.tensor.matmul(
                            out=ps,
                            lhsT=lhs_of(t)[:, tokt * 128 : (tokt + 1) * 128],
                            rhs=wo_sbs[oh][:, t, :],
                            start=(i == 0),
                            stop=(i == len(FINAL_BLOCKS) - 1),
                        )
                    ostg = ostg_pool.tile([128, 512], FP32, tag="ostg")
                    nc.vector.tensor_add(
                        out=ostg, in0=ps, in1=stg_tiles[(oh, tokt)]
                    )
                    nc.sync.dma_start(
                        out=out[
                            tokt * 128 : (tokt + 1) * 128,
                            oh * 512 : (oh + 1) * 512,
                        ],
                        in_=ostg,
                    )

    nc.compile()
    return nc


def kernel(x, wq, bq, wk, bk, wv, bv, wo, bo, trace=False):
    x = np.asarray(x, dtype=np.float32)
    wq = np.asarray(wq, dtype=np.float32)
    bq = np.asarray(bq, dtype=np.float32)
    wk = np.asarray(wk, dtype=np.float32)
    bk = np.asarray(bk, dtype=np.float32)
    wv = np.asarray(wv, dtype=np.float32)
    bv = np.asarray(bv, dtype=np.float32)
    wo = np.asarray(wo, dtype=np.float32)
    bo = np.asarray(bo, dtype=np.float32)

    if "nc" not in _CACHE:
        _CACHE["nc"] = build_nc()
    nc = _CACHE["nc"]

    wq_f = wq.transpose(1, 0, 2).reshape(D, H * E)  # [D, heads*E] head-major cols
    wk_f = wk.transpose(1, 0, 2).reshape(D, H * E)
    wv_f = wv.transpose(1, 0, 2).reshape(D, H * E)
    wo_t = wo.T  # [in 1024, out 1024], in-dim = global head-major
    bo_row = np.ascontiguousarray(bo.reshape(1, D))

    in_maps = []
    for c in range(NCORES):
        b, hg = c // 2, c % 2
        cs = slice(hg * HL * E, (hg + 1) * HL * E)
        xT_b = x[b].T
        if hg == 0:
            xT_c = np.ascontiguousarray(xT_b)
        else:
            # own out-tokens first; K/V/Q all share this local token order
            xT_c = np.ascontiguousarray(
                np.concatenate([xT_b[:, TQ:], xT_b[:, :TQ]], axis=1)
            )
        # augmented wo: 4 own head-pair blocks, then (p, g) gathered blocks
        # with the loopback (g == hg) blocks zeroed
        wo_aug = np.zeros((12 * 128, D), dtype=np.float32)
        for p_ in range(4):
            wo_aug[p_ * 128 : (p_ + 1) * 128] = wo_t[
                hg * 512 + p_ * 128 : hg * 512 + (p_ + 1) * 128
            ]
        for p_ in range(4):
            for g_ in range(2):
                if g_ == hg:
                    continue
                t_ = 4 + p_ * 2 + g_
                wo_aug[t_ * 128 : (t_ + 1) * 128] = wo_t[
                    g_ * 512 + p_ * 128 : g_ * 512 + (p_ + 1) * 128
                ]
        def pmajor(a, nt):
            # [nt*128, m] -> [128, nt, m] partition-major contiguous
            return np.ascontiguousarray(
                a.reshape(nt, 128, a.shape[1]).transpose(1, 0, 2)
            )

        m = {
            "xT": pmajor(xT_c, 8).astype(ml_dtypes.bfloat16),
            "wq_t": pmajor(wq_f[:, cs], 8).astype(ml_dtypes.bfloat16),
            "wk_t": pmajor(wk_f[:, cs], 8).astype(ml_dtypes.bfloat16),
            "wv_t": pmajor(wv_f[:, cs], 8).astype(ml_dtypes.bfloat16),
            "wo_b": pmajor(wo_aug, 12).astype(ml_dtypes.bfloat16),
            "bqp": np.ascontiguousarray(
                bq.reshape(H * E)[cs].reshape(4, 128).T
            ),
            "bkp": np.ascontiguousarray(
                bk.reshape(H * E)[cs].reshape(4, 128).T
            ),
            "bv_row": np.ascontiguousarray(bv.reshape(1, H * E)[:, cs]),
            "bo_row": bo_row,
        }
        in_maps.append(m)

    res = run_bass_kernel_spmd(nc, in_maps, list(range(NCORES)), trace=trace)

    out = np.empty((B, S, D), dtype=np.float32)
    for c in range(NCORES):
        b, hg = c // 2, c % 2
        out[b, hg * TQ : (hg + 1) * TQ, :] = res.results[c]["out"]
    if trace:
        return out, res
    return out


# revision 15
# speedup vs baseline: 1.5239x; 1.0166x over previous
"""Multi-head attention (B=4, S=2048, D=1024, H=16, E=64) on 8 TRN2 NeuronCores.

Sharding: core c handles batch b=c//2 and head-group hg=c%2 (8 heads) over the
full 2048-token sequence; after each 2-head pass the peer-token half of the
attention output is exchanged pairwise (AllGather over [2c,2c+1]) and each
core runs the output projection for its own 1024 tokens over all 16 heads
(augmented wo layout with zeroed loopback blocks, as the program is SPMD).

Kernel structure (v2 — uniform 128x128 PE mode, exp/PE balanced pipeline):
  * All matmuls use the full 128-row PE config; the per-head scores matmuls
    (contraction = E = 64) are padded to 128 contraction rows via a
    zero-padded Q layout qt[128, 2, S]: strip 0 holds head A's Q in rows
    0-63 (rows 64-127 zero), strip 1 head B's in rows 64-127.  lhsT is the
    shared kt[:, kt-tile] so K rows of the other head multiply zeros.  This
    avoids the (64,128)<->(128,128) tiling-mode drains the PE would pay when
    interleaving scores and att@V matmuls.
  * Inner loop per (pass, tqt of 512 queries, kt of 128 keys): 2 scores
    matmuls (one per head, adjacent PSUM banks), one exp ACTIVATE over
    [128, 2, 512] (N=1024), 2 att@V accumulation matmuls (M=65: V plus a
    ones column that yields the softmax sums in PSUM row 64).
  * V is computed once (fp32r) and kept resident in SBUF as bf16
    [128 tok-part, 16 tok-tile, 4 head, 65] (two quads) — no DRAM spill.
  * K/Q are stored bf16 (scores matmul in bf16; PSUM accumulates fp32).
  * Softmax normalize: fast PSUM evacuation (sums row + 64 att rows copied
    to SBUF by DVE to free the bank), reciprocal_approx_fast, GpSimd
    partition broadcast, DVE multiply straight into the own-half tile or the
    exchange staging tile.
  * A filler queue interleaves the V/K/Q projection matmuls (and late DMAs)
    into the exp-bound attention loop at ~2 instructions per kt so the PE
    never idles (keeps the tensor-engine DVFS p-state at max clock).
"""

import numpy as np
import ml_dtypes

import concourse.bass as bass
import concourse.mybir as mybir
import concourse.tile as tile
from concourse import bacc
from concourse.bass_utils import run_bass_kernel_spmd

FP32 = mybir.dt.float32
FP32R = mybir.dt.float32r
BF16 = mybir.dt.bfloat16
AF = mybir.ActivationFunctionType

B, S, D, H, E = 4, 2048, 1024, 16, 64
NCORES = 8
TQ = S // 2  # tokens per core for the output projection
HL = H // 2  # local heads per core
SCALE = 1.0 / float(np.sqrt(E))
PAIRS = [[0, 1], [2, 3], [4, 5], [6, 7]]

_CACHE = {}


def build_nc():
    nc = bacc.Bacc("TRN2", target_bir_lowering=False)

    # inputs arrive pre-transposed to the SBUF layout: partition-major so
    # each DMA is 128 large contiguous runs (descriptor-rate matters)
    xT = nc.dram_tensor("xT", [128, 8, S], BF16, kind="ExternalInput")
    wq_t = nc.dram_tensor("wq_t", [128, 8, HL * E], BF16, kind="ExternalInput")
    wk_t = nc.dram_tensor("wk_t", [128, 8, HL * E], BF16, kind="ExternalInput")
    wv_t = nc.dram_tensor("wv_t", [128, 8, HL * E], BF16, kind="ExternalInput")
    wo_b = nc.dram_tensor("wo_b", [128, 12, D], BF16, kind="ExternalInput")
    bqp = nc.dram_tensor("bqp", [128, 4], FP32, kind="ExternalInput")
    bkp = nc.dram_tensor("bkp", [128, 4], FP32, kind="ExternalInput")
    bv_row = nc.dram_tensor("bv_row", [1, HL * E], FP32R, kind="ExternalInput")
    bo_row = nc.dram_tensor("bo_row", [1, D], FP32R, kind="ExternalInput")
    out = nc.dram_tensor("out", [TQ, D], FP32, kind="ExternalOutput")
    wq_r4 = wq_t.rearrange("p k (t m) -> p t k m", m=128)  # [128, 4, 8, 128]
    wk_r4 = wk_t.rearrange("p k (t m) -> p t k m", m=128)
    att_gin = nc.dram_tensor("att_gin", [4, 2, 128, 512], BF16)
    att_gout = nc.dram_tensor("att_gout", [4, 2, 2, 128, 512], BF16)


    from contextlib import ExitStack

    with tile.TileContext(nc) as tc:
        with ExitStack() as _es:
            qt_pool = _es.enter_context(tc.tile_pool(name="qtp", bufs=2))
            kt_pool = _es.enter_context(tc.tile_pool(name="ktp", bufs=2))
            v_pool = _es.enter_context(tc.tile_pool(name="vp", bufs=2))
            w_pool = _es.enter_context(tc.tile_pool(name="wp", bufs=4))
            exp_pool = _es.enter_context(tc.tile_pool(name="expp", bufs=4))
            own_pool = _es.enter_context(tc.tile_pool(name="own", bufs=4))
            gin_pool = _es.enter_context(tc.tile_pool(name="gin", bufs=2))
            recv_pool = _es.enter_context(tc.tile_pool(name="recv", bufs=8))
            stage_pool = _es.enter_context(tc.tile_pool(name="astg", bufs=2))
            ostg_pool = _es.enter_context(tc.tile_pool(name="ostg", bufs=2))
            small_pool = _es.enter_context(tc.tile_pool(name="small", bufs=2))
            rb_pool = _es.enter_context(tc.tile_pool(name="rbp", bufs=2))
            ones_pool = _es.enter_context(tc.tile_pool(name="ones", bufs=1))
            ps_sc = _es.enter_context(tc.tile_pool(name="ps_s", bufs=2, space="PSUM"))
            ps_att = _es.enter_context(tc.tile_pool(name="ps_a", bufs=2, space="PSUM"))
            ps_gen = _es.enter_context(tc.tile_pool(name="ps_g", bufs=2, space="PSUM"))

            # inner scope: released after pass 2 so the staging pool can
            # reuse the 72KB (xt + K/Q weights are dead by then)
            inner_es = ExitStack()
            xt_pool = inner_es.enter_context(tc.tile_pool(name="xt", bufs=1))
            wkq_pool = inner_es.enter_context(tc.tile_pool(name="wkq", bufs=2))

            # ---- persistent tiles ----
            xt_sb = xt_pool.tile([128, 8, S], BF16, tag="xt")  # 32KB/part

            ones_col_f = ones_pool.tile([128, 4], FP32, tag="onescf")
            nc.vector.memset(ones_col_f, 1.0)
            ones_col = ones_pool.tile([128, 4], BF16, tag="onescol")
            nc.vector.tensor_copy(out=ones_col, in_=ones_col_f)
            # tiny dummy exp: preload the ACT exp table during the xT DMA
            exp_warm = ones_pool.tile([1, 4], FP32, tag="expwarm")
            nc.scalar.activation(
                out=exp_warm, in_=ones_col_f[0:1, :], func=AF.Exp, scale=1.0
            )
            bq_sb = ones_pool.tile([128, 4], FP32, tag="bq")
            bk_sb = ones_pool.tile([128, 4], FP32, tag="bk")
            nc.sync.dma_start(out=bq_sb, in_=bqp[:, :])
            nc.sync.dma_start(out=bk_sb, in_=bkp[:, :])

            bv_sb = ones_pool.tile([1, HL * E], FP32R, tag="bvrow")
            bo_sb = ones_pool.tile([1, D], FP32R, tag="borow")
            nc.sync.dma_start(out=bv_sb, in_=bv_row[:, :])
            nc.sync.dma_start(out=bo_sb, in_=bo_row[:, :])
            bv_bc = ones_pool.tile([128, HL * E], FP32R, tag="bvbc")
            bo_bc = ones_pool.tile([128, D], FP32R, tag="bobc")
            nc.gpsimd.partition_broadcast(bv_bc, bv_sb)
            nc.gpsimd.partition_broadcast(bo_bc, bo_sb)

            # V resident in SBUF: quad q holds heads 4q..4q+3:
            # [tok-in-tile(part), tok-tile, head, E+1]
            v_tiles = [
                v_pool.tile([128, 16, 4, E + 1], BF16, tag="vsb", name=f"v{q}")
                for q in range(2)
            ]
            # Q ping-pong across passes: rows 0-63 head A, 64-127 head B
            qt_tiles = [
                qt_pool.tile([128, S], BF16, tag="qt", name=f"qt{i}")
                for i in range(2)
            ]

            # ---- filler queue ----
            fill_q = []

            def step(n=2):
                if len(fill_q) > 120:
                    n += 1
                for _ in range(n):
                    if fill_q:
                        fill_q.pop(0)()

            def flush():
                while fill_q:
                    fill_q.pop(0)()

            done_marks = set()

            def mark(tag):
                def m():
                    done_marks.add(tag)

                return [m]

            def ensure(tag):
                while tag not in done_marks and fill_q:
                    fill_q.pop(0)()

            # ---- projection group emitters (closure lists) ----
            wv_sbs = {}

            def wv_dma(q):
                def go():
                    if "wv" not in wv_sbs:
                        wv_sbs["wv"] = w_pool.tile(
                            [128, 8, HL * E], BF16, tag="wp", name="wv"
                        )
                        nc.sync.dma_start(out=wv_sbs["wv"], in_=wv_t[:, :, :])
                    wv_sbs[q] = wv_sbs["wv"]

                return [go]

            def v_group(q, tokt):
                cell = {}

                def mm(k):
                    if k == 0:
                        cell["ps"] = ps_gen.tile(
                            [128, 256], FP32, tag="gen", name=f"vps{q}_{tokt}"
                        )
                    nc.tensor.matmul(
                        out=cell["ps"],
                        lhsT=xt_sb[:, k, tokt * 128 : (tokt + 1) * 128],
                        rhs=wv_sbs[q][:, k, q * 256 : (q + 1) * 256],
                        start=(k == 0),
                        stop=(k == 7),
                    )

                def fin():
                    ps = cell["ps"]
                    nc.vector.tensor_add(
                        out=v_tiles[q][:, tokt, :, :E],
                        in0=ps.rearrange("p (h e) -> p h e", e=E),
                        in1=bv_bc[:, q * 256 : (q + 1) * 256].rearrange(
                            "p (h e) -> p h e", e=E
                        ),
                    )
                    nc.vector.tensor_copy(
                        out=v_tiles[q][:, tokt, :, E : E + 1],
                        in_=ones_col[:, :4].unsqueeze(2),
                    )

                return (
                    [lambda k=k: mm(k) for k in range(8)]
                    + [fin]
                    + mark((f"v{q}", tokt))
                )

            wkq_sbs = {}

            def wkq_dma(p):
                def go():
                    wk_sb = wkq_pool.tile(
                        [128, 8, 128], BF16, tag="wk", name=f"wk{p}"
                    )
                    wq_sb = wkq_pool.tile(
                        [128, 8, 128], BF16, tag="wq", name=f"wq{p}"
                    )
                    nc.sync.dma_start(
                        out=wk_sb, in_=wk_r4[:, p, :, :]
                    )
                    nc.sync.dma_start(
                        out=wq_sb, in_=wq_r4[:, p, :, :]
                    )
                    wkq_sbs[p] = (wk_sb, wq_sb)

                return [go]

            kt_sbs = {}

            def kt_alloc(p):
                def go():
                    kt_sbs[p] = kt_pool.tile([128, S], BF16, tag="kt", name=f"kt{p}")

                return [go]

            def k_group(p, ts):
                cell = {}

                def mm(k):
                    if k == 0:
                        cell["ps"] = ps_gen.tile(
                            [128, 512], FP32, tag="gen", name=f"kps{p}_{ts}"
                        )
                    nc.tensor.matmul(
                        out=cell["ps"],
                        lhsT=wkq_sbs[p][0][:, k, :],
                        rhs=xt_sb[:, k, ts * 512 : (ts + 1) * 512],
                        start=(k == 0),
                        stop=(k == 7),
                    )

                def fin():
                    nc.vector.tensor_scalar_add(
                        out=kt_sbs[p][:, ts * 512 : (ts + 1) * 512],
                        in0=cell["ps"],
                        scalar1=bk_sb[:, p : p + 1],
                    )

                return (
                    [lambda k=k: mm(k) for k in range(8)]
                    + [fin]
                    + mark(("k", p, ts))
                )

            def q_group(p, qs):
                cell = {}
                qt_t = qt_tiles[p % 2]

                def mm(k):
                    if k == 0:
                        cell["ps"] = ps_gen.tile(
                            [128, 512], FP32, tag="gen", name=f"qps{p}_{qs}"
                        )
                    nc.tensor.matmul(
                        out=cell["ps"],
                        lhsT=wkq_sbs[p][1][:, k, :],
                        rhs=xt_sb[:, k, qs * 512 : (qs + 1) * 512],
                        start=(k == 0),
                        stop=(k == 7),
                    )

                def fin():
                    nc.vector.tensor_scalar_add(
                        out=qt_t[:, qs * 512 : (qs + 1) * 512],
                        in0=cell["ps"],
                        scalar1=bq_sb[:, p : p + 1],
                    )

                return (
                    [lambda k=k: mm(k) for k in range(8)]
                    + [fin]
                    + mark(("q", p, qs))
                )

            # ---- upfront: weights first (small DMAs ahead of the 4MB xT),
            # then just enough compute for pass-0 tqt0's first key tiles ----
            for cl in wv_dma(0) + wkq_dma(0):
                cl()
            for k in range(8):
                nc.sync.dma_start(out=xt_sb[:, k, :], in_=xT[:, k, :])
            for cl in (
                kt_alloc(0)
                + [c for ts in range(2) for c in k_group(0, ts)]
                + q_group(0, 0)
                + [c for t in range(4) for c in v_group(0, t)]
            ):
                cl()

            # filler for pass 0: rest of pass-0 K/Q/V, V quad 1, pass-1 K/Q.
            # ensure() marks make consumers wait for these.
            fill_q.extend(
                [c for t in range(4, 6) for c in v_group(0, t)]
                + [c for c in k_group(0, 2)]
                + [c for t in range(6, 8) for c in v_group(0, t)]
                + [c for c in k_group(0, 3)]
                + q_group(0, 1)
                + [c for t in range(8, 16) for c in v_group(0, t)]
                + q_group(0, 2)
                + q_group(0, 3)
                + wv_dma(1)
                + wkq_dma(1)
                + kt_alloc(1)
                + [c for ts in range(4) for c in k_group(1, ts)]
                + [c for qs in range(4) for c in q_group(1, qs)]
                + [c for t in range(16) for c in v_group(1, t)]
            )

            own_tiles = []
            wo_sbs = {}
            recv_tiles = {}

            def wo_dma(oh):
                def go():
                    wo_sbs[oh] = w_pool.tile(
                        [128, 12, 512], BF16, tag="wp", name=f"wo{oh}"
                    )
                    nc.sync.dma_start(
                        out=wo_sbs[oh], in_=wo_b[:, :, oh * 512 : (oh + 1) * 512]
                    )

                return [go]

            def recv_dma(p, g, halves=(0, 1)):
                def go():
                    if (p, g) not in recv_tiles:
                        recv_tiles[(p, g)] = recv_pool.tile(
                            [128, TQ], BF16, tag="recv", name=f"rc{p}{g}"
                        )
                    rt = recv_tiles[(p, g)]
                    for h in halves:
                        nc.sync.dma_start(
                            out=rt[:, h * 512 : (h + 1) * 512],
                            in_=att_gout[p, h, g],
                        )

                return [go]

            # ---- attention passes ----
            def emit_pass(p):
                qt_t = qt_tiles[p % 2]
                kt_t = kt_sbs[p]
                vq = 0 if p < 2 else 1
                vA, vB = 2 * (p % 2), 2 * (p % 2) + 1
                v_t = v_tiles[vq]

                own_t = own_pool.tile([128, TQ], BF16, tag="own", name=f"own{p}")
                own_tiles.append(own_t)
                gin_t = gin_pool.tile([128, TQ], BF16, tag="gin", name=f"gin{p}")

                for tqt in range(4):
                    ensure(("q", p, tqt))
                    att_A = ps_att.tile([E + 1, 512], FP32, tag="att")
                    att_B = ps_att.tile([E + 1, 512], FP32, tag="att")
                    for kt in range(16):
                        ensure(("k", p, kt // 4))
                        ensure((f"v{vq}", kt))
                        ps = ps_sc.tile([128, 2, 512], FP32, tag="sc")
                        # 64-row PE tiles: the two heads run concurrently on
                        # row halves (0,0) and (64,0)
                        nc.tensor.matmul(
                            out=ps[:, 0, :],
                            lhsT=kt_t[0:64, kt * 128 : (kt + 1) * 128],
                            rhs=qt_t[0:64, tqt * 512 : (tqt + 1) * 512],
                            start=True,
                            stop=True,
                        )
                        nc.tensor.matmul(
                            out=ps[:, 1, :],
                            lhsT=kt_t[64:128, kt * 128 : (kt + 1) * 128],
                            rhs=qt_t[64:128, tqt * 512 : (tqt + 1) * 512],
                            start=True,
                            stop=True,
                        )
                        ex = exp_pool.tile([128, 2, 512], BF16, tag="exp")
                        nc.scalar.activation(out=ex, in_=ps, func=AF.Exp, scale=SCALE)
                        nc.tensor.matmul(
                            out=att_A,
                            lhsT=v_t[:, kt, vA, :],
                            rhs=ex[:, 0, :],
                            start=(kt == 0),
                            stop=(kt == 15),
                        )
                        nc.tensor.matmul(
                            out=att_B,
                            lhsT=v_t[:, kt, vB, :],
                            rhs=ex[:, 1, :],
                            start=(kt == 0),
                            stop=(kt == 15),
                        )
                        step(2)

                    for hh, att_ps in ((0, att_A), (1, att_B)):
                        # fast PSUM evacuation, then normalize from SBUF
                        sums = small_pool.tile([1, 512], FP32, tag="sums", bufs=1)
                        nc.vector.tensor_copy(out=sums, in_=att_ps[E : E + 1, :])
                        a_sb = stage_pool.tile([64, 512], FP32, tag="astg")
                        nc.vector.tensor_copy(out=a_sb, in_=att_ps[:E, :])
                        recip = small_pool.tile([1, 512], FP32, tag="recip", bufs=1)
                        nc.vector.reciprocal_approx_fast(out=recip, in_=sums)
                        rb = rb_pool.tile([64, 512], FP32, tag="rbb")
                        nc.gpsimd.partition_broadcast(rb, recip)
                        if tqt < 2:
                            dest = own_t[
                                hh * 64 : (hh + 1) * 64,
                                tqt * 512 : (tqt + 1) * 512,
                            ]
                        else:
                            dest = gin_t[
                                hh * 64 : (hh + 1) * 64,
                                (tqt - 2) * 512 : (tqt - 1) * 512,
                            ]
                        nc.vector.tensor_mul(out=dest, in0=a_sb, in1=rb)

                    # pair-exchange each 512-token half as soon as its
                    # drains land, so the last collective hides in pass 3
                    if tqt >= 2:
                        h = tqt - 2
                        nc.sync.dma_start(
                            out=att_gin[p, h],
                            in_=gin_t[:, h * 512 : (h + 1) * 512],
                        )
                        nc.gpsimd.collective_compute(
                            kind="AllGather",
                            op=mybir.AluOpType.bypass,
                            replica_groups=PAIRS,
                            ins=[att_gin[p, h]],
                            outs=[att_gout[p, h]],
                        )
                        if p == 3:
                            for cl in recv_dma(3, 0, (h,)) + recv_dma(
                                3, 1, (h,)
                            ):
                                cl()

            def lhs_of(t):
                if t < 4:
                    return own_tiles[t]
                return recv_tiles[((t - 4) // 2, (t - 4) % 2)]

            # out-proj partials: blocks available before pass 3 ends
            # (own 0-2 + all of recv pass 0-2); finals add own3 + recv3.
            PARTIAL_BLOCKS = [0, 1, 2, 4, 5, 6, 7, 8, 9]
            FINAL_BLOCKS = [3, 10, 11]
            stg_tiles = {}

            def partial_group(oh, tokt):
                cell = {}

                def mm(i):
                    if i == 0:
                        cell["ps"] = ps_gen.tile(
                            [128, 512], FP32, tag="gen", name=f"pps{oh}_{tokt}"
                        )
                    t = PARTIAL_BLOCKS[i]
                    nc.tensor.matmul(
                        out=cell["ps"],
                        lhsT=lhs_of(t)[:, tokt * 128 : (tokt + 1) * 128],
                        rhs=wo_sbs[oh][:, t, :],
                        start=(i == 0),
                        stop=(i == len(PARTIAL_BLOCKS) - 1),
                    )

                def fin():
                    stg = stg_pool.tile(
                        [128, 512], FP32, tag="stg", name=f"stg{oh}_{tokt}"
                    )
                    nc.vector.tensor_add(
                        out=stg,
                        in0=cell["ps"],
                        in1=bo_bc[:, oh * 512 : (oh + 1) * 512],
                    )
                    stg_tiles[(oh, tokt)] = stg

                return [
                    lambda i=i: mm(i) for i in range(len(PARTIAL_BLOCKS))
                ] + [fin]

            emit_pass(0)
            fill_q.extend(
                wkq_dma(2)
                + kt_alloc(2)
                + [c for ts in range(4) for c in k_group(2, ts)]
                + [c for qs in range(4) for c in q_group(2, qs)]
            )
            emit_pass(1)
            fill_q.extend(
                wkq_dma(3)
                + kt_alloc(3)
                + [c for ts in range(4) for c in k_group(3, ts)]
                + [c for qs in range(4) for c in q_group(3, qs)]
                + recv_dma(0, 0)
                + recv_dma(0, 1)
                + [c for oh in range(2) for c in wo_dma(oh)]
            )
            emit_pass(2)
            fill_q.extend(recv_dma(1, 0) + recv_dma(1, 1))
            flush()
            # xT and K/Q weights are dead: release their 72KB for staging
            inner_es.close()
            stg_pool = _es.enter_context(tc.tile_pool(name="stg", bufs=16))

            fill_q.extend(
                recv_dma(2, 0)
                + recv_dma(2, 1)
                + [
                    c
                    for oh in range(2)
                    for tokt in range(8)
                    for c in partial_group(oh, tokt)
                ]
            )
            emit_pass(3)
            flush()

            # ---- finals: own3 + recv3 blocks on top of the staged partials
            # tokt 0-3 depend only on the half-0 exchange (lands mid-pass-3)
            for tokt, oh in [(t, o) for t in range(8) for o in range(2)]:
                if True:
                    ps = ps_gen.tile([128, 512], FP32, tag="gen")
                    for i, t in enumerate(FINAL_BLOCKS):
                        nc.tensor.matmul(
                            out=ps,
                            lhsT=lhs_of(t)[:, tokt * 128 : (tokt + 1) * 128],
                            rhs=wo_sbs[oh][:, t, :],
                            start=(i == 0),
                            stop=(i == len(FINAL_BLOCKS) - 1),
                        )
                    ostg = ostg_pool.tile([128, 512], FP32, tag="ostg")
                    nc.vector.tensor_add(
                        out=ostg, in0=ps, in1=stg_tiles[(oh, tokt)]
                    )
                    nc.sync.dma_start(
                        out=out[
                            tokt * 128 : (tokt + 1) * 128,
                            oh * 512 : (oh + 1) * 512,
                        ],
                        in_=ostg,
                    )

    nc.compile()
    return nc


def kernel(x, wq, bq, wk, bk, wv, bv, wo, bo, trace=False):
    x = np.asarray(x, dtype=np.float32)
    wq = np.asarray(wq, dtype=np.float32)
    bq = np.asarray(bq, dtype=np.float32)
    wk = np.asarray(wk, dtype=np.float32)
    bk = np.asarray(bk, dtype=np.float32)
    wv = np.asarray(wv, dtype=np.float32)
    bv = np.asarray(bv, dtype=np.float32)
    wo = np.asarray(wo, dtype=np.float32)
    bo = np.asarray(bo, dtype=np.float32)

    if "nc" not in _CACHE:
        _CACHE["nc"] = build_nc()
    nc = _CACHE["nc"]

    wq_f = wq.transpose(1, 0, 2).reshape(D, H * E)  # [D, heads*E] head-major cols
    wk_f = wk.transpose(1, 0, 2).reshape(D, H * E)
    wv_f = wv.transpose(1, 0, 2).reshape(D, H * E)
    wo_t = wo.T  # [in 1024, out 1024], in-dim = global head-major
    bo_row = np.ascontiguousarray(bo.reshape(1, D))

    in_maps = []
    for c in range(NCORES):
        b, hg = c // 2, c % 2
        cs = slice(hg * HL * E, (hg + 1) * HL * E)
        xT_b = x[b].T
        if hg == 0:
            xT_c = np.ascontiguousarray(xT_b)
        else:
            # own out-tokens first; K/V/Q all share this local token order
            xT_c = np.ascontiguousarray(
                np.concatenate([xT_b[:, TQ:], xT_b[:, :TQ]], axis=1)
            )
        # augmented wo: 4 own head-pair blocks, then (p, g) gathered blocks
        # with the loopback (g == hg) blocks zeroed
        wo_aug = np.zeros((12 * 128, D), dtype=np.float32)
        for p_ in range(4):
            wo_aug[p_ * 128 : (p_ + 1) * 128] = wo_t[
                hg * 512 + p_ * 128 : hg * 512 + (p_ + 1) * 128
            ]
        for p_ in range(4):
            for g_ in range(2):
                if g_ == hg:
                    continue
                t_ = 4 + p_ * 2 + g_
                wo_aug[t_ * 128 : (t_ + 1) * 128] = wo_t[
                    g_ * 512 + p_ * 128 : g_ * 512 + (p_ + 1) * 128
                ]
        def pmajor(a, nt):
            # [nt*128, m] -> [128, nt, m] partition-major contiguous
            return np.ascontiguousarray(
                a.reshape(nt, 128, a.shape[1]).transpose(1, 0, 2)
            )

        m = {
            "xT": pmajor(xT_c, 8).astype(ml_dtypes.bfloat16),
            "wq_t": pmajor(wq_f[:, cs], 8).astype(ml_dtypes.bfloat16),
            "wk_t": pmajor(wk_f[:, cs], 8).astype(ml_dtypes.bfloat16),
            "wv_t": pmajor(wv_f[:, cs], 8).astype(ml_dtypes.bfloat16),
            "wo_b": pmajor(wo_aug, 12).astype(ml_dtypes.bfloat16),
            "bqp": np.ascontiguousarray(
                bq.reshape(H * E)[cs].reshape(4, 128).T
            ),
            "bkp": np.ascontiguousarray(
                bk.reshape(H * E)[cs].reshape(4, 128).T
            ),
            "bv_row": np.ascontiguousarray(bv.reshape(1, H * E)[:, cs]),
            "bo_row": bo_row,
        }
        in_maps.append(m)

    res = run_bass_kernel_spmd(nc, in_maps, list(range(NCORES)), trace=trace)

    out = np.empty((B, S, D), dtype=np.float32)
    for c in range(NCORES):
        b, hg = c // 2, c % 2
        out[b, hg * TQ : (hg + 1) * TQ, :] = res.results[c]["out"]
    if trace:
        return out, res
    return out


# revision 16
# speedup vs baseline: 1.5408x; 1.0111x over previous
"""Multi-head attention (B=4, S=2048, D=1024, H=16, E=64) on 8 TRN2 NeuronCores.

Sharding: core c handles batch b=c//2 and head-group hg=c%2 (8 heads) over the
full 2048-token sequence; after each 2-head pass the peer-token half of the
attention output is exchanged pairwise (AllGather over [2c,2c+1]) and each
core runs the output projection for its own 1024 tokens over all 16 heads
(augmented wo layout with zeroed loopback blocks, as the program is SPMD).

Kernel structure (v2 — uniform 128x128 PE mode, exp/PE balanced pipeline):
  * All matmuls use the full 128-row PE config; the per-head scores matmuls
    (contraction = E = 64) are padded to 128 contraction rows via a
    zero-padded Q layout qt[128, 2, S]: strip 0 holds head A's Q in rows
    0-63 (rows 64-127 zero), strip 1 head B's in rows 64-127.  lhsT is the
    shared kt[:, kt-tile] so K rows of the other head multiply zeros.  This
    avoids the (64,128)<->(128,128) tiling-mode drains the PE would pay when
    interleaving scores and att@V matmuls.
  * Inner loop per (pass, tqt of 512 queries, kt of 128 keys): 2 scores
    matmuls (one per head, adjacent PSUM banks), one exp ACTIVATE over
    [128, 2, 512] (N=1024), 2 att@V accumulation matmuls (M=65: V plus a
    ones column that yields the softmax sums in PSUM row 64).
  * V is computed once (fp32r) and kept resident in SBUF as bf16
    [128 tok-part, 16 tok-tile, 4 head, 65] (two quads) — no DRAM spill.
  * K/Q are stored bf16 (scores matmul in bf16; PSUM accumulates fp32).
  * Softmax normalize: fast PSUM evacuation (sums row + 64 att rows copied
    to SBUF by DVE to free the bank), reciprocal_approx_fast, GpSimd
    partition broadcast, DVE multiply straight into the own-half tile or the
    exchange staging tile.
  * A filler queue interleaves the V/K/Q projection matmuls (and late DMAs)
    into the exp-bound attention loop at ~2 instructions per kt so the PE
    never idles (keeps the tensor-engine DVFS p-state at max clock).
"""

import numpy as np
import ml_dtypes

import concourse.bass as bass
import concourse.mybir as mybir
import concourse.tile as tile
from concourse import bacc
from concourse.bass_utils import run_bass_kernel_spmd

FP32 = mybir.dt.float32
FP32R = mybir.dt.float32r
BF16 = mybir.dt.bfloat16
AF = mybir.ActivationFunctionType

B, S, D, H, E = 4, 2048, 1024, 16, 64
NCORES = 8
TQ = S // 2  # tokens per core for the output projection
HL = H // 2  # local heads per core
SCALE = 1.0 / float(np.sqrt(E))
PAIRS = [[0, 1], [2, 3], [4, 5], [6, 7]]

_CACHE = {}


def build_nc():
    nc = bacc.Bacc("TRN2", target_bir_lowering=False)

    # inputs arrive pre-transposed to the SBUF layout: partition-major so
    # each DMA is 128 large contiguous runs (descriptor-rate matters)
    xT = nc.dram_tensor("xT", [128, 8, S], BF16, kind="ExternalInput")
    wq_t = nc.dram_tensor("wq_t", [128, 8, HL * E], BF16, kind="ExternalInput")
    wk_t = nc.dram_tensor("wk_t", [128, 8, HL * E], BF16, kind="ExternalInput")
    wv_t = nc.dram_tensor("wv_t", [128, 8, HL * E], BF16, kind="ExternalInput")
    wo_b = nc.dram_tensor("wo_b", [128, 12, D], BF16, kind="ExternalInput")
    bqp = nc.dram_tensor("bqp", [128, 4], FP32, kind="ExternalInput")
    bkp = nc.dram_tensor("bkp", [128, 4], FP32, kind="ExternalInput")
    bv_row = nc.dram_tensor("bv_row", [1, HL * E], FP32R, kind="ExternalInput")
    bo_row = nc.dram_tensor("bo_row", [1, D], FP32R, kind="ExternalInput")
    out = nc.dram_tensor("out", [TQ, D], FP32, kind="ExternalOutput")
    wq_r4 = wq_t.rearrange("p k (t m) -> p t k m", m=128)  # [128, 4, 8, 128]
    wk_r4 = wk_t.rearrange("p k (t m) -> p t k m", m=128)
    att_gin = nc.dram_tensor("att_gin", [4, 2, 128, 512], BF16)
    att_gout = nc.dram_tensor("att_gout", [4, 2, 2, 128, 512], BF16)


    from contextlib import ExitStack

    with tile.TileContext(nc) as tc:
        with ExitStack() as _es:
            qt_pool = _es.enter_context(tc.tile_pool(name="qtp", bufs=2))
            kt_pool = _es.enter_context(tc.tile_pool(name="ktp", bufs=2))
            v_pool = _es.enter_context(tc.tile_pool(name="vp", bufs=2))
            w_pool = _es.enter_context(tc.tile_pool(name="wp", bufs=4))
            exp_pool = _es.enter_context(tc.tile_pool(name="expp", bufs=4))
            own_pool = _es.enter_context(tc.tile_pool(name="own", bufs=4))
            gin_pool = _es.enter_context(tc.tile_pool(name="gin", bufs=2))
            recv_pool = _es.enter_context(tc.tile_pool(name="recv", bufs=8))
            stage_pool = _es.enter_context(tc.tile_pool(name="astg", bufs=2))
            ostg_pool = _es.enter_context(tc.tile_pool(name="ostg", bufs=2))
            small_pool = _es.enter_context(tc.tile_pool(name="small", bufs=2))
            rb_pool = _es.enter_context(tc.tile_pool(name="rbp", bufs=2))
            ones_pool = _es.enter_context(tc.tile_pool(name="ones", bufs=1))
            ps_sc = _es.enter_context(tc.tile_pool(name="ps_s", bufs=2, space="PSUM"))
            ps_att = _es.enter_context(tc.tile_pool(name="ps_a", bufs=2, space="PSUM"))
            ps_gen = _es.enter_context(tc.tile_pool(name="ps_g", bufs=2, space="PSUM"))

            # inner scope: released after pass 2 so the staging pool can
            # reuse the 72KB (xt + K/Q weights are dead by then)
            inner_es = ExitStack()
            xt_pool = inner_es.enter_context(tc.tile_pool(name="xt", bufs=1))
            wkq_pool = inner_es.enter_context(tc.tile_pool(name="wkq", bufs=2))

            # ---- persistent tiles ----
            xt_sb = xt_pool.tile([128, 8, S], BF16, tag="xt")  # 32KB/part

            ones_col_f = ones_pool.tile([128, 4], FP32, tag="onescf")
            nc.vector.memset(ones_col_f, 1.0)
            ones_col = ones_pool.tile([128, 4], BF16, tag="onescol")
            nc.vector.tensor_copy(out=ones_col, in_=ones_col_f)
            # tiny dummy exp: preload the ACT exp table during the xT DMA
            exp_warm = ones_pool.tile([1, 4], FP32, tag="expwarm")
            nc.scalar.activation(
                out=exp_warm, in_=ones_col_f[0:1, :], func=AF.Exp, scale=1.0
            )
            bq_sb = ones_pool.tile([128, 4], FP32, tag="bq")
            bk_sb = ones_pool.tile([128, 4], FP32, tag="bk")
            nc.sync.dma_start(out=bq_sb, in_=bqp[:, :])
            nc.sync.dma_start(out=bk_sb, in_=bkp[:, :])

            bv_sb = ones_pool.tile([1, HL * E], FP32R, tag="bvrow")
            bo_sb = ones_pool.tile([1, D], FP32R, tag="borow")
            nc.sync.dma_start(out=bv_sb, in_=bv_row[:, :])
            nc.sync.dma_start(out=bo_sb, in_=bo_row[:, :])
            bv_bc = ones_pool.tile([128, HL * E], FP32R, tag="bvbc")
            bo_bc = ones_pool.tile([128, D], FP32R, tag="bobc")
            nc.gpsimd.partition_broadcast(bv_bc, bv_sb)
            nc.gpsimd.partition_broadcast(bo_bc, bo_sb)

            # V resident in SBUF: quad q holds heads 4q..4q+3:
            # [tok-in-tile(part), tok-tile, head, E+1]
            v_tiles = [
                v_pool.tile([128, 16, 4, E + 1], BF16, tag="vsb", name=f"v{q}")
                for q in range(2)
            ]
            # Q ping-pong across passes: rows 0-63 head A, 64-127 head B
            qt_tiles = [
                qt_pool.tile([128, S], BF16, tag="qt", name=f"qt{i}")
                for i in range(2)
            ]

            # ---- filler queue ----
            fill_q = []

            def step(n=2):
                if len(fill_q) > 120:
                    n += 1
                for _ in range(n):
                    if fill_q:
                        fill_q.pop(0)()

            def flush():
                while fill_q:
                    fill_q.pop(0)()

            done_marks = set()

            def mark(tag):
                def m():
                    done_marks.add(tag)

                return [m]

            def ensure(tag):
                while tag not in done_marks and fill_q:
                    fill_q.pop(0)()

            # ---- projection group emitters (closure lists) ----
            wv_sbs = {}

            def wv_dma(q):
                def go():
                    if "wv" not in wv_sbs:
                        wv_sbs["wv"] = w_pool.tile(
                            [128, 8, HL * E], BF16, tag="wp", name="wv"
                        )
                        nc.sync.dma_start(out=wv_sbs["wv"], in_=wv_t[:, :, :])
                    wv_sbs[q] = wv_sbs["wv"]

                return [go]

            def v_group(q, tokt):
                cell = {}

                def mm(k):
                    if k == 0:
                        cell["ps"] = ps_gen.tile(
                            [128, 256], FP32, tag="gen", name=f"vps{q}_{tokt}"
                        )
                    nc.tensor.matmul(
                        out=cell["ps"],
                        lhsT=xt_sb[:, k, tokt * 128 : (tokt + 1) * 128],
                        rhs=wv_sbs[q][:, k, q * 256 : (q + 1) * 256],
                        start=(k == 0),
                        stop=(k == 7),
                    )

                def fin():
                    ps = cell["ps"]
                    nc.vector.tensor_add(
                        out=v_tiles[q][:, tokt, :, :E],
                        in0=ps.rearrange("p (h e) -> p h e", e=E),
                        in1=bv_bc[:, q * 256 : (q + 1) * 256].rearrange(
                            "p (h e) -> p h e", e=E
                        ),
                    )
                    nc.vector.tensor_copy(
                        out=v_tiles[q][:, tokt, :, E : E + 1],
                        in_=ones_col[:, :4].unsqueeze(2),
                    )

                return (
                    [lambda k=k: mm(k) for k in range(8)]
                    + [fin]
                    + mark((f"v{q}", tokt))
                )

            wkq_sbs = {}

            def wkq_dma(p):
                def go():
                    wk_sb = wkq_pool.tile(
                        [128, 8, 128], BF16, tag="wk", name=f"wk{p}"
                    )
                    wq_sb = wkq_pool.tile(
                        [128, 8, 128], BF16, tag="wq", name=f"wq{p}"
                    )
                    nc.sync.dma_start(
                        out=wk_sb, in_=wk_r4[:, p, :, :]
                    )
                    nc.sync.dma_start(
                        out=wq_sb, in_=wq_r4[:, p, :, :]
                    )
                    wkq_sbs[p] = (wk_sb, wq_sb)

                return [go]

            kt_sbs = {}

            def kt_alloc(p):
                def go():
                    kt_sbs[p] = kt_pool.tile([128, S], BF16, tag="kt", name=f"kt{p}")

                return [go]

            def k_group(p, ts):
                cell = {}

                def mm(k):
                    if k == 0:
                        cell["ps"] = ps_gen.tile(
                            [128, 512], FP32, tag="gen", name=f"kps{p}_{ts}"
                        )
                    nc.tensor.matmul(
                        out=cell["ps"],
                        lhsT=wkq_sbs[p][0][:, k, :],
                        rhs=xt_sb[:, k, ts * 512 : (ts + 1) * 512],
                        start=(k == 0),
                        stop=(k == 7),
                    )

                def fin():
                    nc.vector.tensor_scalar_add(
                        out=kt_sbs[p][:, ts * 512 : (ts + 1) * 512],
                        in0=cell["ps"],
                        scalar1=bk_sb[:, p : p + 1],
                    )

                return (
                    [lambda k=k: mm(k) for k in range(8)]
                    + [fin]
                    + mark(("k", p, ts))
                )

            def q_group(p, qs):
                cell = {}
                qt_t = qt_tiles[p % 2]

                def mm(k):
                    if k == 0:
                        cell["ps"] = ps_gen.tile(
                            [128, 512], FP32, tag="gen", name=f"qps{p}_{qs}"
                        )
                    nc.tensor.matmul(
                        out=cell["ps"],
                        lhsT=wkq_sbs[p][1][:, k, :],
                        rhs=xt_sb[:, k, qs * 512 : (qs + 1) * 512],
                        start=(k == 0),
                        stop=(k == 7),
                    )

                def fin():
                    nc.vector.tensor_scalar_add(
                        out=qt_t[:, qs * 512 : (qs + 1) * 512],
                        in0=cell["ps"],
                        scalar1=bq_sb[:, p : p + 1],
                    )

                return (
                    [lambda k=k: mm(k) for k in range(8)]
                    + [fin]
                    + mark(("q", p, qs))
                )

            # ---- upfront: weights first (small DMAs ahead of the 4MB xT),
            # then just enough compute for pass-0 tqt0's first key tiles ----
            for cl in wv_dma(0) + wkq_dma(0):
                cl()
            nc.sync.dma_start(out=xt_sb[:, 0:4, :], in_=xT[:, 0:4, :])
            nc.sync.dma_start(out=xt_sb[:, 4:8, :], in_=xT[:, 4:8, :])
            for cl in (
                kt_alloc(0)
                + [c for ts in range(2) for c in k_group(0, ts)]
                + q_group(0, 0)
                + [c for t in range(4) for c in v_group(0, t)]
            ):
                cl()

            # filler for pass 0: rest of pass-0 K/Q/V, V quad 1, pass-1 K/Q.
            # ensure() marks make consumers wait for these.
            fill_q.extend(
                [c for t in range(4, 6) for c in v_group(0, t)]
                + [c for c in k_group(0, 2)]
                + [c for t in range(6, 8) for c in v_group(0, t)]
                + [c for c in k_group(0, 3)]
                + q_group(0, 1)
                + [c for t in range(8, 16) for c in v_group(0, t)]
                + q_group(0, 2)
                + q_group(0, 3)
                + wv_dma(1)
                + wkq_dma(1)
                + kt_alloc(1)
                + [c for ts in range(4) for c in k_group(1, ts)]
                + [c for qs in range(4) for c in q_group(1, qs)]
                + [c for t in range(16) for c in v_group(1, t)]
            )

            own_tiles = []
            wo_sbs = {}
            recv_tiles = {}

            def wo_dma(oh):
                def go():
                    wo_sbs[oh] = w_pool.tile(
                        [128, 12, 512], BF16, tag="wp", name=f"wo{oh}"
                    )
                    nc.sync.dma_start(
                        out=wo_sbs[oh], in_=wo_b[:, :, oh * 512 : (oh + 1) * 512]
                    )

                return [go]

            def recv_dma(p, g, halves=(0, 1)):
                def go():
                    if (p, g) not in recv_tiles:
                        recv_tiles[(p, g)] = recv_pool.tile(
                            [128, TQ], BF16, tag="recv", name=f"rc{p}{g}"
                        )
                    rt = recv_tiles[(p, g)]
                    for h in halves:
                        nc.sync.dma_start(
                            out=rt[:, h * 512 : (h + 1) * 512],
                            in_=att_gout[p, h, g],
                        )

                return [go]

            # ---- attention passes ----
            def emit_pass(p):
                qt_t = qt_tiles[p % 2]
                kt_t = kt_sbs[p]
                vq = 0 if p < 2 else 1
                vA, vB = 2 * (p % 2), 2 * (p % 2) + 1
                v_t = v_tiles[vq]

                own_t = own_pool.tile([128, TQ], BF16, tag="own", name=f"own{p}")
                own_tiles.append(own_t)
                gin_t = gin_pool.tile([128, TQ], BF16, tag="gin", name=f"gin{p}")

                for tqt in range(4):
                    ensure(("q", p, tqt))
                    att_A = ps_att.tile([E + 1, 512], FP32, tag="att")
                    att_B = ps_att.tile([E + 1, 512], FP32, tag="att")
                    for kt in range(16):
                        ensure(("k", p, kt // 4))
                        ensure((f"v{vq}", kt))
                        ps = ps_sc.tile([128, 2, 512], FP32, tag="sc")
                        # 64-row PE tiles: the two heads run concurrently on
                        # row halves (0,0) and (64,0)
                        nc.tensor.matmul(
                            out=ps[:, 0, :],
                            lhsT=kt_t[0:64, kt * 128 : (kt + 1) * 128],
                            rhs=qt_t[0:64, tqt * 512 : (tqt + 1) * 512],
                            start=True,
                            stop=True,
                        )
                        nc.tensor.matmul(
                            out=ps[:, 1, :],
                            lhsT=kt_t[64:128, kt * 128 : (kt + 1) * 128],
                            rhs=qt_t[64:128, tqt * 512 : (tqt + 1) * 512],
                            start=True,
                            stop=True,
                        )
                        ex = exp_pool.tile([128, 2, 512], BF16, tag="exp")
                        nc.scalar.activation(out=ex, in_=ps, func=AF.Exp, scale=SCALE)
                        nc.tensor.matmul(
                            out=att_A,
                            lhsT=v_t[:, kt, vA, :],
                            rhs=ex[:, 0, :],
                            start=(kt == 0),
                            stop=(kt == 15),
                        )
                        nc.tensor.matmul(
                            out=att_B,
                            lhsT=v_t[:, kt, vB, :],
                            rhs=ex[:, 1, :],
                            start=(kt == 0),
                            stop=(kt == 15),
                        )
                        step(2)

                    for hh, att_ps in ((0, att_A), (1, att_B)):
                        # fast PSUM evacuation, then normalize from SBUF
                        sums = small_pool.tile([1, 512], FP32, tag="sums", bufs=2)
                        nc.vector.tensor_copy(out=sums, in_=att_ps[E : E + 1, :])
                        a_sb = stage_pool.tile([64, 512], FP32, tag="astg")
                        nc.vector.tensor_copy(out=a_sb, in_=att_ps[:E, :])
                        recip = small_pool.tile([1, 512], FP32, tag="recip", bufs=2)
                        nc.vector.reciprocal_approx_fast(out=recip, in_=sums)
                        rb = rb_pool.tile([64, 512], FP32, tag="rbb")
                        nc.gpsimd.partition_broadcast(rb, recip)
                        if tqt < 2:
                            dest = own_t[
                                hh * 64 : (hh + 1) * 64,
                                tqt * 512 : (tqt + 1) * 512,
                            ]
                        else:
                            dest = gin_t[
                                hh * 64 : (hh + 1) * 64,
                                (tqt - 2) * 512 : (tqt - 1) * 512,
                            ]
                        nc.vector.tensor_mul(out=dest, in0=a_sb, in1=rb)

                    # pair-exchange each 512-token half as soon as its
                    # drains land, so the last collective hides in pass 3
                    if tqt >= 2:
                        h = tqt - 2
                        nc.sync.dma_start(
                            out=att_gin[p, h],
                            in_=gin_t[:, h * 512 : (h + 1) * 512],
                        )
                        nc.gpsimd.collective_compute(
                            kind="AllGather",
                            op=mybir.AluOpType.bypass,
                            replica_groups=PAIRS,
                            ins=[att_gin[p, h]],
                            outs=[att_gout[p, h]],
                        )
                        if p == 3:
                            for cl in recv_dma(3, 0, (h,)) + recv_dma(
                                3, 1, (h,)
                            ):
                                cl()

            def lhs_of(t):
                if t < 4:
                    return own_tiles[t]
                return recv_tiles[((t - 4) // 2, (t - 4) % 2)]

            # out-proj partials: blocks available before pass 3 ends
            # (own 0-2 + all of recv pass 0-2); finals add own3 + recv3.
            PARTIAL_BLOCKS = [0, 1, 2, 4, 5, 6, 7, 8, 9]
            FINAL_BLOCKS = [3, 10, 11]
            stg_tiles = {}

            def partial_group(oh, tokt):
                cell = {}

                def mm(i):
                    if i == 0:
                        cell["ps"] = ps_gen.tile(
                            [128, 512], FP32, tag="gen", name=f"pps{oh}_{tokt}"
                        )
                    t = PARTIAL_BLOCKS[i]
                    nc.tensor.matmul(
                        out=cell["ps"],
                        lhsT=lhs_of(t)[:, tokt * 128 : (tokt + 1) * 128],
                        rhs=wo_sbs[oh][:, t, :],
                        start=(i == 0),
                        stop=(i == len(PARTIAL_BLOCKS) - 1),
                    )

                def fin():
                    stg = stg_pool.tile(
                        [128, 512], FP32, tag="stg", name=f"stg{oh}_{tokt}"
                    )
                    nc.vector.tensor_add(
                        out=stg,
                        in0=cell["ps"],
                        in1=bo_bc[:, oh * 512 : (oh + 1) * 512],
                    )
                    stg_tiles[(oh, tokt)] = stg

                return [
                    lambda i=i: mm(i) for i in range(len(PARTIAL_BLOCKS))
                ] + [fin]

            emit_pass(0)
            fill_q.extend(
                wkq_dma(2)
                + kt_alloc(2)
                + [c for ts in range(4) for c in k_group(2, ts)]
                + [c for qs in range(4) for c in q_group(2, qs)]
            )
            emit_pass(1)
            fill_q.extend(
                wkq_dma(3)
                + kt_alloc(3)
                + [c for ts in range(4) for c in k_group(3, ts)]
                + [c for qs in range(4) for c in q_group(3, qs)]
                + recv_dma(0, 0)
                + recv_dma(0, 1)
                + [c for oh in range(2) for c in wo_dma(oh)]
            )
            emit_pass(2)
            fill_q.extend(recv_dma(1, 0) + recv_dma(1, 1))
            flush()
            # xT and K/Q weights are dead: release their 72KB for staging
            inner_es.close()
            stg_pool = _es.enter_context(tc.tile_pool(name="stg", bufs=16))

            fill_q.extend(
                recv_dma(2, 0)
                + recv_dma(2, 1)
                + [
                    c
                    for oh in range(2)
                    for tokt in range(8)
                    for c in partial_group(oh, tokt)
                ]
            )
            emit_pass(3)
            flush()

            # ---- finals: own3 + recv3 blocks on top of the staged partials
            # tokt 0-3 depend only on the half-0 exchange (lands mid-pass-3)
            for tokt, oh in [(t, o) for t in range(8) for o in range(2)]:
                if True:
                    ps = ps_gen.tile([128, 512], FP32, tag="gen")
                    for i, t in enumerate(FINAL_BLOCKS):
                        nc.tensor.matmul(
                            out=ps,
                            lhsT=lhs_of(t)[:, tokt * 128 : (tokt + 1) * 128],
                            rhs=wo_sbs[oh][:, t, :],
                            start=(i == 0),
                            stop=(i == len(FINAL_BLOCKS) - 1),
                        )
                    ostg = ostg_pool.tile([128, 512], FP32, tag="ostg")
                    nc.vector.tensor_add(
                        out=ostg, in0=ps, in1=stg_tiles[(oh, tokt)]
                    )
                    nc.sync.dma_start(
                        out=out[
                            tokt * 128 : (tokt + 1) * 128,
                            oh * 512 : (oh + 1) * 512,
                        ],
                        in_=ostg,
                    )

    nc.compile()
    return nc


def kernel(x, wq, bq, wk, bk, wv, bv, wo, bo, trace=False):
    x = np.asarray(x, dtype=np.float32)
    wq = np.asarray(wq, dtype=np.float32)
    bq = np.asarray(bq, dtype=np.float32)
    wk = np.asarray(wk, dtype=np.float32)
    bk = np.asarray(bk, dtype=np.float32)
    wv = np.asarray(wv, dtype=np.float32)
    bv = np.asarray(bv, dtype=np.float32)
    wo = np.asarray(wo, dtype=np.float32)
    bo = np.asarray(bo, dtype=np.float32)

    if "nc" not in _CACHE:
        _CACHE["nc"] = build_nc()
    nc = _CACHE["nc"]

    wq_f = wq.transpose(1, 0, 2).reshape(D, H * E)  # [D, heads*E] head-major cols
    wk_f = wk.transpose(1, 0, 2).reshape(D, H * E)
    wv_f = wv.transpose(1, 0, 2).reshape(D, H * E)
    wo_t = wo.T  # [in 1024, out 1024], in-dim = global head-major
    bo_row = np.ascontiguousarray(bo.reshape(1, D))

    in_maps = []
    for c in range(NCORES):
        b, hg = c // 2, c % 2
        cs = slice(hg * HL * E, (hg + 1) * HL * E)
        xT_b = x[b].T
        if hg == 0:
            xT_c = np.ascontiguousarray(xT_b)
        else:
            # own out-tokens first; K/V/Q all share this local token order
            xT_c = np.ascontiguousarray(
                np.concatenate([xT_b[:, TQ:], xT_b[:, :TQ]], axis=1)
            )
        # augmented wo: 4 own head-pair blocks, then (p, g) gathered blocks
        # with the loopback (g == hg) blocks zeroed
        wo_aug = np.zeros((12 * 128, D), dtype=np.float32)
        for p_ in range(4):
            wo_aug[p_ * 128 : (p_ + 1) * 128] = wo_t[
                hg * 512 + p_ * 128 : hg * 512 + (p_ + 1) * 128
            ]
        for p_ in range(4):
            for g_ in range(2):
                if g_ == hg:
                    continue
                t_ = 4 + p_ * 2 + g_
                wo_aug[t_ * 128 : (t_ + 1) * 128] = wo_t[
                    g_ * 512 + p_ * 128 : g_ * 512 + (p_ + 1) * 128
                ]
        def pmajor(a, nt):
            # [nt*128, m] -> [128, nt, m] partition-major contiguous
            return np.ascontiguousarray(
                a.reshape(nt, 128, a.shape[1]).transpose(1, 0, 2)
            )

        m = {
            "xT": pmajor(xT_c, 8).astype(ml_dtypes.bfloat16),
            "wq_t": pmajor(wq_f[:, cs], 8).astype(ml_dtypes.bfloat16),
            "wk_t": pmajor(wk_f[:, cs], 8).astype(ml_dtypes.bfloat16),
            "wv_t": pmajor(wv_f[:, cs], 8).astype(ml_dtypes.bfloat16),
            "wo_b": pmajor(wo_aug, 12).astype(ml_dtypes.bfloat16),
            "bqp": np.ascontiguousarray(
                bq.reshape(H * E)[cs].reshape(4, 128).T
            ),
            "bkp": np.ascontiguousarray(
                bk.reshape(H * E)[cs].reshape(4, 128).T
            ),
            "bv_row": np.ascontiguousarray(bv.reshape(1, H * E)[:, cs]),
            "bo_row": bo_row,
        }
        in_maps.append(m)

    res = run_bass_kernel_spmd(nc, in_maps, list(range(NCORES)), trace=trace)

    out = np.empty((B, S, D), dtype=np.float32)
    for c in range(NCORES):
        b, hg = c // 2, c % 2
        out[b, hg * TQ : (hg + 1) * TQ, :] = res.results[c]["out"]
    if trace:
        return out, res
    return out
